# revision 75
# baseline (speedup 1.0000x reference)
"""Trainium2 Bass kernel v2 for the ViT transformer block — feature-major dataflow.

Everything on-chip flows feature-major ([feat, tok]); the host pre-transposes
x to xT and the kernel writes token-major output via cheap PE transposes.
LayerNorm statistics come from ones-vector matmuls (partition-dim reduction on
the PE); the per-token scale/shift rows are broadcast across partitions with
K=1 outer-product matmuls into PSUM and applied with two DVE passes.
Attention keeps the scores^T/exp/ones-column layout of v1, but context tiles
leave the attention phase through PE transposes (identity matmul) instead of
serialized DMA-transposes.  x2 returns to token-major through f32 PE
transposes so the fc2 drain and final residual run exactly like v1.

Sharding: data-parallel over batch, 8 samples per core on 8 cores.
"""

import sys
import os

sys.path.insert(0, "/opt/trn_rl_repo")

import numpy as np
import ml_dtypes

import concourse.bass as bass
import concourse.tile as tile
from concourse import mybir
from concourse import masks
from concourse.vector_clock import ScopedClock
from concourse.bass_utils import run_bass_kernel_spmd

F32 = mybir.dt.float32
BF16 = mybir.dt.bfloat16
AF = mybir.ActivationFunctionType
ALU = mybir.AluOpType

B, N_TOK, D = 64, 197, 1024
NCORES = 8
BL = B // NCORES            # samples per core = 8
T = BL * N_TOK              # tokens per core = 1576
NH, HD = 16, 64
HID = 4096
SCALE = HD ** -0.5
WH = WW = 14
NUM_REL = (2 * WH - 1) * (2 * WW - 1) + 3
LN_EPS = 1e-5

IC = 8                       # in-feature chunks of 128
CHUNKS = [(i * 512, min(512, T - i * 512)) for i in range((T + 511) // 512)]
ECHUNKS = [(i * 394, 394) for i in range(4)]   # uniform fc1/fc2 chunks
KTP = [128, N_TOK - 128]     # per-sample key tile sizes [128, 69]
NT = (T + 127) // 128        # 13 token tiles
LASTP = T - 128 * (NT - 1)   # 40


def _tok_tiles():
    return [(t * 128, 128 if t < NT - 1 else LASTP) for t in range(NT)]


def _sample_tiles():
    out = []
    for b in range(BL):
        for kt in range(2):
            out.append((b, kt, b * N_TOK + kt * 128, KTP[kt]))
    return out


def _make_rel_pos_index():
    coords = np.stack(np.meshgrid(np.arange(WH), np.arange(WW), indexing="ij"))
    flat = coords.reshape(2, -1)
    rel = flat[:, :, None] - flat[:, None, :]
    rel = rel.transpose(1, 2, 0).copy()
    rel[:, :, 0] += WH - 1
    rel[:, :, 1] += WW - 1
    rel[:, :, 0] *= 2 * WW - 1
    idx = np.zeros((N_TOK, N_TOK), dtype=np.int32)
    idx[1:, 1:] = rel.sum(-1)
    idx[0, 0:] = NUM_REL - 3
    idx[0:, 0] = NUM_REL - 2
    idx[0, 0] = NUM_REL - 1
    return idx


class SplitDrainTileContext(tile.TileContext):
    """Walrus in this toolchain rejects >1 sync-wait on the kernel-tail
    Drain; split the waits across a chain of drain instructions."""

    def _drain_and_barrier(self, tick_clock, wait_clock):
        drain_inst = self.nc.sync.drain()
        wait_clock.add_sem_waits(
            drain_inst.ins, ScopedClock({None: tick_clock.global_clock})
        )
        si = drain_inst.ins.sync_info
        waits = list(si.on_wait) if si and si.on_wait else []
        if len(waits) > 1:
            si.on_wait = waits[:1]
            for w in waits[1:]:
                d2 = self.nc.sync.drain()
                si2 = d2.ins.sync_info
                if si2 is None:
                    d2.ins.sync_info = mybir.SyncInfo(on_wait=[w], on_update=[])
                else:
                    si2.on_wait = [w]
        self.nc.all_engine_barrier()
        assert self.sems is not None
        popped = self.nc._tile_sem_poison_stack.pop()
        assert popped is self._sem_poison
        self.nc.clear_and_free_semaphores(list(self.sems.allocated().values()))
        self.nc.all_engine_barrier()


def _split_sync_waits(nc, cap=1):
    """Hoist excess sync-waits onto standalone event-semaphore instructions."""
    n = 0
    for fn in nc.m.functions:
        for bb in fn.blocks:
            insts = bb.instructions
            i = 0
            while i < len(insts):
                inst = insts[i]
                si = inst.sync_info
                waits = list(si.on_wait) if si and si.on_wait else []
                if len(waits) > cap and inst.engine != mybir.EngineType.Unassigned:
                    excess = waits[:len(waits) - cap]
                    si.on_wait = waits[len(waits) - cap:]
                    for w in excess:
                        ev = mybir.InstEventSemaphore(
                            name=f"waitsplit_{n}", ins=[], outs=[],
                            sync_info=mybir.SyncInfo(on_wait=[w], on_update=[]))
                        ev.engine = inst.engine
                        nc.register_instruction(ev)
                        insts.insert(i, ev)
                        n += 1
                        i += 1
                i += 1
    return n


def build_program():
    STOP = int(os.environ.get("K2_STOP", "7"))
    ASUB = os.environ.get("K2_ATTN_SUB", "full")
    NB = int(os.environ.get("K2_NB", str(BL)))
    NOEXP = os.environ.get("K2_NOEXP", "0") == "1"
    NOADD = os.environ.get("K2_NOADD", "0") == "1"
    NG = int(os.environ.get("K2_NG", "8"))
    NGI = int(os.environ.get("K2_NGI", "2"))
    nc = bass.Bass("TRN2", target_bir_lowering=False, debug=False,
                   num_devices=NCORES)

    # ---- DRAM I/O ----
    xT_h = nc.declare_dram_parameter("xT", [D, T], F32, isOutput=False)
    xbT_h = nc.declare_dram_parameter("xbT", [D, T], BF16, isOutput=False)
    qkvwT_h = nc.declare_dram_parameter("qkvwT", [D, 3 * D], BF16, isOutput=False)
    qkb_h = nc.declare_dram_parameter("qkb", [128, 16], F32, isOutput=False)
    vb_h = nc.declare_dram_parameter("vb_rep", [128, D], BF16, isOutput=False)
    expbT_h = nc.declare_dram_parameter("expbT", [N_TOK, 2, 8, N_TOK], BF16,
                                        isOutput=False)
    csel_h = nc.declare_dram_parameter("csel", [128, NH, NH], BF16,
                                       isOutput=False)
    psel_h = nc.declare_dram_parameter("psel", [16, IC, 128], BF16,
                                       isOutput=False)
    projwT_h = nc.declare_dram_parameter("projwT", [D, D], BF16, isOutput=False)
    projb_h = nc.declare_dram_parameter("projb", [128, 8], F32, isOutput=False)
    fc1wT_h = nc.declare_dram_parameter("fc1wT", [32, 128, D], BF16, isOutput=False)
    fc1b_h = nc.declare_dram_parameter("fc1b", [128, 32], F32, isOutput=False)
    fc2wp_h = nc.declare_dram_parameter("fc2wp", [IC, 128, 32, 128], BF16,
                                        isOutput=False)
    fc2b_h = nc.declare_dram_parameter("fc2b", [128, 8], F32, isOutput=False)
    out_h = nc.declare_dram_parameter("out", [T, D], F32, isOutput=True)
    x2s_h = nc.dram_tensor("x2s", [D, T], F32)   # x2 + fc2_b, feature-major

    tok_tiles = _tok_tiles()
    samp_tiles = _sample_tiles()

    with SplitDrainTileContext(nc) as tc:
        # ---------- right-side stack: consts > {ctxT | gT} ----------
        consts_cm = tc.tile_pool(name="consts", bufs=1, side="right")
        consts = consts_cm.__enter__()
        identb = consts.tile([128, 128], BF16, tag="identb", name="identb")
        masks.make_identity(nc, identb[:, :])
        ones_col = consts.tile([128, 1], BF16, tag="ones_col", name="ones_col")
        nc.vector.memset(ones_col, 1.0)
        ones_row = consts.tile([1, 128], BF16, tag="ones_row", name="ones_row")
        nc.vector.memset(ones_row, 1.0)
        qkb_t = consts.tile([128, 16], F32, tag="qkb", name="qkb")
        nc.sync.dma_start(out=qkb_t, in_=qkb_h[:, :])
        vb_t = consts.tile([128, D], BF16, tag="vb", name="vb")
        nc.sync.dma_start(out=vb_t, in_=vb_h[:, :])
        projb_t = consts.tile([128, 8], F32, tag="projb", name="projb")
        nc.sync.dma_start(out=projb_t, in_=projb_h[:, :])
        fc2b_t = consts.tile([128, 8], F32, tag="fc2b", name="fc2b")
        nc.sync.dma_start(out=fc2b_t, in_=fc2b_h[:, :])
        fc1b_t = consts.tile([128, 32], F32, tag="fc1b", name="fc1b")
        nc.sync.dma_start(out=fc1b_t, in_=fc1b_h[:, :])
        epsr = consts.tile([1, 1], F32, tag="epsr", name="epsr")
        nc.vector.memset(epsr, LN_EPS)

        def alloc_rows(pool, sfx):
            return dict(
                r0=pool.tile([1, T], F32, tag=f"row0{sfx}", name=f"row0{sfx}"),
                r1=pool.tile([1, T], F32, tag=f"row1{sfx}", name=f"row1{sfx}"),
                r2=pool.tile([1, T], F32, tag=f"row2{sfx}", name=f"row2{sfx}"),
                abf=pool.tile([1, T], BF16, tag=f"ra{sfx}", name=f"ra{sfx}"),
                bbf=pool.tile([1, T], BF16, tag=f"rb{sfx}", name=f"rb{sfx}"))

        def emit_ln_rows(rows, s1_ap_chunks, s2_ap_chunks):
            """a=rsqrt(var+eps) -> abf; b=mu*a -> bbf (normalize SUBTRACTS b).
            Chunk-pipelined so downstream bc/normalize of chunk 0 starts
            early.  r0..r2 scratch; sources may alias r0/r1."""
            r0, r1, r2 = rows['r0'], rows['r1'], rows['r2']
            abf, bbf = rows['abf'], rows['bbf']
            for ci, (c0, cw) in enumerate(CHUNKS):
                sl = slice(c0, c0 + cw)
                nc.scalar.activation(out=r2[0:1, sl], in_=s1_ap_chunks[ci],
                                     func=AF.Copy, scale=1.0 / D)   # mu
                nc.scalar.activation(out=r0[0:1, sl], in_=s2_ap_chunks[ci],
                                     func=AF.Copy, scale=1.0 / D,
                                     bias=float(LN_EPS))            # m2 + eps
                nc.vector.tensor_mul(out=r1[0:1, sl], in0=r2[0:1, sl],
                                     in1=r2[0:1, sl])
                nc.vector.tensor_sub(out=r0[0:1, sl], in0=r0[0:1, sl],
                                     in1=r1[0:1, sl])               # var + eps
                nc.vector.reciprocal_approx_fast(out=r1[0:1, sl],
                                                 in_=r0[0:1, sl])
                nc.scalar.activation(out=abf[0:1, sl], in_=r1[0:1, sl],
                                     func=AF.Sqrt)                  # rsqrt
                nc.vector.tensor_mul(out=bbf[0:1, sl], in0=r2[0:1, sl],
                                     in1=abf[0:1, sl])

        # ---------- left-side stack: attention superstructure ----------
        qk_cm = tc.tile_pool(name="qkp", bufs=1)
        qkp = qk_cm.__enter__()
        qkt = [qkp.tile([128, T], BF16, tag=f"qkt{ft}", name=f"qkt{ft}")
               for ft in range(16)]
        vt_cm = tc.tile_pool(name="vtp", bufs=1)
        vtp = vt_cm.__enter__()
        vt = {}
        for (b, kt, t0, kp) in samp_tiles:
            vt[(b, kt)] = vtp.tile([128, NH, 65], BF16, tag=f"v{b}_{kt}",
                                   name=f"v{b}_{kt}")
        h1T_cm = tc.tile_pool(name="h1Tp", bufs=1)
        h1Tp = h1T_cm.__enter__()
        h1T = h1Tp.tile([128, IC, T], BF16, tag="h1T", name="h1T")
        wv_cm = tc.tile_pool(name="wvp", bufs=1)
        wvp = wv_cm.__enter__()
        wv = [wvp.tile([128, D], BF16, tag=f"wv{c}", name=f"wv{c}")
              for c in range(IC)]
        wqk_cm = tc.tile_pool(name="wqk", bufs=1)
        wqkp = wqk_cm.__enter__()

        # ---------------- Phase A: load xT, LN1 stats ----------------
        ln1_xb_cm = tc.tile_pool(name="ln1_xb", bufs=1)
        xbp = ln1_xb_cm.__enter__()
        ln1_sq_cm = tc.tile_pool(name="ln1_sq", bufs=2)
        sqp = ln1_sq_cm.__enter__()
        ln1_nrm_cm = tc.tile_pool(name="nrm_tmp", bufs=2)
        nrmp = ln1_nrm_cm.__enter__()

        wqs = [wqkp.tile([128, D], BF16, tag=f"wq{c}", name=f"wq{c}")
               for c in range(IC)]

        rows1_cm = tc.tile_pool(name="rows1", bufs=1)
        rows1 = alloc_rows(rows1_cm.__enter__(), "1")
        abf, bbf = rows1['abf'], rows1['bbf']

        xb = []
        with tc.tile_pool(name="st1_ps", bufs=1, space="PSUM") as st1ps:
            # stat rows live at partition 32*ci so the four chunk-MMs hit
            # distinct PE column groups and run concurrently
            s1p = st1ps.tile([128, 4, 512], F32, tag="s1p", name="s1p")
            s2p = st1ps.tile([128, 4, 512], F32, tag="s2p", name="s2p")
            for c in range(IC):
                xb_c = xbp.tile([128, T], BF16, tag=f"xb{c}", name=f"xb{c}")
                for ci, (c0, cw) in enumerate(CHUNKS):
                    nc.sync.dma_start(out=xb_c[:, c0:c0 + cw],
                                      in_=xbT_h[c * 128:(c + 1) * 128,
                                                c0:c0 + cw])
                xb.append(xb_c)
                for ci, (c0, cw) in enumerate(CHUNKS):
                    rb = 32 * ci
                    xsq = sqp.tile([128, 512], BF16, tag="xsq", name="xsq")
                    nc.vector.tensor_mul(out=xsq[:, :cw],
                                         in0=xb_c[:, c0:c0 + cw],
                                         in1=xb_c[:, c0:c0 + cw])
                    nc.tensor.matmul(s1p[rb:rb + 1, ci, :cw],
                                     lhsT=ones_col[:, 0:1],
                                     rhs=xb_c[:, c0:c0 + cw],
                                     start=(c == 0), stop=(c == IC - 1),
                                     tile_position=(0, rb))
                    nc.tensor.matmul(s2p[rb:rb + 1, ci, :cw],
                                     lhsT=ones_col[:, 0:1],
                                     rhs=xsq[:, :cw],
                                     start=(c == 0), stop=(c == IC - 1),
                                     tile_position=(0, rb))
            for c in range(IC):          # Q weight slabs (after xT on the queue)
                nc.sync.dma_start(out=wqs[c],
                                  in_=qkvwT_h[c * 128:(c + 1) * 128, 0:D])
            for c in range(IC):          # V weight slabs (prefetch)
                nc.sync.dma_start(out=wv[c],
                                  in_=qkvwT_h[c * 128:(c + 1) * 128, 2 * D:3 * D])
            emit_ln_rows(rows1,
                         [s1p[32 * ci:32 * ci + 1, ci, :cw]
                          for ci, (c0, cw) in enumerate(CHUNKS)],
                         [s2p[32 * ci:32 * ci + 1, ci, :cw]
                          for ci, (c0, cw) in enumerate(CHUNKS)])

        # ---------- Phase B: normalize + Q (pipelined), K, V ----------
        with tc.tile_pool(name="bc_ps", bufs=2, space="PSUM") as bcps, \
             tc.tile_pool(name="qk_ps", bufs=4, space="PSUM") as qkps_pool:

            def emit_qk8(ci, base_ft):
                c0, cw = CHUNKS[ci]
                for fi in range(8):
                    ft = base_ft + fi
                    ps = qkps_pool.tile([128, 512], F32, tag="qkps", name="qkps")
                    for c in range(IC):
                        nc.tensor.matmul(
                            ps[:, :cw],
                            lhsT=wqs[c][:, fi * 128:(fi + 1) * 128],
                            rhs=h1T[:, c, c0:c0 + cw],
                            start=(c == 0), stop=(c == IC - 1))
                    if ft % 2 == 0:
                        nc.scalar.activation(out=qkt[ft][:, c0:c0 + cw],
                                             in_=ps[:, :cw], func=AF.Identity,
                                             bias=qkb_t[:, ft:ft + 1])
                    else:
                        nc.vector.tensor_add(
                            out=qkt[ft][:, c0:c0 + cw], in0=ps[:, :cw],
                            in1=qkb_t[:, ft:ft + 1].broadcast_to([128, cw]))

            for ci, (c0, cw) in enumerate(CHUNKS):
                if STOP < 2:
                    break
                bc = bcps.tile([128, 2, 512], F32, tag="bc", name="bc")
                nc.tensor.matmul(bc[:, 0, :cw], lhsT=ones_row[0:1, :],
                                 rhs=abf[0:1, c0:c0 + cw], start=True, stop=True)
                nc.tensor.matmul(bc[:, 1, :cw], lhsT=ones_row[0:1, :],
                                 rhs=bbf[0:1, c0:c0 + cw], start=True, stop=True)
                bcs = nrmp.tile([128, 2, 512], BF16, tag="bcs", name="bcs")
                nc.scalar.activation(out=bcs[:, :, :cw], in_=bc[:, :, :cw],
                                     func=AF.Copy)
                for c in range(IC):
                    tmp = nrmp.tile([128, 512], BF16, tag="ntmp", name="ntmp")
                    nc.vector.tensor_mul(out=tmp[:, :cw], in0=xb[c][:, c0:c0 + cw],
                                         in1=bcs[:, 0, :cw])
                    nc.vector.tensor_sub(out=h1T[:, c, c0:c0 + cw],
                                         in0=tmp[:, :cw], in1=bcs[:, 1, :cw])
                if ci > 0:
                    emit_qk8(ci - 1, 0)
            if STOP >= 2:
                emit_qk8(len(CHUNKS) - 1, 0)

            # K: reload the slab slots (overwrite waits on Q readers)
            if STOP >= 2:
                wks = [wqkp.tile([128, D], BF16, tag=f"wq{c}", name=f"wqk{c}")
                       for c in range(IC)]
                for c in range(IC):
                    nc.sync.dma_start(out=wks[c],
                                      in_=qkvwT_h[c * 128:(c + 1) * 128, D:2 * D])
                wqs = wks

            if STOP >= 2:
                for ci in range(len(CHUNKS)):
                    emit_qk8(ci, 8)

            rows1_cm.__exit__(None, None, None)
            ln1_nrm_cm.__exit__(None, None, None)
            ln1_sq_cm.__exit__(None, None, None)
            ln1_xb_cm.__exit__(None, None, None)
            wqk_cm.__exit__(None, None, None)

        # ---------------- Phase C: attention ----------------
        ctxT_cm = tc.tile_pool(name="ctxTp", bufs=1, side="right")
        ctxTp = ctxT_cm.__enter__()
        ctxT = [ctxTp.tile([128, T], BF16, tag=f"ctxT{k}", name=f"ctxT{k}")
                for k in range(IC)]

        attc_cm = tc.tile_pool(name="attc", bufs=1)
        attc = attc_cm.__enter__()
        eb = []
        for kt in range(2):
            kp = KTP[kt]
            t_ = attc.tile([128, 2, 8, N_TOK], BF16, tag=f"expb{kt}",
                           name=f"expb{kt}")
            nc.sync.dma_start(out=t_[:kp, :, :, :],
                              in_=expbT_h[kt * 128: kt * 128 + kp, :, :, :])
            eb.append(t_)
        # csel[:, h, :]: ones in column h (head-select for denominator MMs)
        csel = attc.tile([128, NH, NH], BF16, tag="csel", name="csel")
        nc.sync.dma_start(out=csel, in_=csel_h[:, :, :])
        # psel[:, c, :]: rec-row pair -> partition halves (rc broadcast)
        psel = attc.tile([16, IC, 128], BF16, tag="psel", name="psel")
        nc.sync.dma_start(out=psel, in_=psel_h[:, :, :])

        with tc.tile_pool(name="p_pool", bufs=4) as ppool, \
             tc.tile_pool(name="praw_pool", bufs=3) as prawp, \
             tc.tile_pool(name="rcb_pool", bufs=2) as rcbp, \
             tc.tile_pool(name="rcs_pool", bufs=3) as rcsp, \
             tc.tile_pool(name="sc_ps", bufs=2, space="PSUM") as scps, \
             tc.tile_pool(name="ctx_ps", bufs=2, space="PSUM") as ctxps, \
             tc.tile_pool(name="s16_ps", bufs=1, space="PSUM") as s16ps, \
             tc.tile_pool(name="rc_ps", bufs=1, space="PSUM") as rcps, \
             tc.tile_pool(name="cproj_ps", bufs=2, space="PSUM") as projps:

            pt = {}

            def emit_ctx(b):
                q0 = b * N_TOK
                # softmax denominators, all 16 heads -> one [16, 197] bank
                s16 = s16ps.tile([16, 256], F32, tag="s16", name="s16")
                n = 0
                for h in range(NH):
                    par, a = h % 2, h // 2
                    for kt in range(2):
                        kp = KTP[kt]
                        nc.tensor.matmul(
                            s16[0:NH, 0:N_TOK],
                            lhsT=csel[:kp, h, :],
                            rhs=pt[(b, kt)][:kp, par, a, :],
                            start=(n == 0), stop=(n == 31))
                        n += 1
                rcb = rcbp.tile([16, 256], F32, tag="rcb", name="rcb")
                nc.vector.reciprocal_approx_fast(out=rcb[0:NH, 0:N_TOK],
                                                 in_=s16[0:NH, 0:N_TOK])
                rcbb = rcbp.tile([16, 256], BF16, tag="rcbb", name="rcbb")
                nc.vector.tensor_copy(out=rcbb[0:NH, 0:N_TOK],
                                      in_=rcb[0:NH, 0:N_TOK])
                for c in range(IC):
                    # heads (2c, 2c+1) column-stacked into one psum bank
                    psc = ctxps.tile([128, 256], F32, tag="ctxps", name="ctxps")
                    for par in range(2):
                        h = 2 * c + par
                        for kt in range(2):
                            kp = KTP[kt]
                            nc.tensor.matmul(
                                psc[par * 64:(par + 1) * 64, 0:N_TOK],
                                lhsT=vt[(b, kt)][:kp, h, 0:64],
                                rhs=pt[(b, kt)][:kp, par, c, :],
                                start=(kt == 0), stop=(kt == 1))
                    rc = rcps.tile([128, 256], F32, tag="rcps2", name="rc")
                    nc.tensor.matmul(rc[0:128, 0:N_TOK],
                                     lhsT=psel[0:NH, c, :],
                                     rhs=rcbb[0:NH, 0:N_TOK],
                                     start=True, stop=True)
                    rcs = rcsp.tile([128, 256], F32, tag="rcs", name="rcs")
                    nc.vector.tensor_copy(out=rcs[:, 0:N_TOK],
                                          in_=rc[:, 0:N_TOK])
                    nc.vector.tensor_mul(out=ctxT[c][:, q0:q0 + N_TOK],
                                         in0=psc[0:128, 0:N_TOK],
                                         in1=rcs[0:128, 0:N_TOK])

            def emit_v(b):
                # V matmuls for sample b: dense K=128 full-array MMs keep the
                # HAM clock gate open during the attention phase.
                for kt in range(2):
                    t0 = b * N_TOK + kt * 128
                    kp = KTP[kt]
                    vtile = vt[(b, kt)]
                    for vc in range(2):
                        ps = projps.tile([128, 512], F32, tag="vps",
                                         name="vps")
                        for c in range(IC):
                            nc.tensor.matmul(
                                ps[:kp, :],
                                lhsT=h1T[:, c, t0:t0 + kp],
                                rhs=wv[c][:, vc * 512:(vc + 1) * 512],
                                start=(c == 0), stop=(c == IC - 1))
                        nc.vector.tensor_add(
                            out=vtile[:kp, vc * 8:(vc + 1) * 8, 0:64],
                            in0=ps[:kp, :].rearrange("p (a d) -> p a d", a=8),
                            in1=vb_t[:kp, vc * 512:(vc + 1) * 512].rearrange(
                                "p (a d) -> p a d", a=8))

            for b in range(BL):
                if STOP < 4:
                    break
                q0 = b * N_TOK
                for kt in range(2):
                    kp = KTP[kt]
                    k0 = q0 + kt * 128
                    ptile = ppool.tile([128, 2, 8, N_TOK], BF16, tag="P",
                                       name="P")
                    pt[(b, kt)] = ptile
                    # pair-tile (par, j) holds heads a=j and a=j+4 (same rb:
                    # mixed lhsT base partitions within one PSUM bank fault).
                    # Emission alternates par so consecutive MMs use opposite
                    # rb and LDWEIGHTS overlaps the in-flight matmul.
                    for j in range(4):
                        pss = [scps.tile([128, 2, 256], F32, tag="scps",
                                         name="scps") for _ in range(2)]
                        for s in range(2):
                            a = j + 4 * s
                            for par in range(2):
                                rb = par * 64
                                nc.tensor.matmul(
                                    pss[par][:kp, s, 0:N_TOK],
                                    lhsT=qkt[8 + a][rb:rb + 64, k0:k0 + kp],
                                    rhs=qkt[a][rb:rb + 64, q0:q0 + N_TOK],
                                    start=True, stop=True)
                        for par in range(2):
                            praw = prawp.tile([128, 2, N_TOK], BF16,
                                              tag="praw", name="praw")
                            nc.scalar.activation(out=praw[:kp, :, :],
                                                 in_=pss[par][:kp, :, 0:N_TOK],
                                                 func=AF.Exp)
                            nc.vector.tensor_mul(
                                out=ptile[:kp, par, j::4, :],
                                in0=praw[:kp, :, :],
                                in1=eb[kt][:kp, par, j::4, :])
                emit_v(b)
                if b > 0:
                    emit_ctx(b - 1)
            if STOP >= 4:
                emit_ctx(BL - 1)

        attc_cm.__exit__(None, None, None)
        wv_cm.__exit__(None, None, None)
        h1T_cm.__exit__(None, None, None)
        vt_cm.__exit__(None, None, None)
        qk_cm.__exit__(None, None, None)

        # ------------- Phase D: proj + residual + LN2 + x2->token-major -------
        fc2w_cm = tc.tile_pool(name="fc2w", bufs=3)
        fc2wsb = fc2w_cm.__enter__()
        h2T_cm = tc.tile_pool(name="h2Tp", bufs=1)
        h2Tp = h2T_cm.__enter__()
        h2T = h2Tp.tile([128, IC, T], BF16, tag="h2T", name="h2T")
        xb2_cm = tc.tile_pool(name="xb2p", bufs=1)
        xb2p = xb2_cm.__enter__()

        rows2_cm = tc.tile_pool(name="rows2", bufs=1)
        rows2 = alloc_rows(rows2_cm.__enter__(), "2")
        r0, r1 = rows2['r0'], rows2['r1']
        abf, bbf = rows2['abf'], rows2['bbf']
        nc.vector.memset(r0[0:1, :], 0.0)
        nc.vector.memset(r1[0:1, :], 0.0)

        xb2 = []

        pw_cm = tc.tile_pool(name="pwp", bufs=1)
        pwp = pw_cm.__enter__()
        pw = [pwp.tile([128, D], BF16, tag=f"pw{c}", name=f"pw{c}")
              for c in range(IC)]
        for c in range(IC):
            nc.sync.dma_start(out=pw[c], in_=projwT_h[c * 128:(c + 1) * 128, :])

        with tc.tile_pool(name="xt2", bufs=2) as xt2p, \
             tc.tile_pool(name="x2p", bufs=2) as x2p, \
             tc.tile_pool(name="sq2p", bufs=2) as sq2p, \
             tc.tile_pool(name="proj_ps", bufs=2, space="PSUM") as projps2, \
             tc.tile_pool(name="st2_ps", bufs=4, space="PSUM") as st2ps:
            for ft in range(IC):
                if STOP < 5:
                    break
                xt2 = xt2p.tile([128, T], F32, tag="xt2", name="xt2")
                nc.gpsimd.dma_start(out=xt2, in_=xT_h[ft * 128:(ft + 1) * 128, :])
                x2T = x2p.tile([128, T], F32, tag="x2T", name="x2T")
                for ci, (c0, cw) in enumerate(CHUNKS):
                    ps = projps2.tile([128, 512], F32, tag="projps",
                                      name="projps")
                    for c in range(IC):
                        nc.tensor.matmul(
                            ps[:, :cw],
                            lhsT=pw[c][:, ft * 128:(ft + 1) * 128],
                            rhs=ctxT[c][:, c0:c0 + cw],
                            start=(c == 0), stop=(c == IC - 1))
                    nc.scalar.activation(out=x2T[:, c0:c0 + cw], in_=ps[:, :cw],
                                         func=AF.Identity,
                                         bias=projb_t[:, ft:ft + 1])
                nc.vector.tensor_add(out=x2T, in0=x2T, in1=xt2)
                xb2_f = xb2p.tile([128, T], BF16, tag=f"xb2_{ft}",
                                  name=f"xb2_{ft}")
                nc.scalar.activation(out=xb2_f, in_=x2T, func=AF.Copy)
                xb2.append(xb2_f)
                xsq2 = sq2p.tile([128, T], BF16, tag="xsq2", name="xsq2")
                nc.gpsimd.tensor_mul(out=xsq2, in0=xb2_f, in1=xb2_f)
                for (src_t, accr) in ((xb2_f, r0), (xsq2, r1)):
                    for ci, (c0, cw) in enumerate(CHUNKS):
                        rb = 32 * ci
                        p1 = st2ps.tile([128, 512], F32, tag="st2", name="st2")
                        nc.tensor.matmul(p1[rb:rb + 1, :cw],
                                         lhsT=ones_col[:, 0:1],
                                         rhs=src_t[:, c0:c0 + cw],
                                         start=True, stop=True,
                                         tile_position=(0, rb))
                        nc.vector.tensor_add(out=accr[0:1, c0:c0 + cw],
                                             in0=accr[0:1, c0:c0 + cw],
                                             in1=p1[rb:rb + 1, :cw])
                # x2 + fc2_b -> feature-major DRAM scratch (read back in fc2)
                x2fb = x2p.tile([128, T], F32, tag="x2fb", name="x2fb")
                nc.scalar.activation(out=x2fb, in_=x2T, func=AF.Identity,
                                     bias=fc2b_t[:, ft:ft + 1])
                nc.sync.dma_start(out=x2s_h[ft * 128:(ft + 1) * 128, :],
                                  in_=x2fb)

        ctxT_cm.__exit__(None, None, None)
        pw_cm.__exit__(None, None, None)

        if STOP >= 5:
            emit_ln_rows(rows2,
                         [r0[0:1, c0:c0 + cw] for (c0, cw) in CHUNKS],
                         [r1[0:1, c0:c0 + cw] for (c0, cw) in CHUNKS])

        with tc.tile_pool(name="bc2_ps", bufs=2, space="PSUM") as bc2ps, \
             tc.tile_pool(name="nrm2", bufs=3) as nrm2p:
            for ci, (c0, cw) in enumerate(CHUNKS):
                if STOP < 5:
                    break
                bc = bc2ps.tile([128, 2, 512], F32, tag="bc2", name="bc2")
                nc.tensor.matmul(bc[:, 0, :cw], lhsT=ones_row[0:1, :],
                                 rhs=abf[0:1, c0:c0 + cw], start=True, stop=True)
                nc.tensor.matmul(bc[:, 1, :cw], lhsT=ones_row[0:1, :],
                                 rhs=bbf[0:1, c0:c0 + cw], start=True, stop=True)
                bcs = nrm2p.tile([128, 2, 512], BF16, tag="bcs2", name="bcs2")
                nc.scalar.activation(out=bcs[:, :, :cw], in_=bc[:, :, :cw],
                                     func=AF.Copy)
                for c in range(IC):
                    tmp = nrm2p.tile([128, 512], BF16, tag="n2tmp", name="n2tmp")
                    nc.vector.tensor_mul(out=tmp[:, :cw],
                                         in0=xb2[c][:, c0:c0 + cw],
                                         in1=bcs[:, 0, :cw])
                    nc.vector.tensor_sub(out=h2T[:, c, c0:c0 + cw],
                                         in0=tmp[:, :cw], in1=bcs[:, 1, :cw])
        rows2_cm.__exit__(None, None, None)
        xb2_cm.__exit__(None, None, None)

        # ---------------- Phase E: MLP ----------------
        gT_cm = tc.tile_pool(name="gT_pool", bufs=1, side="right")
        gTp = gT_cm.__enter__()
        gT = gTp.tile([128, 32, T], BF16, tag="gT", name="gT")
        identf = gTp.tile([128, 128], F32, tag="identf", name="identf")
        masks.make_identity(nc, identf[:, :])
        with tc.tile_pool(name="fc1w", bufs=3) as fc1wp, \
             tc.tile_pool(name="fc1_ps", bufs=4, space="PSUM") as fc1ps:
            for Ht in range(32):
                if STOP < 6:
                    break
                wt = fc1wp.tile([128, D], BF16, tag="fc1w", name="fc1w")
                nc.sync.dma_start(out=wt, in_=fc1wT_h[Ht, :, :])
                for j in range(2):           # super-chunks of 788 = 2x394
                    ps = fc1ps.tile([128, 2, 512], F32, tag="fc1ps",
                                    name="fc1ps")
                    for k in range(2):
                        c0, cw = ECHUNKS[j * 2 + k]
                        for c in range(IC):
                            nc.tensor.matmul(
                                ps[:, k, :cw],
                                lhsT=wt[:, c * 128:(c + 1) * 128],
                                rhs=h2T[:, c, c0:c0 + cw],
                                start=(c == 0), stop=(c == IC - 1))
                    nc.scalar.activation(
                        out=gT[:, Ht, j * 788:(j + 1) * 788],
                        in_=ps[:, :, 0:394],
                        func=AF.Gelu, bias=fc1b_t[:, Ht:Ht + 1])
        h2T_cm.__exit__(None, None, None)

        # ---- fc2 feature-major: out^T[ft, t] = sum_H fc2w^T . gT ----
        with tc.tile_pool(name="xf_sb", bufs=2) as xfp, \
             tc.tile_pool(name="ot_sb", bufs=2) as otp, \
             tc.tile_pool(name="stg_sb", bufs=2) as stgp, \
             tc.tile_pool(name="fc2_ps", bufs=2, space="PSUM") as fc2ps:
            for ft in range(IC):
                if STOP < 7:
                    break
                w2 = fc2wsb.tile([128, 32, 128], BF16, tag="fc2w", name="fc2w")
                nc.sync.dma_start(out=w2, in_=fc2wp_h[ft, :, :, :])
                xf = xfp.tile([128, T], F32, tag="xf", name="xf")
                nc.gpsimd.dma_start(out=xf,
                                    in_=x2s_h[ft * 128:(ft + 1) * 128, :])
                ps = fc2ps.tile([128, 4, 512], F32, tag="eps", name="eps_mm")
                ot = otp.tile([128, T], F32, tag="ot", name="ot")
                for ci, (c0, cw) in enumerate(ECHUNKS):
                    for Hkt in range(32):
                        nc.tensor.matmul(
                            ps[:, ci, :cw],
                            lhsT=w2[:, Hkt, :],
                            rhs=gT[:, Hkt, c0:c0 + cw],
                            start=(Hkt == 0), stop=(Hkt == 31))
                    nc.vector.tensor_add(out=ot[:, c0:c0 + cw],
                                         in0=ps[:, ci, :cw],
                                         in1=xf[:, c0:c0 + cw])
                # transpose to token-major + drain + store
                tps = fc2ps.tile([128, 16, 128], F32, tag="eps", name="eps_tp")
                for tt, (t0, p) in enumerate(tok_tiles):
                    nc.tensor.transpose(tps[:p, tt, :], ot[:, t0:t0 + p],
                                        identf[:, :])
                stg = stgp.tile([128, 16, 128], F32, tag="stg", name="stg")
                for tt, (t0, p) in enumerate(tok_tiles):
                    if tt % 2 == 0:
                        nc.vector.tensor_copy(out=stg[:p, tt, :],
                                              in_=tps[:p, tt, :])
                    else:
                        nc.scalar.activation(out=stg[:p, tt, :],
                                             in_=tps[:p, tt, :],
                                             func=AF.Identity, bias=0.0)
                    nc.gpsimd.dma_start(
                        out=out_h[t0:t0 + p, ft * 128:(ft + 1) * 128],
                        in_=stg[:p, tt, :])
        fc2w_cm.__exit__(None, None, None)
        gT_cm.__exit__(None, None, None)
        consts_cm.__exit__(None, None, None)
    _split_sync_waits(nc)
    from concourse.library_overlay import lower_extended_insts
    lower_extended_insts(nc)
    return nc


_CACHED_NC = None


def _get_nc():
    global _CACHED_NC
    if _CACHED_NC is None:
        _CACHED_NC = build_program()
    return _CACHED_NC


def prepare_host_inputs(x, qkv_w, q_bias, v_bias, rel_bias_table, proj_w, proj_b,
                        ln1_g, ln1_b, ln2_g, ln2_b, fc1_w, fc1_b, fc2_w, fc2_b):
    bf = ml_dtypes.bfloat16
    f32 = np.float32
    x = np.asarray(x, f32)

    # fold LN1 gamma/beta into qkv weights, scale q by 1/8
    qkv_b = np.concatenate([q_bias, np.zeros_like(v_bias), v_bias]).astype(f32)
    W1 = qkv_w.astype(f32) * ln1_g[None, :].astype(f32)
    b1 = qkv_b + qkv_w.astype(f32) @ ln1_b.astype(f32)
    W1[:D] *= SCALE
    b1[:D] *= SCALE
    qkvwT = np.ascontiguousarray(W1.T).astype(bf)            # [1024, 3072]
    qkb = np.ascontiguousarray(b1[:2 * D].reshape(16, 128).T).astype(f32)
    vb_rep = np.broadcast_to(b1[2 * D:], (128, D)).copy().astype(bf)

    idx = _make_rel_pos_index()
    rel = rel_bias_table.astype(f32)[idx]                    # [q, k, h]
    # expbT[k, par, a, q] = exp(rel[q, k, 2a+par]): exp(S+B) = exp(S)*exp(B)
    ebk = np.exp(rel.transpose(1, 2, 0))                     # [k, h, q]
    expbT = np.ascontiguousarray(
        ebk.reshape(N_TOK, 8, 2, N_TOK).transpose(0, 2, 1, 3)).astype(bf)

    projwT = np.ascontiguousarray(proj_w.astype(f32).T).astype(bf)
    projb = np.ascontiguousarray(proj_b.astype(f32).reshape(8, 128).T)
    fc2b = np.ascontiguousarray(fc2_b.astype(f32).reshape(8, 128).T)

    W3 = fc1_w.astype(f32) * ln2_g[None, :].astype(f32)
    b3 = fc1_b.astype(f32) + fc1_w.astype(f32) @ ln2_b.astype(f32)
    W3T = np.ascontiguousarray(W3.T)                         # [1024, 4096]
    fc1wT = W3T.reshape(8, 128, 32, 128).transpose(2, 1, 0, 3)
    fc1wT = np.ascontiguousarray(fc1wT.reshape(32, 128, D)).astype(bf)
    fc1b = np.ascontiguousarray(b3.reshape(32, 128).T).astype(f32)

    # fc2 packed: fc2wp[ft, p, k, j] = fc2_w[ft*128+j, k*128+p]
    fc2wp = fc2_w.astype(f32).reshape(8, 128, 32, 128)       # [ft, j, k, p]
    fc2wp = np.ascontiguousarray(fc2wp.transpose(0, 3, 2, 1)).astype(bf)

    csel = np.zeros((128, NH, NH), np.float32)
    for h in range(NH):
        csel[:, h, h] = 1.0
    csel = csel.astype(bf)
    psel = np.zeros((16, IC, 128), f32)
    for c in range(IC):
        psel[2 * c, c, 0:64] = 1.0
        psel[2 * c + 1, c, 64:128] = 1.0
    psel = psel.astype(bf)

    shared = dict(qkvwT=qkvwT, qkb=qkb, vb_rep=vb_rep, expbT=expbT,
                  csel=csel, psel=psel,
                  projwT=projwT, projb=projb, fc1wT=fc1wT, fc1b=fc1b,
                  fc2wp=fc2wp, fc2b=fc2b)
    in_maps = []
    for cid in range(NCORES):
        sl = slice(cid * BL, (cid + 1) * BL)
        m = dict(shared)
        xTc = np.ascontiguousarray(x[sl].reshape(T, D).T)
        m["xT"] = xTc
        m["xbT"] = xTc.astype(bf)
        in_maps.append(m)
    return in_maps


def kernel(**inputs):
    nc = _get_nc()
    in_maps = prepare_host_inputs(**inputs)
    res = run_bass_kernel_spmd(nc, in_maps, list(range(NCORES)))
    outs = [res.results[c]["out"].reshape(BL, N_TOK, D) for c in range(NCORES)]
    return np.concatenate(outs, axis=0).astype(np.float32)



# revision 77
# speedup vs baseline: 1.0188x; 1.0188x over previous
"""Trainium2 Bass kernel v2 for the ViT transformer block — feature-major dataflow.

Everything on-chip flows feature-major ([feat, tok]); the host pre-transposes
x to xT and the kernel writes token-major output via cheap PE transposes.
LayerNorm statistics come from ones-vector matmuls (partition-dim reduction on
the PE); the per-token scale/shift rows are broadcast across partitions with
K=1 outer-product matmuls into PSUM and applied with two DVE passes.
Attention keeps the scores^T/exp/ones-column layout of v1, but context tiles
leave the attention phase through PE transposes (identity matmul) instead of
serialized DMA-transposes.  x2 returns to token-major through f32 PE
transposes so the fc2 drain and final residual run exactly like v1.

Sharding: data-parallel over batch, 8 samples per core on 8 cores.
"""

import sys
import os

sys.path.insert(0, "/opt/trn_rl_repo")

import numpy as np
import ml_dtypes

import concourse.bass as bass
import concourse.tile as tile
from concourse import mybir
from concourse import masks
from concourse.vector_clock import ScopedClock
from concourse.bass_utils import run_bass_kernel_spmd

F32 = mybir.dt.float32
BF16 = mybir.dt.bfloat16
AF = mybir.ActivationFunctionType
ALU = mybir.AluOpType

B, N_TOK, D = 64, 197, 1024
NCORES = 8
BL = B // NCORES            # samples per core = 8
T = BL * N_TOK              # tokens per core = 1576
NH, HD = 16, 64
HID = 4096
SCALE = HD ** -0.5
WH = WW = 14
NUM_REL = (2 * WH - 1) * (2 * WW - 1) + 3
LN_EPS = 1e-5

IC = 8                       # in-feature chunks of 128
CHUNKS = [(i * 512, min(512, T - i * 512)) for i in range((T + 511) // 512)]
ECHUNKS = [(i * 394, 394) for i in range(4)]   # uniform fc1/fc2 chunks
KTP = [128, N_TOK - 128]     # per-sample key tile sizes [128, 69]
NT = (T + 127) // 128        # 13 token tiles
LASTP = T - 128 * (NT - 1)   # 40


def _tok_tiles():
    return [(t * 128, 128 if t < NT - 1 else LASTP) for t in range(NT)]


def _sample_tiles():
    out = []
    for b in range(BL):
        for kt in range(2):
            out.append((b, kt, b * N_TOK + kt * 128, KTP[kt]))
    return out


def _make_rel_pos_index():
    coords = np.stack(np.meshgrid(np.arange(WH), np.arange(WW), indexing="ij"))
    flat = coords.reshape(2, -1)
    rel = flat[:, :, None] - flat[:, None, :]
    rel = rel.transpose(1, 2, 0).copy()
    rel[:, :, 0] += WH - 1
    rel[:, :, 1] += WW - 1
    rel[:, :, 0] *= 2 * WW - 1
    idx = np.zeros((N_TOK, N_TOK), dtype=np.int32)
    idx[1:, 1:] = rel.sum(-1)
    idx[0, 0:] = NUM_REL - 3
    idx[0:, 0] = NUM_REL - 2
    idx[0, 0] = NUM_REL - 1
    return idx


class SplitDrainTileContext(tile.TileContext):
    """Walrus in this toolchain rejects >1 sync-wait on the kernel-tail
    Drain; split the waits across a chain of drain instructions."""

    def _drain_and_barrier(self, tick_clock, wait_clock):
        drain_inst = self.nc.sync.drain()
        wait_clock.add_sem_waits(
            drain_inst.ins, ScopedClock({None: tick_clock.global_clock})
        )
        si = drain_inst.ins.sync_info
        waits = list(si.on_wait) if si and si.on_wait else []
        if len(waits) > 1:
            si.on_wait = waits[:1]
            for w in waits[1:]:
                d2 = self.nc.sync.drain()
                si2 = d2.ins.sync_info
                if si2 is None:
                    d2.ins.sync_info = mybir.SyncInfo(on_wait=[w], on_update=[])
                else:
                    si2.on_wait = [w]
        self.nc.all_engine_barrier()
        assert self.sems is not None
        popped = self.nc._tile_sem_poison_stack.pop()
        assert popped is self._sem_poison
        self.nc.clear_and_free_semaphores(list(self.sems.allocated().values()))
        self.nc.all_engine_barrier()


def _split_sync_waits(nc, cap=1):
    """Hoist excess sync-waits onto standalone event-semaphore instructions."""
    n = 0
    for fn in nc.m.functions:
        for bb in fn.blocks:
            insts = bb.instructions
            i = 0
            while i < len(insts):
                inst = insts[i]
                si = inst.sync_info
                waits = list(si.on_wait) if si and si.on_wait else []
                if len(waits) > cap and inst.engine != mybir.EngineType.Unassigned:
                    excess = waits[:len(waits) - cap]
                    si.on_wait = waits[len(waits) - cap:]
                    for w in excess:
                        ev = mybir.InstEventSemaphore(
                            name=f"waitsplit_{n}", ins=[], outs=[],
                            sync_info=mybir.SyncInfo(on_wait=[w], on_update=[]))
                        ev.engine = inst.engine
                        nc.register_instruction(ev)
                        insts.insert(i, ev)
                        n += 1
                        i += 1
                i += 1
    return n


def build_program():
    STOP = int(os.environ.get("K2_STOP", "7"))
    ASUB = os.environ.get("K2_ATTN_SUB", "full")
    NB = int(os.environ.get("K2_NB", str(BL)))
    NOEXP = os.environ.get("K2_NOEXP", "0") == "1"
    NOADD = os.environ.get("K2_NOADD", "0") == "1"
    NG = int(os.environ.get("K2_NG", "8"))
    NGI = int(os.environ.get("K2_NGI", "2"))
    nc = bass.Bass("TRN2", target_bir_lowering=False, debug=False,
                   num_devices=NCORES)

    # ---- DRAM I/O ----
    xT_h = nc.declare_dram_parameter("xT", [D, T], F32, isOutput=False)
    xbT_h = nc.declare_dram_parameter("xbT", [D, T], BF16, isOutput=False)
    qkvwT_h = nc.declare_dram_parameter("qkvwT", [D, 3 * D], BF16, isOutput=False)
    qkb_h = nc.declare_dram_parameter("qkb", [128, 16], F32, isOutput=False)
    vb_h = nc.declare_dram_parameter("vb_rep", [128, D], BF16, isOutput=False)
    expbT_h = nc.declare_dram_parameter("expbT", [N_TOK, 2, 8, N_TOK], BF16,
                                        isOutput=False)
    csel_h = nc.declare_dram_parameter("csel", [128, NH, NH], BF16,
                                       isOutput=False)
    psel_h = nc.declare_dram_parameter("psel", [16, IC, 128], BF16,
                                       isOutput=False)
    projwT_h = nc.declare_dram_parameter("projwT", [D, D], BF16, isOutput=False)
    projb_h = nc.declare_dram_parameter("projb", [128, 8], F32, isOutput=False)
    fc1wT_h = nc.declare_dram_parameter("fc1wT", [32, 128, D], BF16, isOutput=False)
    fc1b_h = nc.declare_dram_parameter("fc1b", [128, 32], F32, isOutput=False)
    fc2wp_h = nc.declare_dram_parameter("fc2wp", [IC, 128, 32, 128], BF16,
                                        isOutput=False)
    fc2b_h = nc.declare_dram_parameter("fc2b", [128, 8], F32, isOutput=False)
    out_h = nc.declare_dram_parameter("out", [T, D], F32, isOutput=True)
    x2s_h = nc.dram_tensor("x2s", [D, T], F32)   # x2 + fc2_b, feature-major

    tok_tiles = _tok_tiles()
    samp_tiles = _sample_tiles()

    with SplitDrainTileContext(nc) as tc:
        # ---------- right-side stack: consts > {ctxT | gT} ----------
        consts_cm = tc.tile_pool(name="consts", bufs=1, side="right")
        consts = consts_cm.__enter__()
        identb = consts.tile([128, 128], BF16, tag="identb", name="identb")
        masks.make_identity(nc, identb[:, :])
        ones_col = consts.tile([128, 1], BF16, tag="ones_col", name="ones_col")
        nc.vector.memset(ones_col, 1.0)
        ones_row = consts.tile([1, 128], BF16, tag="ones_row", name="ones_row")
        nc.vector.memset(ones_row, 1.0)
        qkb_t = consts.tile([128, 16], F32, tag="qkb", name="qkb")
        nc.sync.dma_start(out=qkb_t, in_=qkb_h[:, :])
        vb_t = consts.tile([128, D], BF16, tag="vb", name="vb")
        nc.sync.dma_start(out=vb_t, in_=vb_h[:, :])
        projb_t = consts.tile([128, 8], F32, tag="projb", name="projb")
        nc.sync.dma_start(out=projb_t, in_=projb_h[:, :])
        fc2b_t = consts.tile([128, 8], F32, tag="fc2b", name="fc2b")
        nc.sync.dma_start(out=fc2b_t, in_=fc2b_h[:, :])
        fc1b_t = consts.tile([128, 32], F32, tag="fc1b", name="fc1b")
        nc.sync.dma_start(out=fc1b_t, in_=fc1b_h[:, :])
        epsr = consts.tile([1, 1], F32, tag="epsr", name="epsr")
        nc.vector.memset(epsr, LN_EPS)

        def alloc_rows(pool, sfx):
            return dict(
                r0=pool.tile([1, T], F32, tag=f"row0{sfx}", name=f"row0{sfx}"),
                r1=pool.tile([1, T], F32, tag=f"row1{sfx}", name=f"row1{sfx}"),
                r2=pool.tile([1, T], F32, tag=f"row2{sfx}", name=f"row2{sfx}"),
                abf=pool.tile([1, T], BF16, tag=f"ra{sfx}", name=f"ra{sfx}"),
                bbf=pool.tile([1, T], BF16, tag=f"rb{sfx}", name=f"rb{sfx}"))

        def emit_ln_rows(rows, s1_ap_chunks, s2_ap_chunks):
            """a=rsqrt(var+eps) -> abf; b=mu*a -> bbf (normalize SUBTRACTS b).
            Chunk-pipelined so downstream bc/normalize of chunk 0 starts
            early.  r0..r2 scratch; sources may alias r0/r1."""
            r0, r1, r2 = rows['r0'], rows['r1'], rows['r2']
            abf, bbf = rows['abf'], rows['bbf']
            for ci, (c0, cw) in enumerate(CHUNKS):
                sl = slice(c0, c0 + cw)
                nc.scalar.activation(out=r2[0:1, sl], in_=s1_ap_chunks[ci],
                                     func=AF.Copy, scale=1.0 / D)   # mu
                nc.scalar.activation(out=r0[0:1, sl], in_=s2_ap_chunks[ci],
                                     func=AF.Copy, scale=1.0 / D,
                                     bias=float(LN_EPS))            # m2 + eps
                nc.vector.tensor_mul(out=r1[0:1, sl], in0=r2[0:1, sl],
                                     in1=r2[0:1, sl])
                nc.vector.tensor_sub(out=r0[0:1, sl], in0=r0[0:1, sl],
                                     in1=r1[0:1, sl])               # var + eps
                nc.vector.reciprocal_approx_fast(out=r1[0:1, sl],
                                                 in_=r0[0:1, sl])
                nc.scalar.activation(out=abf[0:1, sl], in_=r1[0:1, sl],
                                     func=AF.Sqrt)                  # rsqrt
                nc.vector.tensor_mul(out=bbf[0:1, sl], in0=r2[0:1, sl],
                                     in1=abf[0:1, sl])

        # ---------- left-side stack: attention superstructure ----------
        qk_cm = tc.tile_pool(name="qkp", bufs=1)
        qkp = qk_cm.__enter__()
        qkt = [qkp.tile([128, T], BF16, tag=f"qkt{ft}", name=f"qkt{ft}")
               for ft in range(16)]
        vt_cm = tc.tile_pool(name="vtp", bufs=1)
        vtp = vt_cm.__enter__()
        vt = {}
        for (b, kt, t0, kp) in samp_tiles:
            vt[(b, kt)] = vtp.tile([128, NH, 65], BF16, tag=f"v{b}_{kt}",
                                   name=f"v{b}_{kt}")
        h1T_cm = tc.tile_pool(name="h1Tp", bufs=1)
        h1Tp = h1T_cm.__enter__()
        h1T = h1Tp.tile([128, IC, T], BF16, tag="h1T", name="h1T")
        wv_cm = tc.tile_pool(name="wvp", bufs=1)
        wvp = wv_cm.__enter__()
        wv = [wvp.tile([128, D], BF16, tag=f"wv{c}", name=f"wv{c}")
              for c in range(IC)]
        wqk_cm = tc.tile_pool(name="wqk", bufs=1)
        wqkp = wqk_cm.__enter__()

        # ---------------- Phase A: load xT, LN1 stats ----------------
        ln1_xb_cm = tc.tile_pool(name="ln1_xb", bufs=1)
        xbp = ln1_xb_cm.__enter__()
        ln1_sq_cm = tc.tile_pool(name="ln1_sq", bufs=2)
        sqp = ln1_sq_cm.__enter__()
        ln1_nrm_cm = tc.tile_pool(name="nrm_tmp", bufs=2)
        nrmp = ln1_nrm_cm.__enter__()

        wqs = [wqkp.tile([128, D], BF16, tag=f"wq{c}", name=f"wq{c}")
               for c in range(IC)]

        rows1_cm = tc.tile_pool(name="rows1", bufs=1)
        rows1 = alloc_rows(rows1_cm.__enter__(), "1")
        abf, bbf = rows1['abf'], rows1['bbf']

        xb = []
        with tc.tile_pool(name="st1_ps", bufs=1, space="PSUM") as st1ps:
            # stat rows live at partition 32*ci so the four chunk-MMs hit
            # distinct PE column groups and run concurrently
            s1p = st1ps.tile([128, 4, 512], F32, tag="s1p", name="s1p")
            s2p = st1ps.tile([128, 4, 512], F32, tag="s2p", name="s2p")
            for c in range(IC):
                xb_c = xbp.tile([128, T], BF16, tag=f"xb{c}", name=f"xb{c}")
                for ci, (c0, cw) in enumerate(CHUNKS):
                    nc.sync.dma_start(out=xb_c[:, c0:c0 + cw],
                                      in_=xbT_h[c * 128:(c + 1) * 128,
                                                c0:c0 + cw])
                xb.append(xb_c)
                for ci, (c0, cw) in enumerate(CHUNKS):
                    rb = 32 * ci
                    xsq = sqp.tile([128, 512], BF16, tag="xsq", name="xsq")
                    nc.vector.tensor_mul(out=xsq[:, :cw],
                                         in0=xb_c[:, c0:c0 + cw],
                                         in1=xb_c[:, c0:c0 + cw])
                    nc.tensor.matmul(s1p[rb:rb + 1, ci, :cw],
                                     lhsT=ones_col[:, 0:1],
                                     rhs=xb_c[:, c0:c0 + cw],
                                     start=(c == 0), stop=(c == IC - 1),
                                     tile_position=(0, rb))
                    nc.tensor.matmul(s2p[rb:rb + 1, ci, :cw],
                                     lhsT=ones_col[:, 0:1],
                                     rhs=xsq[:, :cw],
                                     start=(c == 0), stop=(c == IC - 1),
                                     tile_position=(0, rb))
            for c in range(IC):          # Q weight slabs (after xT on the queue)
                nc.sync.dma_start(out=wqs[c],
                                  in_=qkvwT_h[c * 128:(c + 1) * 128, 0:D])
            for c in range(IC):          # V weight slabs (prefetch)
                nc.sync.dma_start(out=wv[c],
                                  in_=qkvwT_h[c * 128:(c + 1) * 128, 2 * D:3 * D])
            emit_ln_rows(rows1,
                         [s1p[32 * ci:32 * ci + 1, ci, :cw]
                          for ci, (c0, cw) in enumerate(CHUNKS)],
                         [s2p[32 * ci:32 * ci + 1, ci, :cw]
                          for ci, (c0, cw) in enumerate(CHUNKS)])

        # ---------- Phase B: normalize + Q (pipelined), K, V ----------
        with tc.tile_pool(name="bc_ps", bufs=2, space="PSUM") as bcps, \
             tc.tile_pool(name="qk_ps", bufs=4, space="PSUM") as qkps_pool:

            def emit_qk8(ci, base_ft):
                c0, cw = CHUNKS[ci]
                for fi in range(8):
                    ft = base_ft + fi
                    ps = qkps_pool.tile([128, 512], F32, tag="qkps", name="qkps")
                    for c in range(IC):
                        nc.tensor.matmul(
                            ps[:, :cw],
                            lhsT=wqs[c][:, fi * 128:(fi + 1) * 128],
                            rhs=h1T[:, c, c0:c0 + cw],
                            start=(c == 0), stop=(c == IC - 1))
                    if ft % 2 == 0:
                        nc.scalar.activation(out=qkt[ft][:, c0:c0 + cw],
                                             in_=ps[:, :cw], func=AF.Identity,
                                             bias=qkb_t[:, ft:ft + 1])
                    else:
                        nc.vector.tensor_add(
                            out=qkt[ft][:, c0:c0 + cw], in0=ps[:, :cw],
                            in1=qkb_t[:, ft:ft + 1].broadcast_to([128, cw]))

            for ci, (c0, cw) in enumerate(CHUNKS):
                if STOP < 2:
                    break
                bc = bcps.tile([128, 2, 512], F32, tag="bc", name="bc")
                nc.tensor.matmul(bc[:, 0, :cw], lhsT=ones_row[0:1, :],
                                 rhs=abf[0:1, c0:c0 + cw], start=True, stop=True)
                nc.tensor.matmul(bc[:, 1, :cw], lhsT=ones_row[0:1, :],
                                 rhs=bbf[0:1, c0:c0 + cw], start=True, stop=True)
                bcs = nrmp.tile([128, 2, 512], BF16, tag="bcs", name="bcs")
                nc.scalar.activation(out=bcs[:, :, :cw], in_=bc[:, :, :cw],
                                     func=AF.Copy)
                for c in range(IC):
                    tmp = nrmp.tile([128, 512], BF16, tag="ntmp", name="ntmp")
                    nc.vector.tensor_mul(out=tmp[:, :cw], in0=xb[c][:, c0:c0 + cw],
                                         in1=bcs[:, 0, :cw])
                    nc.vector.tensor_sub(out=h1T[:, c, c0:c0 + cw],
                                         in0=tmp[:, :cw], in1=bcs[:, 1, :cw])
                if ci > 0:
                    emit_qk8(ci - 1, 0)
            if STOP >= 2:
                emit_qk8(len(CHUNKS) - 1, 0)

            # K: reload the slab slots (overwrite waits on Q readers)
            if STOP >= 2:
                wks = [wqkp.tile([128, D], BF16, tag=f"wq{c}", name=f"wqk{c}")
                       for c in range(IC)]
                for c in range(IC):
                    nc.sync.dma_start(out=wks[c],
                                      in_=qkvwT_h[c * 128:(c + 1) * 128, D:2 * D])
                wqs = wks

            if STOP >= 2:
                for ci in range(len(CHUNKS)):
                    emit_qk8(ci, 8)

            rows1_cm.__exit__(None, None, None)
            ln1_nrm_cm.__exit__(None, None, None)
            ln1_sq_cm.__exit__(None, None, None)
            ln1_xb_cm.__exit__(None, None, None)
            wqk_cm.__exit__(None, None, None)

        # ---------------- Phase C: attention ----------------
        ctxT_cm = tc.tile_pool(name="ctxTp", bufs=1, side="right")
        ctxTp = ctxT_cm.__enter__()
        ctxT = [ctxTp.tile([128, T], BF16, tag=f"ctxT{k}", name=f"ctxT{k}")
                for k in range(IC)]

        attc_cm = tc.tile_pool(name="attc", bufs=1)
        attc = attc_cm.__enter__()
        eb = []
        for kt in range(2):
            kp = KTP[kt]
            t_ = attc.tile([128, 2, 8, N_TOK], BF16, tag=f"expb{kt}",
                           name=f"expb{kt}")
            nc.sync.dma_start(out=t_[:kp, :, :, :],
                              in_=expbT_h[kt * 128: kt * 128 + kp, :, :, :])
            eb.append(t_)
        # csel[:, h, :]: ones in column h (head-select for denominator MMs)
        csel = attc.tile([128, NH, NH], BF16, tag="csel", name="csel")
        nc.sync.dma_start(out=csel, in_=csel_h[:, :, :])
        # psel[:, c, :]: rec-row pair -> partition halves (rc broadcast)
        psel = attc.tile([16, IC, 128], BF16, tag="psel", name="psel")
        nc.sync.dma_start(out=psel, in_=psel_h[:, :, :])

        with tc.tile_pool(name="p_pool", bufs=4) as ppool, \
             tc.tile_pool(name="praw_pool", bufs=3) as prawp, \
             tc.tile_pool(name="rcb_pool", bufs=2) as rcbp, \
             tc.tile_pool(name="rcs_pool", bufs=3) as rcsp, \
             tc.tile_pool(name="sc_ps", bufs=2, space="PSUM") as scps, \
             tc.tile_pool(name="ctx_ps", bufs=2, space="PSUM") as ctxps, \
             tc.tile_pool(name="s16_ps", bufs=1, space="PSUM") as s16ps, \
             tc.tile_pool(name="rc_ps", bufs=1, space="PSUM") as rcps, \
             tc.tile_pool(name="cproj_ps", bufs=2, space="PSUM") as projps:

            pt = {}

            def emit_ctx(b):
                q0 = b * N_TOK
                # softmax denominators, all 16 heads -> one [16, 197] bank
                s16 = s16ps.tile([16, 256], F32, tag="s16", name="s16")
                n = 0
                for h in range(NH):
                    par, a = h % 2, h // 2
                    for kt in range(2):
                        kp = KTP[kt]
                        nc.tensor.matmul(
                            s16[0:NH, 0:N_TOK],
                            lhsT=csel[:kp, h, :],
                            rhs=pt[(b, kt)][:kp, par, a, :],
                            start=(n == 0), stop=(n == 31))
                        n += 1
                rcb = rcbp.tile([16, 256], F32, tag="rcb", name="rcb")
                nc.vector.reciprocal_approx_fast(out=rcb[0:NH, 0:N_TOK],
                                                 in_=s16[0:NH, 0:N_TOK])
                rcbb = rcbp.tile([16, 256], BF16, tag="rcbb", name="rcbb")
                nc.vector.tensor_copy(out=rcbb[0:NH, 0:N_TOK],
                                      in_=rcb[0:NH, 0:N_TOK])
                for c in range(IC):
                    # heads (2c, 2c+1) column-stacked into one psum bank
                    psc = ctxps.tile([128, 256], F32, tag="ctxps", name="ctxps")
                    for par in range(2):
                        h = 2 * c + par
                        for kt in range(2):
                            kp = KTP[kt]
                            nc.tensor.matmul(
                                psc[par * 64:(par + 1) * 64, 0:N_TOK],
                                lhsT=vt[(b, kt)][:kp, h, 0:64],
                                rhs=pt[(b, kt)][:kp, par, c, :],
                                start=(kt == 0), stop=(kt == 1))
                    rc = rcps.tile([128, 256], F32, tag="rcps2", name="rc")
                    nc.tensor.matmul(rc[0:128, 0:N_TOK],
                                     lhsT=psel[0:NH, c, :],
                                     rhs=rcbb[0:NH, 0:N_TOK],
                                     start=True, stop=True)
                    rcs = rcsp.tile([128, 256], F32, tag="rcs", name="rcs")
                    nc.vector.tensor_copy(out=rcs[:, 0:N_TOK],
                                          in_=rc[:, 0:N_TOK])
                    nc.vector.tensor_mul(out=ctxT[c][:, q0:q0 + N_TOK],
                                         in0=psc[0:128, 0:N_TOK],
                                         in1=rcs[0:128, 0:N_TOK])

            def emit_v(b):
                # V matmuls for sample b: dense K=128 full-array MMs keep the
                # HAM clock gate open during the attention phase.
                for kt in range(2):
                    t0 = b * N_TOK + kt * 128
                    kp = KTP[kt]
                    vtile = vt[(b, kt)]
                    for vc in range(2):
                        ps = projps.tile([128, 512], F32, tag="vps",
                                         name="vps")
                        for c in range(IC):
                            nc.tensor.matmul(
                                ps[:kp, :],
                                lhsT=h1T[:, c, t0:t0 + kp],
                                rhs=wv[c][:, vc * 512:(vc + 1) * 512],
                                start=(c == 0), stop=(c == IC - 1))
                        nc.vector.tensor_add(
                            out=vtile[:kp, vc * 8:(vc + 1) * 8, 0:64],
                            in0=ps[:kp, :].rearrange("p (a d) -> p a d", a=8),
                            in1=vb_t[:kp, vc * 512:(vc + 1) * 512].rearrange(
                                "p (a d) -> p a d", a=8))

            for b in range(BL):
                if STOP < 4:
                    break
                q0 = b * N_TOK
                for kt in range(2):
                    kp = KTP[kt]
                    k0 = q0 + kt * 128
                    ptile = ppool.tile([128, 2, 8, N_TOK], BF16, tag="P",
                                       name="P")
                    pt[(b, kt)] = ptile
                    # pair-tile (par, j) holds heads a=j and a=j+4 (same rb:
                    # mixed lhsT base partitions within one PSUM bank fault).
                    # Emission alternates par so consecutive MMs use opposite
                    # rb and LDWEIGHTS overlaps the in-flight matmul.
                    for j in range(4):
                        pss = [scps.tile([128, 2, 256], F32, tag="scps",
                                         name="scps") for _ in range(2)]
                        for s in range(2):
                            a = j + 4 * s
                            for par in range(2):
                                rb = par * 64
                                nc.tensor.matmul(
                                    pss[par][:kp, s, 0:N_TOK],
                                    lhsT=qkt[8 + a][rb:rb + 64, k0:k0 + kp],
                                    rhs=qkt[a][rb:rb + 64, q0:q0 + N_TOK],
                                    start=True, stop=True)
                        for par in range(2):
                            praw = prawp.tile([128, 2, N_TOK], BF16,
                                              tag="praw", name="praw")
                            nc.scalar.activation(out=praw[:kp, :, :],
                                                 in_=pss[par][:kp, :, 0:N_TOK],
                                                 func=AF.Exp)
                            nc.vector.tensor_mul(
                                out=ptile[:kp, par, j::4, :],
                                in0=praw[:kp, :, :],
                                in1=eb[kt][:kp, par, j::4, :])
                emit_v(b)
                if b > 0:
                    emit_ctx(b - 1)
            if STOP >= 4:
                emit_ctx(BL - 1)

        attc_cm.__exit__(None, None, None)
        wv_cm.__exit__(None, None, None)
        h1T_cm.__exit__(None, None, None)
        vt_cm.__exit__(None, None, None)
        qk_cm.__exit__(None, None, None)

        # ------------- Phase D: proj + residual + LN2 + x2->token-major -------
        fc2w_cm = tc.tile_pool(name="fc2w", bufs=3)
        fc2wsb = fc2w_cm.__enter__()
        h2T_cm = tc.tile_pool(name="h2Tp", bufs=1)
        h2Tp = h2T_cm.__enter__()
        h2T = h2Tp.tile([128, IC, T], BF16, tag="h2T", name="h2T")
        xb2_cm = tc.tile_pool(name="xb2p", bufs=1)
        xb2p = xb2_cm.__enter__()

        rows2_cm = tc.tile_pool(name="rows2", bufs=1)
        rows2 = alloc_rows(rows2_cm.__enter__(), "2")
        r0, r1 = rows2['r0'], rows2['r1']
        abf, bbf = rows2['abf'], rows2['bbf']
        nc.vector.memset(r0[0:1, :], 0.0)
        nc.vector.memset(r1[0:1, :], 0.0)

        xb2 = []

        pw_cm = tc.tile_pool(name="pwp", bufs=1)
        pwp = pw_cm.__enter__()
        pw = [pwp.tile([128, D], BF16, tag=f"pw{c}", name=f"pw{c}")
              for c in range(IC)]
        for c in range(IC):
            nc.sync.dma_start(out=pw[c], in_=projwT_h[c * 128:(c + 1) * 128, :])

        with tc.tile_pool(name="xt2", bufs=2) as xt2p, \
             tc.tile_pool(name="x2p", bufs=2) as x2p, \
             tc.tile_pool(name="sq2p", bufs=2) as sq2p, \
             tc.tile_pool(name="proj_ps", bufs=2, space="PSUM") as projps2, \
             tc.tile_pool(name="st2_ps", bufs=4, space="PSUM") as st2ps:
            for ft in range(IC):
                if STOP < 5:
                    break
                xt2 = xt2p.tile([128, T], F32, tag="xt2", name="xt2")
                nc.gpsimd.dma_start(out=xt2, in_=xT_h[ft * 128:(ft + 1) * 128, :])
                x2T = x2p.tile([128, T], F32, tag="x2T", name="x2T")
                for ci, (c0, cw) in enumerate(CHUNKS):
                    ps = projps2.tile([128, 512], F32, tag="projps",
                                      name="projps")
                    for c in range(IC):
                        nc.tensor.matmul(
                            ps[:, :cw],
                            lhsT=pw[c][:, ft * 128:(ft + 1) * 128],
                            rhs=ctxT[c][:, c0:c0 + cw],
                            start=(c == 0), stop=(c == IC - 1))
                    nc.scalar.activation(out=x2T[:, c0:c0 + cw], in_=ps[:, :cw],
                                         func=AF.Identity,
                                         bias=projb_t[:, ft:ft + 1])
                nc.vector.tensor_add(out=x2T, in0=x2T, in1=xt2)
                xb2_f = xb2p.tile([128, T], BF16, tag=f"xb2_{ft}",
                                  name=f"xb2_{ft}")
                nc.scalar.activation(out=xb2_f, in_=x2T, func=AF.Copy)
                xb2.append(xb2_f)
                xsq2 = sq2p.tile([128, T], BF16, tag="xsq2", name="xsq2")
                nc.vector.tensor_mul(out=xsq2, in0=xb2_f, in1=xb2_f)
                for (src_t, accr) in ((xb2_f, r0), (xsq2, r1)):
                    for ci, (c0, cw) in enumerate(CHUNKS):
                        rb = 32 * ci
                        p1 = st2ps.tile([128, 512], F32, tag="st2", name="st2")
                        nc.tensor.matmul(p1[rb:rb + 1, :cw],
                                         lhsT=ones_col[:, 0:1],
                                         rhs=src_t[:, c0:c0 + cw],
                                         start=True, stop=True,
                                         tile_position=(0, rb))
                        nc.vector.tensor_add(out=accr[0:1, c0:c0 + cw],
                                             in0=accr[0:1, c0:c0 + cw],
                                             in1=p1[rb:rb + 1, :cw])
                # x2 + fc2_b -> feature-major DRAM scratch (read back in fc2)
                x2fb = x2p.tile([128, T], F32, tag="x2fb", name="x2fb")
                nc.scalar.activation(out=x2fb, in_=x2T, func=AF.Identity,
                                     bias=fc2b_t[:, ft:ft + 1])
                nc.sync.dma_start(out=x2s_h[ft * 128:(ft + 1) * 128, :],
                                  in_=x2fb)

        ctxT_cm.__exit__(None, None, None)
        pw_cm.__exit__(None, None, None)

        if STOP >= 5:
            emit_ln_rows(rows2,
                         [r0[0:1, c0:c0 + cw] for (c0, cw) in CHUNKS],
                         [r1[0:1, c0:c0 + cw] for (c0, cw) in CHUNKS])

        with tc.tile_pool(name="bc2_ps", bufs=2, space="PSUM") as bc2ps, \
             tc.tile_pool(name="nrm2", bufs=3) as nrm2p:
            for ci, (c0, cw) in enumerate(CHUNKS):
                if STOP < 5:
                    break
                bc = bc2ps.tile([128, 2, 512], F32, tag="bc2", name="bc2")
                nc.tensor.matmul(bc[:, 0, :cw], lhsT=ones_row[0:1, :],
                                 rhs=abf[0:1, c0:c0 + cw], start=True, stop=True)
                nc.tensor.matmul(bc[:, 1, :cw], lhsT=ones_row[0:1, :],
                                 rhs=bbf[0:1, c0:c0 + cw], start=True, stop=True)
                bcs = nrm2p.tile([128, 2, 512], BF16, tag="bcs2", name="bcs2")
                nc.scalar.activation(out=bcs[:, :, :cw], in_=bc[:, :, :cw],
                                     func=AF.Copy)
                for c in range(IC):
                    tmp = nrm2p.tile([128, 512], BF16, tag="n2tmp", name="n2tmp")
                    nc.vector.tensor_mul(out=tmp[:, :cw],
                                         in0=xb2[c][:, c0:c0 + cw],
                                         in1=bcs[:, 0, :cw])
                    nc.vector.tensor_sub(out=h2T[:, c, c0:c0 + cw],
                                         in0=tmp[:, :cw], in1=bcs[:, 1, :cw])
        rows2_cm.__exit__(None, None, None)
        xb2_cm.__exit__(None, None, None)

        # ---------------- Phase E: MLP ----------------
        gT_cm = tc.tile_pool(name="gT_pool", bufs=1, side="right")
        gTp = gT_cm.__enter__()
        gT = gTp.tile([128, 32, T], BF16, tag="gT", name="gT")
        identf = gTp.tile([128, 128], F32, tag="identf", name="identf")
        masks.make_identity(nc, identf[:, :])
        with tc.tile_pool(name="fc1w", bufs=3) as fc1wp, \
             tc.tile_pool(name="fc1_ps", bufs=4, space="PSUM") as fc1ps:
            for Ht in range(32):
                if STOP < 6:
                    break
                wt = fc1wp.tile([128, D], BF16, tag="fc1w", name="fc1w")
                nc.sync.dma_start(out=wt, in_=fc1wT_h[Ht, :, :])
                for j in range(2):           # super-chunks of 788 = 2x394
                    ps = fc1ps.tile([128, 2, 512], F32, tag="fc1ps",
                                    name="fc1ps")
                    for k in range(2):
                        c0, cw = ECHUNKS[j * 2 + k]
                        for c in range(IC):
                            nc.tensor.matmul(
                                ps[:, k, :cw],
                                lhsT=wt[:, c * 128:(c + 1) * 128],
                                rhs=h2T[:, c, c0:c0 + cw],
                                start=(c == 0), stop=(c == IC - 1))
                    nc.scalar.activation(
                        out=gT[:, Ht, j * 788:(j + 1) * 788],
                        in_=ps[:, :, 0:394],
                        func=AF.Gelu, bias=fc1b_t[:, Ht:Ht + 1])
        h2T_cm.__exit__(None, None, None)

        # ---- fc2 feature-major: out^T[ft, t] = sum_H fc2w^T . gT ----
        with tc.tile_pool(name="xf_sb", bufs=2) as xfp, \
             tc.tile_pool(name="ot_sb", bufs=2) as otp, \
             tc.tile_pool(name="stg_sb", bufs=2) as stgp, \
             tc.tile_pool(name="fc2_ps", bufs=2, space="PSUM") as fc2ps:
            for ft in range(IC):
                if STOP < 7:
                    break
                w2 = fc2wsb.tile([128, 32, 128], BF16, tag="fc2w", name="fc2w")
                nc.sync.dma_start(out=w2, in_=fc2wp_h[ft, :, :, :])
                xf = xfp.tile([128, T], F32, tag="xf", name="xf")
                nc.gpsimd.dma_start(out=xf,
                                    in_=x2s_h[ft * 128:(ft + 1) * 128, :])
                ps = fc2ps.tile([128, 4, 512], F32, tag="eps", name="eps_mm")
                ot = otp.tile([128, T], F32, tag="ot", name="ot")
                for ci, (c0, cw) in enumerate(ECHUNKS):
                    for Hkt in range(32):
                        nc.tensor.matmul(
                            ps[:, ci, :cw],
                            lhsT=w2[:, Hkt, :],
                            rhs=gT[:, Hkt, c0:c0 + cw],
                            start=(Hkt == 0), stop=(Hkt == 31))
                    nc.vector.tensor_add(out=ot[:, c0:c0 + cw],
                                         in0=ps[:, ci, :cw],
                                         in1=xf[:, c0:c0 + cw])
                # transpose to token-major + drain + store
                tps = fc2ps.tile([128, 16, 128], F32, tag="eps", name="eps_tp")
                for tt, (t0, p) in enumerate(tok_tiles):
                    nc.tensor.transpose(tps[:p, tt, :], ot[:, t0:t0 + p],
                                        identf[:, :])
                stg = stgp.tile([128, 16, 128], F32, tag="stg", name="stg")
                nc.vector.tensor_copy(out=stg[:, 0:8, :], in_=tps[:, 0:8, :])
                nc.scalar.activation(out=stg[:, 8:12, :], in_=tps[:, 8:12, :],
                                     func=AF.Identity, bias=0.0)
                nc.vector.tensor_copy(out=stg[0:LASTP, 12, :],
                                      in_=tps[0:LASTP, 12, :])
                for tt, (t0, p) in enumerate(tok_tiles):
                    nc.gpsimd.dma_start(
                        out=out_h[t0:t0 + p, ft * 128:(ft + 1) * 128],
                        in_=stg[:p, tt, :])
        fc2w_cm.__exit__(None, None, None)
        gT_cm.__exit__(None, None, None)
        consts_cm.__exit__(None, None, None)
    _split_sync_waits(nc)
    from concourse.library_overlay import lower_extended_insts
    lower_extended_insts(nc)
    return nc


_CACHED_NC = None


def _get_nc():
    global _CACHED_NC
    if _CACHED_NC is None:
        _CACHED_NC = build_program()
    return _CACHED_NC


def prepare_host_inputs(x, qkv_w, q_bias, v_bias, rel_bias_table, proj_w, proj_b,
                        ln1_g, ln1_b, ln2_g, ln2_b, fc1_w, fc1_b, fc2_w, fc2_b):
    bf = ml_dtypes.bfloat16
    f32 = np.float32
    x = np.asarray(x, f32)

    # fold LN1 gamma/beta into qkv weights, scale q by 1/8
    qkv_b = np.concatenate([q_bias, np.zeros_like(v_bias), v_bias]).astype(f32)
    W1 = qkv_w.astype(f32) * ln1_g[None, :].astype(f32)
    b1 = qkv_b + qkv_w.astype(f32) @ ln1_b.astype(f32)
    W1[:D] *= SCALE
    b1[:D] *= SCALE
    qkvwT = np.ascontiguousarray(W1.T).astype(bf)            # [1024, 3072]
    qkb = np.ascontiguousarray(b1[:2 * D].reshape(16, 128).T).astype(f32)
    vb_rep = np.broadcast_to(b1[2 * D:], (128, D)).copy().astype(bf)

    idx = _make_rel_pos_index()
    rel = rel_bias_table.astype(f32)[idx]                    # [q, k, h]
    # expbT[k, par, a, q] = exp(rel[q, k, 2a+par]): exp(S+B) = exp(S)*exp(B)
    ebk = np.exp(rel.transpose(1, 2, 0))                     # [k, h, q]
    expbT = np.ascontiguousarray(
        ebk.reshape(N_TOK, 8, 2, N_TOK).transpose(0, 2, 1, 3)).astype(bf)

    projwT = np.ascontiguousarray(proj_w.astype(f32).T).astype(bf)
    projb = np.ascontiguousarray(proj_b.astype(f32).reshape(8, 128).T)
    fc2b = np.ascontiguousarray(fc2_b.astype(f32).reshape(8, 128).T)

    W3 = fc1_w.astype(f32) * ln2_g[None, :].astype(f32)
    b3 = fc1_b.astype(f32) + fc1_w.astype(f32) @ ln2_b.astype(f32)
    W3T = np.ascontiguousarray(W3.T)                         # [1024, 4096]
    fc1wT = W3T.reshape(8, 128, 32, 128).transpose(2, 1, 0, 3)
    fc1wT = np.ascontiguousarray(fc1wT.reshape(32, 128, D)).astype(bf)
    fc1b = np.ascontiguousarray(b3.reshape(32, 128).T).astype(f32)

    # fc2 packed: fc2wp[ft, p, k, j] = fc2_w[ft*128+j, k*128+p]
    fc2wp = fc2_w.astype(f32).reshape(8, 128, 32, 128)       # [ft, j, k, p]
    fc2wp = np.ascontiguousarray(fc2wp.transpose(0, 3, 2, 1)).astype(bf)

    csel = np.zeros((128, NH, NH), np.float32)
    for h in range(NH):
        csel[:, h, h] = 1.0
    csel = csel.astype(bf)
    psel = np.zeros((16, IC, 128), f32)
    for c in range(IC):
        psel[2 * c, c, 0:64] = 1.0
        psel[2 * c + 1, c, 64:128] = 1.0
    psel = psel.astype(bf)

    shared = dict(qkvwT=qkvwT, qkb=qkb, vb_rep=vb_rep, expbT=expbT,
                  csel=csel, psel=psel,
                  projwT=projwT, projb=projb, fc1wT=fc1wT, fc1b=fc1b,
                  fc2wp=fc2wp, fc2b=fc2b)
    in_maps = []
    for cid in range(NCORES):
        sl = slice(cid * BL, (cid + 1) * BL)
        m = dict(shared)
        xTc = np.ascontiguousarray(x[sl].reshape(T, D).T)
        m["xT"] = xTc
        m["xbT"] = xTc.astype(bf)
        in_maps.append(m)
    return in_maps


def kernel(**inputs):
    nc = _get_nc()
    in_maps = prepare_host_inputs(**inputs)
    res = run_bass_kernel_spmd(nc, in_maps, list(range(NCORES)))
    outs = [res.results[c]["out"].reshape(BL, N_TOK, D) for c in range(NCORES)]
    return np.concatenate(outs, axis=0).astype(np.float32)



# revision 81
# speedup vs baseline: 1.0491x; 1.0298x over previous
"""Trainium2 Bass kernel v2 for the ViT transformer block — feature-major dataflow.

Everything on-chip flows feature-major ([feat, tok]); the host pre-transposes
x to xT and the kernel writes token-major output via cheap PE transposes.
LayerNorm statistics come from ones-vector matmuls (partition-dim reduction on
the PE); the per-token scale/shift rows are broadcast across partitions with
K=1 outer-product matmuls into PSUM and applied with two DVE passes.
Attention keeps the scores^T/exp/ones-column layout of v1, but context tiles
leave the attention phase through PE transposes (identity matmul) instead of
serialized DMA-transposes.  x2 returns to token-major through f32 PE
transposes so the fc2 drain and final residual run exactly like v1.

Sharding: data-parallel over batch, 8 samples per core on 8 cores.
"""

import sys
import os

sys.path.insert(0, "/opt/trn_rl_repo")

import numpy as np
import ml_dtypes

import concourse.bass as bass
import concourse.tile as tile
from concourse import mybir
from concourse import masks
from concourse.vector_clock import ScopedClock
from concourse.bass_utils import run_bass_kernel_spmd

F32 = mybir.dt.float32
BF16 = mybir.dt.bfloat16
AF = mybir.ActivationFunctionType
ALU = mybir.AluOpType

B, N_TOK, D = 64, 197, 1024
NCORES = 8
BL = B // NCORES            # samples per core = 8
T = BL * N_TOK              # tokens per core = 1576
NH, HD = 16, 64
HID = 4096
SCALE = HD ** -0.5
WH = WW = 14
NUM_REL = (2 * WH - 1) * (2 * WW - 1) + 3
LN_EPS = 1e-5

IC = 8                       # in-feature chunks of 128
CHUNKS = [(i * 512, min(512, T - i * 512)) for i in range((T + 511) // 512)]
ECHUNKS = [(i * 394, 394) for i in range(4)]   # uniform fc1/fc2 chunks
KTP = [128, N_TOK - 128]     # per-sample key tile sizes [128, 69]
NT = (T + 127) // 128        # 13 token tiles
LASTP = T - 128 * (NT - 1)   # 40


def _tok_tiles():
    return [(t * 128, 128 if t < NT - 1 else LASTP) for t in range(NT)]


def _sample_tiles():
    out = []
    for b in range(BL):
        for kt in range(2):
            out.append((b, kt, b * N_TOK + kt * 128, KTP[kt]))
    return out


def _make_rel_pos_index():
    coords = np.stack(np.meshgrid(np.arange(WH), np.arange(WW), indexing="ij"))
    flat = coords.reshape(2, -1)
    rel = flat[:, :, None] - flat[:, None, :]
    rel = rel.transpose(1, 2, 0).copy()
    rel[:, :, 0] += WH - 1
    rel[:, :, 1] += WW - 1
    rel[:, :, 0] *= 2 * WW - 1
    idx = np.zeros((N_TOK, N_TOK), dtype=np.int32)
    idx[1:, 1:] = rel.sum(-1)
    idx[0, 0:] = NUM_REL - 3
    idx[0:, 0] = NUM_REL - 2
    idx[0, 0] = NUM_REL - 1
    return idx


class SplitDrainTileContext(tile.TileContext):
    """Walrus in this toolchain rejects >1 sync-wait on the kernel-tail
    Drain; split the waits across a chain of drain instructions."""

    def _drain_and_barrier(self, tick_clock, wait_clock):
        drain_inst = self.nc.sync.drain()
        wait_clock.add_sem_waits(
            drain_inst.ins, ScopedClock({None: tick_clock.global_clock})
        )
        si = drain_inst.ins.sync_info
        waits = list(si.on_wait) if si and si.on_wait else []
        if len(waits) > 1:
            si.on_wait = waits[:1]
            for w in waits[1:]:
                d2 = self.nc.sync.drain()
                si2 = d2.ins.sync_info
                if si2 is None:
                    d2.ins.sync_info = mybir.SyncInfo(on_wait=[w], on_update=[])
                else:
                    si2.on_wait = [w]
        self.nc.all_engine_barrier()
        assert self.sems is not None
        popped = self.nc._tile_sem_poison_stack.pop()
        assert popped is self._sem_poison
        self.nc.clear_and_free_semaphores(list(self.sems.allocated().values()))
        self.nc.all_engine_barrier()


def _split_sync_waits(nc, cap=1):
    """Hoist excess sync-waits onto standalone event-semaphore instructions."""
    n = 0
    for fn in nc.m.functions:
        for bb in fn.blocks:
            insts = bb.instructions
            i = 0
            while i < len(insts):
                inst = insts[i]
                si = inst.sync_info
                waits = list(si.on_wait) if si and si.on_wait else []
                if len(waits) > cap and inst.engine != mybir.EngineType.Unassigned:
                    excess = waits[:len(waits) - cap]
                    si.on_wait = waits[len(waits) - cap:]
                    for w in excess:
                        ev = mybir.InstEventSemaphore(
                            name=f"waitsplit_{n}", ins=[], outs=[],
                            sync_info=mybir.SyncInfo(on_wait=[w], on_update=[]))
                        ev.engine = inst.engine
                        nc.register_instruction(ev)
                        insts.insert(i, ev)
                        n += 1
                        i += 1
                i += 1
    return n


def build_program():
    STOP = int(os.environ.get("K2_STOP", "7"))
    ASUB = os.environ.get("K2_ATTN_SUB", "full")
    NB = int(os.environ.get("K2_NB", str(BL)))
    NOEXP = os.environ.get("K2_NOEXP", "0") == "1"
    NOADD = os.environ.get("K2_NOADD", "0") == "1"
    NG = int(os.environ.get("K2_NG", "8"))
    NGI = int(os.environ.get("K2_NGI", "2"))
    nc = bass.Bass("TRN2", target_bir_lowering=False, debug=False,
                   num_devices=NCORES)

    # ---- DRAM I/O ----
    xT_h = nc.declare_dram_parameter("xT", [D, T], F32, isOutput=False)
    xbT_h = nc.declare_dram_parameter("xbT", [D, T], BF16, isOutput=False)
    qkvwT_h = nc.declare_dram_parameter("qkvwT", [D, 3 * D], BF16, isOutput=False)
    qkb_h = nc.declare_dram_parameter("qkb", [128, 16], F32, isOutput=False)
    vb_h = nc.declare_dram_parameter("vb_rep", [128, D], BF16, isOutput=False)
    expbT_h = nc.declare_dram_parameter("expbT", [N_TOK, 2, 8, N_TOK], BF16,
                                        isOutput=False)
    csel_h = nc.declare_dram_parameter("csel", [128, NH, NH], BF16,
                                       isOutput=False)
    psel_h = nc.declare_dram_parameter("psel", [16, IC, 128], BF16,
                                       isOutput=False)
    projwT_h = nc.declare_dram_parameter("projwT", [D, D], BF16, isOutput=False)
    projb_h = nc.declare_dram_parameter("projb", [128, 8], F32, isOutput=False)
    fc1wT_h = nc.declare_dram_parameter("fc1wT", [32, 128, D], BF16, isOutput=False)
    fc1b_h = nc.declare_dram_parameter("fc1b", [128, 32], F32, isOutput=False)
    fc2wp_h = nc.declare_dram_parameter("fc2wp", [IC, 128, 32, 128], BF16,
                                        isOutput=False)
    fc2b_h = nc.declare_dram_parameter("fc2b", [128, 8], F32, isOutput=False)
    out_h = nc.declare_dram_parameter("out", [T, D], F32, isOutput=True)
    x2s_h = nc.dram_tensor("x2s", [D, T], F32)   # x2 + fc2_b, feature-major

    tok_tiles = _tok_tiles()
    samp_tiles = _sample_tiles()

    with SplitDrainTileContext(nc) as tc:
        # ---------- right-side stack: consts > {ctxT | gT} ----------
        consts_cm = tc.tile_pool(name="consts", bufs=1, side="right")
        consts = consts_cm.__enter__()
        identb = consts.tile([128, 128], BF16, tag="identb", name="identb")
        masks.make_identity(nc, identb[:, :])
        ones_col = consts.tile([128, 1], BF16, tag="ones_col", name="ones_col")
        nc.vector.memset(ones_col, 1.0)
        ones_row = consts.tile([1, 128], BF16, tag="ones_row", name="ones_row")
        nc.vector.memset(ones_row, 1.0)
        qkb_t = consts.tile([128, 16], F32, tag="qkb", name="qkb")
        nc.sync.dma_start(out=qkb_t, in_=qkb_h[:, :])
        vb_t = consts.tile([128, D], BF16, tag="vb", name="vb")
        nc.sync.dma_start(out=vb_t, in_=vb_h[:, :])
        projb_t = consts.tile([128, 8], F32, tag="projb", name="projb")
        nc.sync.dma_start(out=projb_t, in_=projb_h[:, :])
        fc2b_t = consts.tile([128, 8], F32, tag="fc2b", name="fc2b")
        nc.sync.dma_start(out=fc2b_t, in_=fc2b_h[:, :])
        fc1b_t = consts.tile([128, 32], F32, tag="fc1b", name="fc1b")
        nc.sync.dma_start(out=fc1b_t, in_=fc1b_h[:, :])
        epsr = consts.tile([1, 1], F32, tag="epsr", name="epsr")
        nc.vector.memset(epsr, LN_EPS)

        def alloc_rows(pool, sfx):
            return dict(
                r0=pool.tile([1, T], F32, tag=f"row0{sfx}", name=f"row0{sfx}"),
                r1=pool.tile([1, T], F32, tag=f"row1{sfx}", name=f"row1{sfx}"),
                r2=pool.tile([1, T], F32, tag=f"row2{sfx}", name=f"row2{sfx}"),
                abf=pool.tile([1, T], BF16, tag=f"ra{sfx}", name=f"ra{sfx}"),
                bbf=pool.tile([1, T], BF16, tag=f"rb{sfx}", name=f"rb{sfx}"))

        def emit_ln_rows(rows, s1_ap_chunks, s2_ap_chunks):
            """a=rsqrt(var+eps) -> abf; b=mu*a -> bbf (normalize SUBTRACTS b).
            Chunk-pipelined so downstream bc/normalize of chunk 0 starts
            early.  r0..r2 scratch; sources may alias r0/r1."""
            r0, r1, r2 = rows['r0'], rows['r1'], rows['r2']
            abf, bbf = rows['abf'], rows['bbf']
            for ci, (c0, cw) in enumerate(CHUNKS):
                sl = slice(c0, c0 + cw)
                nc.scalar.activation(out=r2[0:1, sl], in_=s1_ap_chunks[ci],
                                     func=AF.Copy, scale=1.0 / D)   # mu
                nc.scalar.activation(out=r0[0:1, sl], in_=s2_ap_chunks[ci],
                                     func=AF.Copy, scale=1.0 / D,
                                     bias=float(LN_EPS))            # m2 + eps
                nc.vector.tensor_mul(out=r1[0:1, sl], in0=r2[0:1, sl],
                                     in1=r2[0:1, sl])
                nc.vector.tensor_sub(out=r0[0:1, sl], in0=r0[0:1, sl],
                                     in1=r1[0:1, sl])               # var + eps
                nc.vector.reciprocal_approx_fast(out=r1[0:1, sl],
                                                 in_=r0[0:1, sl])
                nc.scalar.activation(out=abf[0:1, sl], in_=r1[0:1, sl],
                                     func=AF.Sqrt)                  # rsqrt
                nc.vector.tensor_mul(out=bbf[0:1, sl], in0=r2[0:1, sl],
                                     in1=abf[0:1, sl])

        # ---------- left-side stack: attention superstructure ----------
        qk_cm = tc.tile_pool(name="qkp", bufs=1)
        qkp = qk_cm.__enter__()
        qkt = [qkp.tile([128, T], BF16, tag=f"qkt{ft}", name=f"qkt{ft}")
               for ft in range(16)]
        vt_cm = tc.tile_pool(name="vtp", bufs=1)
        vtp = vt_cm.__enter__()
        vt = {}
        for (b, kt, t0, kp) in samp_tiles:
            vt[(b, kt)] = vtp.tile([128, NH, 65], BF16, tag=f"v{b}_{kt}",
                                   name=f"v{b}_{kt}")
        h1T_cm = tc.tile_pool(name="h1Tp", bufs=1)
        h1Tp = h1T_cm.__enter__()
        h1T = h1Tp.tile([128, IC, T], BF16, tag="h1T", name="h1T")
        wv_cm = tc.tile_pool(name="wvp", bufs=1)
        wvp = wv_cm.__enter__()
        wv = [wvp.tile([128, D], BF16, tag=f"wv{c}", name=f"wv{c}")
              for c in range(IC)]
        wqk_cm = tc.tile_pool(name="wqk", bufs=1)
        wqkp = wqk_cm.__enter__()

        # ---------------- Phase A: load xT, LN1 stats ----------------
        ln1_xb_cm = tc.tile_pool(name="ln1_xb", bufs=1)
        xbp = ln1_xb_cm.__enter__()
        ln1_sq_cm = tc.tile_pool(name="ln1_sq", bufs=2)
        sqp = ln1_sq_cm.__enter__()
        ln1_nrm_cm = tc.tile_pool(name="nrm_tmp", bufs=2)
        nrmp = ln1_nrm_cm.__enter__()

        wqs = [wqkp.tile([128, D], BF16, tag=f"wq{c}", name=f"wq{c}")
               for c in range(IC)]

        rows1_cm = tc.tile_pool(name="rows1", bufs=1)
        rows1 = alloc_rows(rows1_cm.__enter__(), "1")
        abf, bbf = rows1['abf'], rows1['bbf']

        xb = []
        with tc.tile_pool(name="st1_ps", bufs=1, space="PSUM") as st1ps:
            # stat rows live at partition 32*ci so the four chunk-MMs hit
            # distinct PE column groups and run concurrently
            s1p = st1ps.tile([128, 4, 512], F32, tag="s1p", name="s1p")
            s2p = st1ps.tile([128, 4, 512], F32, tag="s2p", name="s2p")
            for c in range(IC):
                xb_c = xbp.tile([128, T], BF16, tag=f"xb{c}", name=f"xb{c}")
                for ci, (c0, cw) in enumerate(CHUNKS):
                    nc.sync.dma_start(out=xb_c[:, c0:c0 + cw],
                                      in_=xbT_h[c * 128:(c + 1) * 128,
                                                c0:c0 + cw])
                xb.append(xb_c)
                for ci, (c0, cw) in enumerate(CHUNKS):
                    rb = 32 * ci
                    xsq = sqp.tile([128, 512], BF16, tag="xsq", name="xsq")
                    nc.vector.tensor_mul(out=xsq[:, :cw],
                                         in0=xb_c[:, c0:c0 + cw],
                                         in1=xb_c[:, c0:c0 + cw])
                    nc.tensor.matmul(s1p[rb:rb + 1, ci, :cw],
                                     lhsT=ones_col[:, 0:1],
                                     rhs=xb_c[:, c0:c0 + cw],
                                     start=(c == 0), stop=(c == IC - 1),
                                     tile_position=(0, rb))
                    nc.tensor.matmul(s2p[rb:rb + 1, ci, :cw],
                                     lhsT=ones_col[:, 0:1],
                                     rhs=xsq[:, :cw],
                                     start=(c == 0), stop=(c == IC - 1),
                                     tile_position=(0, rb))
            for c in range(IC):          # Q weight slabs (after xT on the queue)
                nc.sync.dma_start(out=wqs[c],
                                  in_=qkvwT_h[c * 128:(c + 1) * 128, 0:D])
            for c in range(IC):          # V weight slabs (prefetch)
                nc.sync.dma_start(out=wv[c],
                                  in_=qkvwT_h[c * 128:(c + 1) * 128, 2 * D:3 * D])
            emit_ln_rows(rows1,
                         [s1p[32 * ci:32 * ci + 1, ci, :cw]
                          for ci, (c0, cw) in enumerate(CHUNKS)],
                         [s2p[32 * ci:32 * ci + 1, ci, :cw]
                          for ci, (c0, cw) in enumerate(CHUNKS)])

        # ---------- Phase B: normalize + Q (pipelined), K, V ----------
        with tc.tile_pool(name="bc_ps", bufs=2, space="PSUM") as bcps, \
             tc.tile_pool(name="qk_ps", bufs=4, space="PSUM") as qkps_pool:

            def emit_qk8(ci, base_ft):
                c0, cw = CHUNKS[ci]
                for fi in range(8):
                    ft = base_ft + fi
                    ps = qkps_pool.tile([128, 512], F32, tag="qkps", name="qkps")
                    for c in range(IC):
                        nc.tensor.matmul(
                            ps[:, :cw],
                            lhsT=wqs[c][:, fi * 128:(fi + 1) * 128],
                            rhs=h1T[:, c, c0:c0 + cw],
                            start=(c == 0), stop=(c == IC - 1))
                    if ft % 2 == 0:
                        nc.scalar.activation(out=qkt[ft][:, c0:c0 + cw],
                                             in_=ps[:, :cw], func=AF.Identity,
                                             bias=qkb_t[:, ft:ft + 1])
                    else:
                        nc.vector.tensor_add(
                            out=qkt[ft][:, c0:c0 + cw], in0=ps[:, :cw],
                            in1=qkb_t[:, ft:ft + 1].broadcast_to([128, cw]))

            for ci, (c0, cw) in enumerate(CHUNKS):
                if STOP < 2:
                    break
                bc = bcps.tile([128, 2, 512], F32, tag="bc", name="bc")
                nc.tensor.matmul(bc[:, 0, :cw], lhsT=ones_row[0:1, :],
                                 rhs=abf[0:1, c0:c0 + cw], start=True, stop=True)
                nc.tensor.matmul(bc[:, 1, :cw], lhsT=ones_row[0:1, :],
                                 rhs=bbf[0:1, c0:c0 + cw], start=True, stop=True)
                bcs = nrmp.tile([128, 2, 512], BF16, tag="bcs", name="bcs")
                nc.scalar.activation(out=bcs[:, :, :cw], in_=bc[:, :, :cw],
                                     func=AF.Copy)
                for c in range(IC):
                    tmp = nrmp.tile([128, 512], BF16, tag="ntmp", name="ntmp")
                    nc.vector.tensor_mul(out=tmp[:, :cw], in0=xb[c][:, c0:c0 + cw],
                                         in1=bcs[:, 0, :cw])
                    nc.vector.tensor_sub(out=h1T[:, c, c0:c0 + cw],
                                         in0=tmp[:, :cw], in1=bcs[:, 1, :cw])
                if ci > 0:
                    emit_qk8(ci - 1, 0)
            if STOP >= 2:
                emit_qk8(len(CHUNKS) - 1, 0)

            # K: reload the slab slots (overwrite waits on Q readers)
            if STOP >= 2:
                wks = [wqkp.tile([128, D], BF16, tag=f"wq{c}", name=f"wqk{c}")
                       for c in range(IC)]
                for c in range(IC):
                    nc.sync.dma_start(out=wks[c],
                                      in_=qkvwT_h[c * 128:(c + 1) * 128, D:2 * D])
                wqs = wks

            if STOP >= 2:
                for ci in range(len(CHUNKS)):
                    emit_qk8(ci, 8)

            rows1_cm.__exit__(None, None, None)
            ln1_nrm_cm.__exit__(None, None, None)
            ln1_sq_cm.__exit__(None, None, None)
            ln1_xb_cm.__exit__(None, None, None)
            wqk_cm.__exit__(None, None, None)

        # ---------------- Phase C: attention ----------------
        ctxT_cm = tc.tile_pool(name="ctxTp", bufs=1, side="right")
        ctxTp = ctxT_cm.__enter__()
        ctxT = [ctxTp.tile([128, T], BF16, tag=f"ctxT{k}", name=f"ctxT{k}")
                for k in range(IC)]

        attc_cm = tc.tile_pool(name="attc", bufs=1)
        attc = attc_cm.__enter__()
        eb = []
        for kt in range(2):
            kp = KTP[kt]
            t_ = attc.tile([128, 2, 8, N_TOK], BF16, tag=f"expb{kt}",
                           name=f"expb{kt}")
            nc.sync.dma_start(out=t_[:kp, :, :, :],
                              in_=expbT_h[kt * 128: kt * 128 + kp, :, :, :])
            eb.append(t_)
        # csel[:, h, :]: ones in column h (head-select for denominator MMs)
        csel = attc.tile([128, NH, NH], BF16, tag="csel", name="csel")
        nc.sync.dma_start(out=csel, in_=csel_h[:, :, :])
        # psel[:, c, :]: rec-row pair -> partition halves (rc broadcast)
        psel = attc.tile([16, IC, 128], BF16, tag="psel", name="psel")
        nc.sync.dma_start(out=psel, in_=psel_h[:, :, :])

        with tc.tile_pool(name="p_pool", bufs=4) as ppool, \
             tc.tile_pool(name="praw_pool", bufs=3) as prawp, \
             tc.tile_pool(name="rcb_pool", bufs=2) as rcbp, \
             tc.tile_pool(name="rcs_pool", bufs=3) as rcsp, \
             tc.tile_pool(name="sc_ps", bufs=2, space="PSUM") as scps, \
             tc.tile_pool(name="ctx_ps", bufs=2, space="PSUM") as ctxps, \
             tc.tile_pool(name="s16_ps", bufs=1, space="PSUM") as s16ps, \
             tc.tile_pool(name="rc_ps", bufs=1, space="PSUM") as rcps, \
             tc.tile_pool(name="cproj_ps", bufs=2, space="PSUM") as projps:

            pt = {}

            def emit_ctx(b):
                q0 = b * N_TOK
                # softmax denominators, all 16 heads -> one [16, 197] bank
                s16 = s16ps.tile([16, 256], F32, tag="s16", name="s16")
                n = 0
                for h in range(NH):
                    par, a = h % 2, h // 2
                    for kt in range(2):
                        kp = KTP[kt]
                        nc.tensor.matmul(
                            s16[0:NH, 0:N_TOK],
                            lhsT=csel[:kp, h, :],
                            rhs=pt[(b, kt)][:kp, par, a, :],
                            start=(n == 0), stop=(n == 31))
                        n += 1
                rcb = rcbp.tile([16, 256], F32, tag="rcb", name="rcb")
                nc.vector.reciprocal_approx_fast(out=rcb[0:NH, 0:N_TOK],
                                                 in_=s16[0:NH, 0:N_TOK])
                rcbb = rcbp.tile([16, 256], BF16, tag="rcbb", name="rcbb")
                nc.vector.tensor_copy(out=rcbb[0:NH, 0:N_TOK],
                                      in_=rcb[0:NH, 0:N_TOK])
                for c in range(IC):
                    # heads (2c, 2c+1) column-stacked into one psum bank
                    psc = ctxps.tile([128, 256], F32, tag="ctxps", name="ctxps")
                    for par in range(2):
                        h = 2 * c + par
                        for kt in range(2):
                            kp = KTP[kt]
                            nc.tensor.matmul(
                                psc[par * 64:(par + 1) * 64, 0:N_TOK],
                                lhsT=vt[(b, kt)][:kp, h, 0:64],
                                rhs=pt[(b, kt)][:kp, par, c, :],
                                start=(kt == 0), stop=(kt == 1))
                    rc = rcps.tile([128, 256], F32, tag="rcps2", name="rc")
                    nc.tensor.matmul(rc[0:128, 0:N_TOK],
                                     lhsT=psel[0:NH, c, :],
                                     rhs=rcbb[0:NH, 0:N_TOK],
                                     start=True, stop=True)
                    rcs = rcsp.tile([128, 256], F32, tag="rcs", name="rcs")
                    nc.vector.tensor_copy(out=rcs[:, 0:N_TOK],
                                          in_=rc[:, 0:N_TOK])
                    nc.vector.tensor_mul(out=ctxT[c][:, q0:q0 + N_TOK],
                                         in0=psc[0:128, 0:N_TOK],
                                         in1=rcs[0:128, 0:N_TOK])

            def emit_v(b):
                # V matmuls for sample b: dense K=128 full-array MMs keep the
                # HAM clock gate open during the attention phase.
                for kt in range(2):
                    t0 = b * N_TOK + kt * 128
                    kp = KTP[kt]
                    vtile = vt[(b, kt)]
                    for vc in range(2):
                        ps = projps.tile([128, 512], F32, tag="vps",
                                         name="vps")
                        for c in range(IC):
                            nc.tensor.matmul(
                                ps[:kp, :],
                                lhsT=h1T[:, c, t0:t0 + kp],
                                rhs=wv[c][:, vc * 512:(vc + 1) * 512],
                                start=(c == 0), stop=(c == IC - 1))
                        nc.vector.tensor_add(
                            out=vtile[:kp, vc * 8:(vc + 1) * 8, 0:64],
                            in0=ps[:kp, :].rearrange("p (a d) -> p a d", a=8),
                            in1=vb_t[:kp, vc * 512:(vc + 1) * 512].rearrange(
                                "p (a d) -> p a d", a=8))

            for b in range(BL):
                if STOP < 4:
                    break
                q0 = b * N_TOK
                for kt in range(2):
                    kp = KTP[kt]
                    k0 = q0 + kt * 128
                    ptile = ppool.tile([128, 2, 8, N_TOK], BF16, tag="P",
                                       name="P")
                    pt[(b, kt)] = ptile
                    # pair-tile (par, j) holds heads a=j and a=j+4 (same rb:
                    # mixed lhsT base partitions within one PSUM bank fault).
                    # Emission alternates par so consecutive MMs use opposite
                    # rb and LDWEIGHTS overlaps the in-flight matmul.
                    for j in range(4):
                        pss = [scps.tile([128, 2, 256], F32, tag="scps",
                                         name="scps") for _ in range(2)]
                        for s in range(2):
                            a = j + 4 * s
                            for par in range(2):
                                rb = par * 64
                                nc.tensor.matmul(
                                    pss[par][:kp, s, 0:N_TOK],
                                    lhsT=qkt[8 + a][rb:rb + 64, k0:k0 + kp],
                                    rhs=qkt[a][rb:rb + 64, q0:q0 + N_TOK],
                                    start=True, stop=True)
                        for par in range(2):
                            praw = prawp.tile([128, 2, N_TOK], BF16,
                                              tag="praw", name="praw")
                            nc.scalar.activation(out=praw[:kp, :, :],
                                                 in_=pss[par][:kp, :, 0:N_TOK],
                                                 func=AF.Exp)
                            nc.vector.tensor_mul(
                                out=ptile[:kp, par, j::4, :],
                                in0=praw[:kp, :, :],
                                in1=eb[kt][:kp, par, j::4, :])
                emit_v(b)
                if b > 0:
                    emit_ctx(b - 1)
            if STOP >= 4:
                emit_ctx(BL - 1)

        attc_cm.__exit__(None, None, None)
        wv_cm.__exit__(None, None, None)
        h1T_cm.__exit__(None, None, None)
        vt_cm.__exit__(None, None, None)
        qk_cm.__exit__(None, None, None)

        # ------------- Phase D: proj + residual + LN2 + x2->token-major -------
        fc2w_cm = tc.tile_pool(name="fc2w", bufs=3)
        fc2wsb = fc2w_cm.__enter__()
        fc1w_cm = tc.tile_pool(name="fc1w", bufs=3)
        fc1wp = fc1w_cm.__enter__()
        h2T_cm = tc.tile_pool(name="h2Tp", bufs=1)
        h2Tp = h2T_cm.__enter__()
        h2T = h2Tp.tile([128, IC, T], BF16, tag="h2T", name="h2T")
        xb2_cm = tc.tile_pool(name="xb2p", bufs=1)
        xb2p = xb2_cm.__enter__()

        rows2_cm = tc.tile_pool(name="rows2", bufs=1)
        rows2 = alloc_rows(rows2_cm.__enter__(), "2")
        r0, r1 = rows2['r0'], rows2['r1']
        abf, bbf = rows2['abf'], rows2['bbf']
        nc.vector.memset(r0[0:1, :], 0.0)
        nc.vector.memset(r1[0:1, :], 0.0)

        xb2 = []

        pw_cm = tc.tile_pool(name="pwp", bufs=1)
        pwp = pw_cm.__enter__()
        pw = [pwp.tile([128, D], BF16, tag=f"pw{c}", name=f"pw{c}")
              for c in range(IC)]
        for c in range(IC):
            nc.sync.dma_start(out=pw[c], in_=projwT_h[c * 128:(c + 1) * 128, :])

        with tc.tile_pool(name="xt2", bufs=2) as xt2p, \
             tc.tile_pool(name="x2p", bufs=2) as x2p, \
             tc.tile_pool(name="sq2p", bufs=2) as sq2p, \
             tc.tile_pool(name="proj_ps", bufs=2, space="PSUM") as projps2, \
             tc.tile_pool(name="st2_ps", bufs=4, space="PSUM") as st2ps:
            for ft in range(IC):
                if STOP < 5:
                    break
                xt2 = xt2p.tile([128, T], F32, tag="xt2", name="xt2")
                nc.gpsimd.dma_start(out=xt2, in_=xT_h[ft * 128:(ft + 1) * 128, :])
                x2T = x2p.tile([128, T], F32, tag="x2T", name="x2T")
                for ci, (c0, cw) in enumerate(CHUNKS):
                    ps = projps2.tile([128, 512], F32, tag="projps",
                                      name="projps")
                    for c in range(IC):
                        nc.tensor.matmul(
                            ps[:, :cw],
                            lhsT=pw[c][:, ft * 128:(ft + 1) * 128],
                            rhs=ctxT[c][:, c0:c0 + cw],
                            start=(c == 0), stop=(c == IC - 1))
                    nc.scalar.activation(out=x2T[:, c0:c0 + cw], in_=ps[:, :cw],
                                         func=AF.Identity,
                                         bias=projb_t[:, ft:ft + 1])
                nc.vector.tensor_add(out=x2T, in0=x2T, in1=xt2)
                xb2_f = xb2p.tile([128, T], BF16, tag=f"xb2_{ft}",
                                  name=f"xb2_{ft}")
                nc.vector.tensor_copy(out=xb2_f, in_=x2T)
                xb2.append(xb2_f)
                xsq2 = sq2p.tile([128, T], BF16, tag="xsq2", name="xsq2")
                nc.vector.tensor_mul(out=xsq2, in0=xb2_f, in1=xb2_f)
                for (src_t, accr) in ((xb2_f, r0), (xsq2, r1)):
                    for ci, (c0, cw) in enumerate(CHUNKS):
                        rb = 32 * ci
                        p1 = st2ps.tile([128, 512], F32, tag="st2", name="st2")
                        nc.tensor.matmul(p1[rb:rb + 1, :cw],
                                         lhsT=ones_col[:, 0:1],
                                         rhs=src_t[:, c0:c0 + cw],
                                         start=True, stop=True,
                                         tile_position=(0, rb))
                        nc.vector.tensor_add(out=accr[0:1, c0:c0 + cw],
                                             in0=accr[0:1, c0:c0 + cw],
                                             in1=p1[rb:rb + 1, :cw])
                # x2 + fc2_b -> feature-major DRAM scratch (read back in fc2)
                x2fb = x2p.tile([128, T], F32, tag="x2fb", name="x2fb")
                nc.scalar.activation(out=x2fb, in_=x2T, func=AF.Identity,
                                     bias=fc2b_t[:, ft:ft + 1])
                nc.sync.dma_start(out=x2s_h[ft * 128:(ft + 1) * 128, :],
                                  in_=x2fb)

        ctxT_cm.__exit__(None, None, None)
        pw_cm.__exit__(None, None, None)

        if STOP >= 5:
            emit_ln_rows(rows2,
                         [r0[0:1, c0:c0 + cw] for (c0, cw) in CHUNKS],
                         [r1[0:1, c0:c0 + cw] for (c0, cw) in CHUNKS])

        with tc.tile_pool(name="bc2_ps", bufs=2, space="PSUM") as bc2ps, \
             tc.tile_pool(name="nrm2", bufs=3) as nrm2p:
            for ci, (c0, cw) in enumerate(CHUNKS):
                if STOP < 5:
                    break
                bc = bc2ps.tile([128, 2, 512], F32, tag="bc2", name="bc2")
                nc.tensor.matmul(bc[:, 0, :cw], lhsT=ones_row[0:1, :],
                                 rhs=abf[0:1, c0:c0 + cw], start=True, stop=True)
                nc.tensor.matmul(bc[:, 1, :cw], lhsT=ones_row[0:1, :],
                                 rhs=bbf[0:1, c0:c0 + cw], start=True, stop=True)
                bcs = nrm2p.tile([128, 2, 512], BF16, tag="bcs2", name="bcs2")
                nc.scalar.activation(out=bcs[:, :, :cw], in_=bc[:, :, :cw],
                                     func=AF.Copy)
                for c in range(IC):
                    tmp = nrm2p.tile([128, 512], BF16, tag="n2tmp", name="n2tmp")
                    nc.vector.tensor_mul(out=tmp[:, :cw],
                                         in0=xb2[c][:, c0:c0 + cw],
                                         in1=bcs[:, 0, :cw])
                    nc.vector.tensor_sub(out=h2T[:, c, c0:c0 + cw],
                                         in0=tmp[:, :cw], in1=bcs[:, 1, :cw])
        rows2_cm.__exit__(None, None, None)
        xb2_cm.__exit__(None, None, None)

        # ---------------- Phase E: MLP ----------------
        gT_cm = tc.tile_pool(name="gT_pool", bufs=1, side="right")
        gTp = gT_cm.__enter__()
        gT = gTp.tile([128, 32, T], BF16, tag="gT", name="gT")
        identf = gTp.tile([128, 128], F32, tag="identf", name="identf")
        masks.make_identity(nc, identf[:, :])
        with tc.tile_pool(name="fc1_ps", bufs=4, space="PSUM") as fc1ps:
            for Ht in range(32):
                if STOP < 6:
                    break
                wt = fc1wp.tile([128, D], BF16, tag="fc1w", name="fc1w")
                nc.sync.dma_start(out=wt, in_=fc1wT_h[Ht, :, :])
                for j in range(2):           # super-chunks of 788 = 2x394
                    ps = fc1ps.tile([128, 2, 512], F32, tag="fc1ps",
                                    name="fc1ps")
                    for k in range(2):
                        c0, cw = ECHUNKS[j * 2 + k]
                        for c in range(IC):
                            nc.tensor.matmul(
                                ps[:, k, :cw],
                                lhsT=wt[:, c * 128:(c + 1) * 128],
                                rhs=h2T[:, c, c0:c0 + cw],
                                start=(c == 0), stop=(c == IC - 1))
                    nc.scalar.activation(
                        out=gT[:, Ht, j * 788:(j + 1) * 788],
                        in_=ps[:, :, 0:394],
                        func=AF.Gelu, bias=fc1b_t[:, Ht:Ht + 1])
        h2T_cm.__exit__(None, None, None)
        fc1w_cm.__exit__(None, None, None)

        # ---- fc2 feature-major: out^T[ft, t] = sum_H fc2w^T . gT ----
        with tc.tile_pool(name="xf_sb", bufs=2) as xfp, \
             tc.tile_pool(name="ot_sb", bufs=2) as otp, \
             tc.tile_pool(name="stg_sb", bufs=2) as stgp, \
             tc.tile_pool(name="fc2_ps", bufs=2, space="PSUM") as fc2ps:
            for ft in range(IC):
                if STOP < 7:
                    break
                w2 = fc2wsb.tile([128, 32, 128], BF16, tag="fc2w", name="fc2w")
                nc.sync.dma_start(out=w2, in_=fc2wp_h[ft, :, :, :])
                xf = xfp.tile([128, T], F32, tag="xf", name="xf")
                nc.gpsimd.dma_start(out=xf,
                                    in_=x2s_h[ft * 128:(ft + 1) * 128, :])
                ps = fc2ps.tile([128, 4, 512], F32, tag="eps", name="eps_mm")
                ot = otp.tile([128, T], F32, tag="ot", name="ot")
                for ci, (c0, cw) in enumerate(ECHUNKS):
                    for Hkt in range(32):
                        nc.tensor.matmul(
                            ps[:, ci, :cw],
                            lhsT=w2[:, Hkt, :],
                            rhs=gT[:, Hkt, c0:c0 + cw],
                            start=(Hkt == 0), stop=(Hkt == 31))
                    nc.vector.tensor_add(out=ot[:, c0:c0 + cw],
                                         in0=ps[:, ci, :cw],
                                         in1=xf[:, c0:c0 + cw])
                # transpose to token-major + drain + store
                tps = fc2ps.tile([128, 16, 128], F32, tag="eps", name="eps_tp")
                for tt, (t0, p) in enumerate(tok_tiles):
                    nc.tensor.transpose(tps[:p, tt, :], ot[:, t0:t0 + p],
                                        identf[:, :])
                stg = stgp.tile([128, 16, 128], F32, tag="stg", name="stg")
                nc.vector.tensor_copy(out=stg[:, 0:8, :], in_=tps[:, 0:8, :])
                nc.scalar.activation(out=stg[:, 8:12, :], in_=tps[:, 8:12, :],
                                     func=AF.Identity, bias=0.0)
                nc.vector.tensor_copy(out=stg[0:LASTP, 12, :],
                                      in_=tps[0:LASTP, 12, :])
                for tt, (t0, p) in enumerate(tok_tiles):
                    nc.gpsimd.dma_start(
                        out=out_h[t0:t0 + p, ft * 128:(ft + 1) * 128],
                        in_=stg[:p, tt, :])
        fc2w_cm.__exit__(None, None, None)
        gT_cm.__exit__(None, None, None)
        consts_cm.__exit__(None, None, None)
    _split_sync_waits(nc)
    from concourse.library_overlay import lower_extended_insts
    lower_extended_insts(nc)
    return nc


_CACHED_NC = None


def _get_nc():
    global _CACHED_NC
    if _CACHED_NC is None:
        _CACHED_NC = build_program()
    return _CACHED_NC


def prepare_host_inputs(x, qkv_w, q_bias, v_bias, rel_bias_table, proj_w, proj_b,
                        ln1_g, ln1_b, ln2_g, ln2_b, fc1_w, fc1_b, fc2_w, fc2_b):
    bf = ml_dtypes.bfloat16
    f32 = np.float32
    x = np.asarray(x, f32)

    # fold LN1 gamma/beta into qkv weights, scale q by 1/8
    qkv_b = np.concatenate([q_bias, np.zeros_like(v_bias), v_bias]).astype(f32)
    W1 = qkv_w.astype(f32) * ln1_g[None, :].astype(f32)
    b1 = qkv_b + qkv_w.astype(f32) @ ln1_b.astype(f32)
    W1[:D] *= SCALE
    b1[:D] *= SCALE
    qkvwT = np.ascontiguousarray(W1.T).astype(bf)            # [1024, 3072]
    qkb = np.ascontiguousarray(b1[:2 * D].reshape(16, 128).T).astype(f32)
    vb_rep = np.broadcast_to(b1[2 * D:], (128, D)).copy().astype(bf)

    idx = _make_rel_pos_index()
    rel = rel_bias_table.astype(f32)[idx]                    # [q, k, h]
    # expbT[k, par, a, q] = exp(rel[q, k, 2a+par]): exp(S+B) = exp(S)*exp(B)
    ebk = np.exp(rel.transpose(1, 2, 0))                     # [k, h, q]
    expbT = np.ascontiguousarray(
        ebk.reshape(N_TOK, 8, 2, N_TOK).transpose(0, 2, 1, 3)).astype(bf)

    projwT = np.ascontiguousarray(proj_w.astype(f32).T).astype(bf)
    projb = np.ascontiguousarray(proj_b.astype(f32).reshape(8, 128).T)
    fc2b = np.ascontiguousarray(fc2_b.astype(f32).reshape(8, 128).T)

    W3 = fc1_w.astype(f32) * ln2_g[None, :].astype(f32)
    b3 = fc1_b.astype(f32) + fc1_w.astype(f32) @ ln2_b.astype(f32)
    W3T = np.ascontiguousarray(W3.T)                         # [1024, 4096]
    fc1wT = W3T.reshape(8, 128, 32, 128).transpose(2, 1, 0, 3)
    fc1wT = np.ascontiguousarray(fc1wT.reshape(32, 128, D)).astype(bf)
    fc1b = np.ascontiguousarray(b3.reshape(32, 128).T).astype(f32)

    # fc2 packed: fc2wp[ft, p, k, j] = fc2_w[ft*128+j, k*128+p]
    fc2wp = fc2_w.astype(f32).reshape(8, 128, 32, 128)       # [ft, j, k, p]
    fc2wp = np.ascontiguousarray(fc2wp.transpose(0, 3, 2, 1)).astype(bf)

    csel = np.zeros((128, NH, NH), np.float32)
    for h in range(NH):
        csel[:, h, h] = 1.0
    csel = csel.astype(bf)
    psel = np.zeros((16, IC, 128), f32)
    for c in range(IC):
        psel[2 * c, c, 0:64] = 1.0
        psel[2 * c + 1, c, 64:128] = 1.0
    psel = psel.astype(bf)

    shared = dict(qkvwT=qkvwT, qkb=qkb, vb_rep=vb_rep, expbT=expbT,
                  csel=csel, psel=psel,
                  projwT=projwT, projb=projb, fc1wT=fc1wT, fc1b=fc1b,
                  fc2wp=fc2wp, fc2b=fc2b)
    in_maps = []
    for cid in range(NCORES):
        sl = slice(cid * BL, (cid + 1) * BL)
        m = dict(shared)
        xTc = np.ascontiguousarray(x[sl].reshape(T, D).T)
        m["xT"] = xTc
        m["xbT"] = xTc.astype(bf)
        in_maps.append(m)
    return in_maps


def kernel(**inputs):
    nc = _get_nc()
    in_maps = prepare_host_inputs(**inputs)
    res = run_bass_kernel_spmd(nc, in_maps, list(range(NCORES)))
    outs = [res.results[c]["out"].reshape(BL, N_TOK, D) for c in range(NCORES)]
    return np.concatenate(outs, axis=0).astype(np.float32)



# revision 83
# speedup vs baseline: 1.0524x; 1.0032x over previous
"""Trainium2 Bass kernel v2 for the ViT transformer block — feature-major dataflow.

Everything on-chip flows feature-major ([feat, tok]); the host pre-transposes
x to xT and the kernel writes token-major output via cheap PE transposes.
LayerNorm statistics come from ones-vector matmuls (partition-dim reduction on
the PE); the per-token scale/shift rows are broadcast across partitions with
K=1 outer-product matmuls into PSUM and applied with two DVE passes.
Attention keeps the scores^T/exp/ones-column layout of v1, but context tiles
leave the attention phase through PE transposes (identity matmul) instead of
serialized DMA-transposes.  x2 returns to token-major through f32 PE
transposes so the fc2 drain and final residual run exactly like v1.

Sharding: data-parallel over batch, 8 samples per core on 8 cores.
"""

import sys
import os

sys.path.insert(0, "/opt/trn_rl_repo")

import numpy as np
import ml_dtypes

import concourse.bass as bass
import concourse.tile as tile
from concourse import mybir
from concourse import masks
from concourse.vector_clock import ScopedClock
from concourse.bass_utils import run_bass_kernel_spmd

F32 = mybir.dt.float32
BF16 = mybir.dt.bfloat16
AF = mybir.ActivationFunctionType
ALU = mybir.AluOpType

B, N_TOK, D = 64, 197, 1024
NCORES = 8
BL = B // NCORES            # samples per core = 8
T = BL * N_TOK              # tokens per core = 1576
NH, HD = 16, 64
HID = 4096
SCALE = HD ** -0.5
WH = WW = 14
NUM_REL = (2 * WH - 1) * (2 * WW - 1) + 3
LN_EPS = 1e-5

IC = 8                       # in-feature chunks of 128
CHUNKS = [(i * 512, min(512, T - i * 512)) for i in range((T + 511) // 512)]
ECHUNKS = [(i * 394, 394) for i in range(4)]   # uniform fc1/fc2 chunks
KTP = [128, N_TOK - 128]     # per-sample key tile sizes [128, 69]
NT = (T + 127) // 128        # 13 token tiles
LASTP = T - 128 * (NT - 1)   # 40


def _tok_tiles():
    return [(t * 128, 128 if t < NT - 1 else LASTP) for t in range(NT)]


def _sample_tiles():
    out = []
    for b in range(BL):
        for kt in range(2):
            out.append((b, kt, b * N_TOK + kt * 128, KTP[kt]))
    return out


def _make_rel_pos_index():
    coords = np.stack(np.meshgrid(np.arange(WH), np.arange(WW), indexing="ij"))
    flat = coords.reshape(2, -1)
    rel = flat[:, :, None] - flat[:, None, :]
    rel = rel.transpose(1, 2, 0).copy()
    rel[:, :, 0] += WH - 1
    rel[:, :, 1] += WW - 1
    rel[:, :, 0] *= 2 * WW - 1
    idx = np.zeros((N_TOK, N_TOK), dtype=np.int32)
    idx[1:, 1:] = rel.sum(-1)
    idx[0, 0:] = NUM_REL - 3
    idx[0:, 0] = NUM_REL - 2
    idx[0, 0] = NUM_REL - 1
    return idx


class SplitDrainTileContext(tile.TileContext):
    """Walrus in this toolchain rejects >1 sync-wait on the kernel-tail
    Drain; split the waits across a chain of drain instructions."""

    def _drain_and_barrier(self, tick_clock, wait_clock):
        drain_inst = self.nc.sync.drain()
        wait_clock.add_sem_waits(
            drain_inst.ins, ScopedClock({None: tick_clock.global_clock})
        )
        si = drain_inst.ins.sync_info
        waits = list(si.on_wait) if si and si.on_wait else []
        if len(waits) > 1:
            si.on_wait = waits[:1]
            for w in waits[1:]:
                d2 = self.nc.sync.drain()
                si2 = d2.ins.sync_info
                if si2 is None:
                    d2.ins.sync_info = mybir.SyncInfo(on_wait=[w], on_update=[])
                else:
                    si2.on_wait = [w]
        self.nc.all_engine_barrier()
        assert self.sems is not None
        popped = self.nc._tile_sem_poison_stack.pop()
        assert popped is self._sem_poison
        self.nc.clear_and_free_semaphores(list(self.sems.allocated().values()))
        self.nc.all_engine_barrier()


def _split_sync_waits(nc, cap=1):
    """Hoist excess sync-waits onto standalone event-semaphore instructions."""
    n = 0
    for fn in nc.m.functions:
        for bb in fn.blocks:
            insts = bb.instructions
            i = 0
            while i < len(insts):
                inst = insts[i]
                si = inst.sync_info
                waits = list(si.on_wait) if si and si.on_wait else []
                if len(waits) > cap and inst.engine != mybir.EngineType.Unassigned:
                    excess = waits[:len(waits) - cap]
                    si.on_wait = waits[len(waits) - cap:]
                    for w in excess:
                        ev = mybir.InstEventSemaphore(
                            name=f"waitsplit_{n}", ins=[], outs=[],
                            sync_info=mybir.SyncInfo(on_wait=[w], on_update=[]))
                        ev.engine = inst.engine
                        nc.register_instruction(ev)
                        insts.insert(i, ev)
                        n += 1
                        i += 1
                i += 1
    return n


def build_program():
    STOP = int(os.environ.get("K2_STOP", "7"))
    ASUB = os.environ.get("K2_ATTN_SUB", "full")
    NB = int(os.environ.get("K2_NB", str(BL)))
    NOEXP = os.environ.get("K2_NOEXP", "0") == "1"
    NOADD = os.environ.get("K2_NOADD", "0") == "1"
    NG = int(os.environ.get("K2_NG", "8"))
    NGI = int(os.environ.get("K2_NGI", "2"))
    nc = bass.Bass("TRN2", target_bir_lowering=False, debug=False,
                   num_devices=NCORES)

    # ---- DRAM I/O ----
    xT_h = nc.declare_dram_parameter("xT", [D, T], F32, isOutput=False)
    xbT_h = nc.declare_dram_parameter("xbT", [D, T], BF16, isOutput=False)
    qkvwT_h = nc.declare_dram_parameter("qkvwT", [D, 3 * D], BF16, isOutput=False)
    qkb_h = nc.declare_dram_parameter("qkb", [128, 16], F32, isOutput=False)
    vb_h = nc.declare_dram_parameter("vb_rep", [128, D], BF16, isOutput=False)
    expbT_h = nc.declare_dram_parameter("expbT", [N_TOK, 2, 8, N_TOK], BF16,
                                        isOutput=False)
    csel_h = nc.declare_dram_parameter("csel", [128, NH, NH], BF16,
                                       isOutput=False)
    psel_h = nc.declare_dram_parameter("psel", [16, IC, 128], BF16,
                                       isOutput=False)
    projwT_h = nc.declare_dram_parameter("projwT", [D, D], BF16, isOutput=False)
    projb_h = nc.declare_dram_parameter("projb", [128, 8], F32, isOutput=False)
    fc1wT_h = nc.declare_dram_parameter("fc1wT", [32, 128, D], BF16, isOutput=False)
    fc1b_h = nc.declare_dram_parameter("fc1b", [128, 32], F32, isOutput=False)
    fc2wp_h = nc.declare_dram_parameter("fc2wp", [IC, 128, 32, 128], BF16,
                                        isOutput=False)
    fc2b_h = nc.declare_dram_parameter("fc2b", [128, 8], F32, isOutput=False)
    out_h = nc.declare_dram_parameter("out", [T, D], F32, isOutput=True)
    x2s_h = nc.dram_tensor("x2s", [D, T], F32)   # x2 + fc2_b, feature-major

    tok_tiles = _tok_tiles()
    samp_tiles = _sample_tiles()

    with SplitDrainTileContext(nc) as tc:
        # ---------- right-side stack: consts > {ctxT | gT} ----------
        consts_cm = tc.tile_pool(name="consts", bufs=1, side="right")
        consts = consts_cm.__enter__()
        identb = consts.tile([128, 128], BF16, tag="identb", name="identb")
        masks.make_identity(nc, identb[:, :])
        ones_col = consts.tile([128, 1], BF16, tag="ones_col", name="ones_col")
        nc.vector.memset(ones_col, 1.0)
        ones_row = consts.tile([1, 128], BF16, tag="ones_row", name="ones_row")
        nc.vector.memset(ones_row, 1.0)
        qkb_t = consts.tile([128, 16], F32, tag="qkb", name="qkb")
        nc.sync.dma_start(out=qkb_t, in_=qkb_h[:, :])
        vb_t = consts.tile([128, D], BF16, tag="vb", name="vb")
        nc.sync.dma_start(out=vb_t, in_=vb_h[:, :])
        projb_t = consts.tile([128, 8], F32, tag="projb", name="projb")
        nc.sync.dma_start(out=projb_t, in_=projb_h[:, :])
        fc2b_t = consts.tile([128, 8], F32, tag="fc2b", name="fc2b")
        nc.sync.dma_start(out=fc2b_t, in_=fc2b_h[:, :])
        fc1b_t = consts.tile([128, 32], F32, tag="fc1b", name="fc1b")
        nc.sync.dma_start(out=fc1b_t, in_=fc1b_h[:, :])
        epsr = consts.tile([1, 1], F32, tag="epsr", name="epsr")
        nc.vector.memset(epsr, LN_EPS)

        def alloc_rows(pool, sfx):
            return dict(
                r0=pool.tile([1, T], F32, tag=f"row0{sfx}", name=f"row0{sfx}"),
                r1=pool.tile([1, T], F32, tag=f"row1{sfx}", name=f"row1{sfx}"),
                r2=pool.tile([1, T], F32, tag=f"row2{sfx}", name=f"row2{sfx}"),
                abf=pool.tile([1, T], BF16, tag=f"ra{sfx}", name=f"ra{sfx}"),
                bbf=pool.tile([1, T], BF16, tag=f"rb{sfx}", name=f"rb{sfx}"))

        def emit_ln_rows(rows, s1_ap_chunks, s2_ap_chunks):
            """a=rsqrt(var+eps) -> abf; b=mu*a -> bbf (normalize SUBTRACTS b).
            Chunk-pipelined so downstream bc/normalize of chunk 0 starts
            early.  r0..r2 scratch; sources may alias r0/r1."""
            r0, r1, r2 = rows['r0'], rows['r1'], rows['r2']
            abf, bbf = rows['abf'], rows['bbf']
            for ci, (c0, cw) in enumerate(CHUNKS):
                sl = slice(c0, c0 + cw)
                nc.scalar.activation(out=r2[0:1, sl], in_=s1_ap_chunks[ci],
                                     func=AF.Copy, scale=1.0 / D)   # mu
                nc.scalar.activation(out=r0[0:1, sl], in_=s2_ap_chunks[ci],
                                     func=AF.Copy, scale=1.0 / D,
                                     bias=float(LN_EPS))            # m2 + eps
                nc.vector.tensor_mul(out=r1[0:1, sl], in0=r2[0:1, sl],
                                     in1=r2[0:1, sl])
                nc.vector.tensor_sub(out=r0[0:1, sl], in0=r0[0:1, sl],
                                     in1=r1[0:1, sl])               # var + eps
                nc.vector.reciprocal_approx_fast(out=r1[0:1, sl],
                                                 in_=r0[0:1, sl])
                nc.scalar.activation(out=abf[0:1, sl], in_=r1[0:1, sl],
                                     func=AF.Sqrt)                  # rsqrt
                nc.vector.tensor_mul(out=bbf[0:1, sl], in0=r2[0:1, sl],
                                     in1=abf[0:1, sl])

        # ---------- left-side stack: attention superstructure ----------
        qk_cm = tc.tile_pool(name="qkp", bufs=1)
        qkp = qk_cm.__enter__()
        qkt = [qkp.tile([128, T], BF16, tag=f"qkt{ft}", name=f"qkt{ft}")
               for ft in range(16)]
        vt_cm = tc.tile_pool(name="vtp", bufs=1)
        vtp = vt_cm.__enter__()
        vt = {}
        for (b, kt, t0, kp) in samp_tiles:
            vt[(b, kt)] = vtp.tile([128, NH, 65], BF16, tag=f"v{b}_{kt}",
                                   name=f"v{b}_{kt}")
        h1T_cm = tc.tile_pool(name="h1Tp", bufs=1)
        h1Tp = h1T_cm.__enter__()
        h1T = h1Tp.tile([128, IC, T], BF16, tag="h1T", name="h1T")
        wv_cm = tc.tile_pool(name="wvp", bufs=1)
        wvp = wv_cm.__enter__()
        wv = [wvp.tile([128, D], BF16, tag=f"wv{c}", name=f"wv{c}")
              for c in range(IC)]
        wqk_cm = tc.tile_pool(name="wqk", bufs=1)
        wqkp = wqk_cm.__enter__()

        # ---------------- Phase A: load xT, LN1 stats ----------------
        ln1_xb_cm = tc.tile_pool(name="ln1_xb", bufs=1)
        xbp = ln1_xb_cm.__enter__()
        ln1_sq_cm = tc.tile_pool(name="ln1_sq", bufs=2)
        sqp = ln1_sq_cm.__enter__()
        ln1_nrm_cm = tc.tile_pool(name="nrm_tmp", bufs=2)
        nrmp = ln1_nrm_cm.__enter__()

        wqs = [wqkp.tile([128, D], BF16, tag=f"wq{c}", name=f"wq{c}")
               for c in range(IC)]

        rows1_cm = tc.tile_pool(name="rows1", bufs=1)
        rows1 = alloc_rows(rows1_cm.__enter__(), "1")
        abf, bbf = rows1['abf'], rows1['bbf']

        xb = []
        with tc.tile_pool(name="st1_ps", bufs=1, space="PSUM") as st1ps:
            # stat rows live at partition 32*ci so the four chunk-MMs hit
            # distinct PE column groups and run concurrently
            s1p = st1ps.tile([128, 4, 512], F32, tag="s1p", name="s1p")
            s2p = st1ps.tile([128, 4, 512], F32, tag="s2p", name="s2p")
            for c in range(IC):
                xb_cc = []
                for ci, (c0, cw) in enumerate(CHUNKS):
                    xt = xbp.tile([128, 512], BF16, tag=f"xb{c}_{ci}",
                                  name=f"xb{c}_{ci}")
                    nc.sync.dma_start(out=xt[:, :cw],
                                      in_=xbT_h[c * 128:(c + 1) * 128,
                                                c0:c0 + cw])
                    xb_cc.append(xt)
                xb.append(xb_cc)
                for ci, (c0, cw) in enumerate(CHUNKS):
                    rb = 32 * ci
                    xsq = sqp.tile([128, 512], BF16, tag="xsq", name="xsq")
                    nc.vector.tensor_mul(out=xsq[:, :cw],
                                         in0=xb_cc[ci][:, :cw],
                                         in1=xb_cc[ci][:, :cw])
                    nc.tensor.matmul(s1p[rb:rb + 1, ci, :cw],
                                     lhsT=ones_col[:, 0:1],
                                     rhs=xb_cc[ci][:, :cw],
                                     start=(c == 0), stop=(c == IC - 1),
                                     tile_position=(0, rb))
                    nc.tensor.matmul(s2p[rb:rb + 1, ci, :cw],
                                     lhsT=ones_col[:, 0:1],
                                     rhs=xsq[:, :cw],
                                     start=(c == 0), stop=(c == IC - 1),
                                     tile_position=(0, rb))
            for c in range(IC):          # Q weight slabs (after xT on the queue)
                nc.sync.dma_start(out=wqs[c],
                                  in_=qkvwT_h[c * 128:(c + 1) * 128, 0:D])
            for c in range(IC):          # V weight slabs (prefetch)
                nc.sync.dma_start(out=wv[c],
                                  in_=qkvwT_h[c * 128:(c + 1) * 128, 2 * D:3 * D])
            emit_ln_rows(rows1,
                         [s1p[32 * ci:32 * ci + 1, ci, :cw]
                          for ci, (c0, cw) in enumerate(CHUNKS)],
                         [s2p[32 * ci:32 * ci + 1, ci, :cw]
                          for ci, (c0, cw) in enumerate(CHUNKS)])

        # ---------- Phase B: normalize + Q (pipelined), K, V ----------
        with tc.tile_pool(name="bc_ps", bufs=2, space="PSUM") as bcps, \
             tc.tile_pool(name="qk_ps", bufs=4, space="PSUM") as qkps_pool:

            def emit_qk8(ci, base_ft):
                c0, cw = CHUNKS[ci]
                for fi in range(8):
                    ft = base_ft + fi
                    ps = qkps_pool.tile([128, 512], F32, tag="qkps", name="qkps")
                    for c in range(IC):
                        nc.tensor.matmul(
                            ps[:, :cw],
                            lhsT=wqs[c][:, fi * 128:(fi + 1) * 128],
                            rhs=h1T[:, c, c0:c0 + cw],
                            start=(c == 0), stop=(c == IC - 1))
                    if ft % 2 == 0:
                        nc.scalar.activation(out=qkt[ft][:, c0:c0 + cw],
                                             in_=ps[:, :cw], func=AF.Identity,
                                             bias=qkb_t[:, ft:ft + 1])
                    else:
                        nc.vector.tensor_add(
                            out=qkt[ft][:, c0:c0 + cw], in0=ps[:, :cw],
                            in1=qkb_t[:, ft:ft + 1].broadcast_to([128, cw]))

            for ci, (c0, cw) in enumerate(CHUNKS):
                if STOP < 2:
                    break
                bc = bcps.tile([128, 2, 512], F32, tag="bc", name="bc")
                nc.tensor.matmul(bc[:, 0, :cw], lhsT=ones_row[0:1, :],
                                 rhs=abf[0:1, c0:c0 + cw], start=True, stop=True)
                nc.tensor.matmul(bc[:, 1, :cw], lhsT=ones_row[0:1, :],
                                 rhs=bbf[0:1, c0:c0 + cw], start=True, stop=True)
                bcs = nrmp.tile([128, 2, 512], BF16, tag="bcs", name="bcs")
                nc.scalar.activation(out=bcs[:, :, :cw], in_=bc[:, :, :cw],
                                     func=AF.Copy)
                for c in range(IC):
                    tmp = nrmp.tile([128, 512], BF16, tag="ntmp", name="ntmp")
                    nc.vector.tensor_mul(out=tmp[:, :cw], in0=xb[c][ci][:, :cw],
                                         in1=bcs[:, 0, :cw])
                    nc.vector.tensor_sub(out=h1T[:, c, c0:c0 + cw],
                                         in0=tmp[:, :cw], in1=bcs[:, 1, :cw])
                if ci > 0:
                    emit_qk8(ci - 1, 0)
            if STOP >= 2:
                emit_qk8(len(CHUNKS) - 1, 0)

            # K: reload the slab slots (overwrite waits on Q readers)
            if STOP >= 2:
                wks = [wqkp.tile([128, D], BF16, tag=f"wq{c}", name=f"wqk{c}")
                       for c in range(IC)]
                for c in range(IC):
                    nc.sync.dma_start(out=wks[c],
                                      in_=qkvwT_h[c * 128:(c + 1) * 128, D:2 * D])
                wqs = wks

            if STOP >= 2:
                for ci in range(len(CHUNKS)):
                    emit_qk8(ci, 8)

            rows1_cm.__exit__(None, None, None)
            ln1_nrm_cm.__exit__(None, None, None)
            ln1_sq_cm.__exit__(None, None, None)
            ln1_xb_cm.__exit__(None, None, None)
            wqk_cm.__exit__(None, None, None)

        # ---------------- Phase C: attention ----------------
        ctxT_cm = tc.tile_pool(name="ctxTp", bufs=1, side="right")
        ctxTp = ctxT_cm.__enter__()
        ctxT = [ctxTp.tile([128, T], BF16, tag=f"ctxT{k}", name=f"ctxT{k}")
                for k in range(IC)]

        attc_cm = tc.tile_pool(name="attc", bufs=1)
        attc = attc_cm.__enter__()
        eb = []
        for kt in range(2):
            kp = KTP[kt]
            t_ = attc.tile([128, 2, 8, N_TOK], BF16, tag=f"expb{kt}",
                           name=f"expb{kt}")
            nc.sync.dma_start(out=t_[:kp, :, :, :],
                              in_=expbT_h[kt * 128: kt * 128 + kp, :, :, :])
            eb.append(t_)
        # csel[:, h, :]: ones in column h (head-select for denominator MMs)
        csel = attc.tile([128, NH, NH], BF16, tag="csel", name="csel")
        nc.sync.dma_start(out=csel, in_=csel_h[:, :, :])
        # psel[:, c, :]: rec-row pair -> partition halves (rc broadcast)
        psel = attc.tile([16, IC, 128], BF16, tag="psel", name="psel")
        nc.sync.dma_start(out=psel, in_=psel_h[:, :, :])

        with tc.tile_pool(name="p_pool", bufs=4) as ppool, \
             tc.tile_pool(name="praw_pool", bufs=3) as prawp, \
             tc.tile_pool(name="rcb_pool", bufs=2) as rcbp, \
             tc.tile_pool(name="rcs_pool", bufs=3) as rcsp, \
             tc.tile_pool(name="sc_ps", bufs=2, space="PSUM") as scps, \
             tc.tile_pool(name="ctx_ps", bufs=2, space="PSUM") as ctxps, \
             tc.tile_pool(name="s16_ps", bufs=1, space="PSUM") as s16ps, \
             tc.tile_pool(name="rc_ps", bufs=1, space="PSUM") as rcps, \
             tc.tile_pool(name="cproj_ps", bufs=2, space="PSUM") as projps:

            pt = {}

            def emit_ctx(b):
                q0 = b * N_TOK
                # softmax denominators, all 16 heads -> one [16, 197] bank
                s16 = s16ps.tile([16, 256], F32, tag="s16", name="s16")
                n = 0
                for h in range(NH):
                    par, a = h % 2, h // 2
                    for kt in range(2):
                        kp = KTP[kt]
                        nc.tensor.matmul(
                            s16[0:NH, 0:N_TOK],
                            lhsT=csel[:kp, h, :],
                            rhs=pt[(b, kt)][:kp, par, a, :],
                            start=(n == 0), stop=(n == 31))
                        n += 1
                rcb = rcbp.tile([16, 256], F32, tag="rcb", name="rcb")
                nc.vector.reciprocal_approx_fast(out=rcb[0:NH, 0:N_TOK],
                                                 in_=s16[0:NH, 0:N_TOK])
                rcbb = rcbp.tile([16, 256], BF16, tag="rcbb", name="rcbb")
                nc.vector.tensor_copy(out=rcbb[0:NH, 0:N_TOK],
                                      in_=rcb[0:NH, 0:N_TOK])
                for c in range(IC):
                    # heads (2c, 2c+1) column-stacked into one psum bank
                    psc = ctxps.tile([128, 256], F32, tag="ctxps", name="ctxps")
                    for par in range(2):
                        h = 2 * c + par
                        for kt in range(2):
                            kp = KTP[kt]
                            nc.tensor.matmul(
                                psc[par * 64:(par + 1) * 64, 0:N_TOK],
                                lhsT=vt[(b, kt)][:kp, h, 0:64],
                                rhs=pt[(b, kt)][:kp, par, c, :],
                                start=(kt == 0), stop=(kt == 1))
                    rc = rcps.tile([128, 256], F32, tag="rcps2", name="rc")
                    nc.tensor.matmul(rc[0:128, 0:N_TOK],
                                     lhsT=psel[0:NH, c, :],
                                     rhs=rcbb[0:NH, 0:N_TOK],
                                     start=True, stop=True)
                    rcs = rcsp.tile([128, 256], F32, tag="rcs", name="rcs")
                    nc.vector.tensor_copy(out=rcs[:, 0:N_TOK],
                                          in_=rc[:, 0:N_TOK])
                    nc.vector.tensor_mul(out=ctxT[c][:, q0:q0 + N_TOK],
                                         in0=psc[0:128, 0:N_TOK],
                                         in1=rcs[0:128, 0:N_TOK])

            def emit_v(b):
                # V matmuls for sample b: dense K=128 full-array MMs keep the
                # HAM clock gate open during the attention phase.
                for kt in range(2):
                    t0 = b * N_TOK + kt * 128
                    kp = KTP[kt]
                    vtile = vt[(b, kt)]
                    for vc in range(2):
                        ps = projps.tile([128, 512], F32, tag="vps",
                                         name="vps")
                        for c in range(IC):
                            nc.tensor.matmul(
                                ps[:kp, :],
                                lhsT=h1T[:, c, t0:t0 + kp],
                                rhs=wv[c][:, vc * 512:(vc + 1) * 512],
                                start=(c == 0), stop=(c == IC - 1))
                        nc.vector.tensor_add(
                            out=vtile[:kp, vc * 8:(vc + 1) * 8, 0:64],
                            in0=ps[:kp, :].rearrange("p (a d) -> p a d", a=8),
                            in1=vb_t[:kp, vc * 512:(vc + 1) * 512].rearrange(
                                "p (a d) -> p a d", a=8))

            for b in range(BL):
                if STOP < 4:
                    break
                q0 = b * N_TOK
                for kt in range(2):
                    kp = KTP[kt]
                    k0 = q0 + kt * 128
                    ptile = ppool.tile([128, 2, 8, N_TOK], BF16, tag="P",
                                       name="P")
                    pt[(b, kt)] = ptile
                    # pair-tile (par, j) holds heads a=j and a=j+4 (same rb:
                    # mixed lhsT base partitions within one PSUM bank fault).
                    # Emission alternates par so consecutive MMs use opposite
                    # rb and LDWEIGHTS overlaps the in-flight matmul.
                    for j in range(4):
                        pss = [scps.tile([128, 2, 256], F32, tag="scps",
                                         name="scps") for _ in range(2)]
                        for s in range(2):
                            a = j + 4 * s
                            for par in range(2):
                                rb = par * 64
                                nc.tensor.matmul(
                                    pss[par][:kp, s, 0:N_TOK],
                                    lhsT=qkt[8 + a][rb:rb + 64, k0:k0 + kp],
                                    rhs=qkt[a][rb:rb + 64, q0:q0 + N_TOK],
                                    start=True, stop=True)
                        for par in range(2):
                            praw = prawp.tile([128, 2, N_TOK], BF16,
                                              tag="praw", name="praw")
                            nc.scalar.activation(out=praw[:kp, :, :],
                                                 in_=pss[par][:kp, :, 0:N_TOK],
                                                 func=AF.Exp)
                            nc.vector.tensor_mul(
                                out=ptile[:kp, par, j::4, :],
                                in0=praw[:kp, :, :],
                                in1=eb[kt][:kp, par, j::4, :])
                emit_v(b)
                if b > 0:
                    emit_ctx(b - 1)
            if STOP >= 4:
                emit_ctx(BL - 1)

        attc_cm.__exit__(None, None, None)
        wv_cm.__exit__(None, None, None)
        h1T_cm.__exit__(None, None, None)
        vt_cm.__exit__(None, None, None)
        qk_cm.__exit__(None, None, None)

        # ------------- Phase D: proj + residual + LN2 + x2->token-major -------
        fc2w_cm = tc.tile_pool(name="fc2w", bufs=3)
        fc2wsb = fc2w_cm.__enter__()
        fc1w_cm = tc.tile_pool(name="fc1w", bufs=3)
        fc1wp = fc1w_cm.__enter__()
        h2T_cm = tc.tile_pool(name="h2Tp", bufs=1)
        h2Tp = h2T_cm.__enter__()
        h2T = h2Tp.tile([128, IC, T], BF16, tag="h2T", name="h2T")
        xb2_cm = tc.tile_pool(name="xb2p", bufs=1)
        xb2p = xb2_cm.__enter__()

        rows2_cm = tc.tile_pool(name="rows2", bufs=1)
        rows2 = alloc_rows(rows2_cm.__enter__(), "2")
        r0, r1 = rows2['r0'], rows2['r1']
        abf, bbf = rows2['abf'], rows2['bbf']
        nc.vector.memset(r0[0:1, :], 0.0)
        nc.vector.memset(r1[0:1, :], 0.0)

        xb2 = []

        pw_cm = tc.tile_pool(name="pwp", bufs=1)
        pwp = pw_cm.__enter__()
        pw = [pwp.tile([128, D], BF16, tag=f"pw{c}", name=f"pw{c}")
              for c in range(IC)]
        for c in range(IC):
            nc.sync.dma_start(out=pw[c], in_=projwT_h[c * 128:(c + 1) * 128, :])

        with tc.tile_pool(name="xt2", bufs=2) as xt2p, \
             tc.tile_pool(name="x2p", bufs=2) as x2p, \
             tc.tile_pool(name="sq2p", bufs=2) as sq2p, \
             tc.tile_pool(name="proj_ps", bufs=2, space="PSUM") as projps2, \
             tc.tile_pool(name="st2_ps", bufs=4, space="PSUM") as st2ps:
            for ft in range(IC):
                if STOP < 5:
                    break
                xt2 = xt2p.tile([128, T], F32, tag="xt2", name="xt2")
                nc.gpsimd.dma_start(out=xt2, in_=xT_h[ft * 128:(ft + 1) * 128, :])
                x2T = x2p.tile([128, T], F32, tag="x2T", name="x2T")
                for ci, (c0, cw) in enumerate(CHUNKS):
                    ps = projps2.tile([128, 512], F32, tag="projps",
                                      name="projps")
                    for c in range(IC):
                        nc.tensor.matmul(
                            ps[:, :cw],
                            lhsT=pw[c][:, ft * 128:(ft + 1) * 128],
                            rhs=ctxT[c][:, c0:c0 + cw],
                            start=(c == 0), stop=(c == IC - 1))
                    nc.scalar.activation(out=x2T[:, c0:c0 + cw], in_=ps[:, :cw],
                                         func=AF.Identity,
                                         bias=projb_t[:, ft:ft + 1])
                nc.vector.tensor_add(out=x2T, in0=x2T, in1=xt2)
                xb2_f = xb2p.tile([128, T], BF16, tag=f"xb2_{ft}",
                                  name=f"xb2_{ft}")
                nc.vector.tensor_copy(out=xb2_f, in_=x2T)
                xb2.append(xb2_f)
                xsq2 = sq2p.tile([128, T], BF16, tag="xsq2", name="xsq2")
                nc.vector.tensor_mul(out=xsq2, in0=xb2_f, in1=xb2_f)
                for (src_t, accr) in ((xb2_f, r0), (xsq2, r1)):
                    for ci, (c0, cw) in enumerate(CHUNKS):
                        rb = 32 * ci
                        p1 = st2ps.tile([128, 512], F32, tag="st2", name="st2")
                        nc.tensor.matmul(p1[rb:rb + 1, :cw],
                                         lhsT=ones_col[:, 0:1],
                                         rhs=src_t[:, c0:c0 + cw],
                                         start=True, stop=True,
                                         tile_position=(0, rb))
                        nc.vector.tensor_add(out=accr[0:1, c0:c0 + cw],
                                             in0=accr[0:1, c0:c0 + cw],
                                             in1=p1[rb:rb + 1, :cw])
                # x2 + fc2_b -> feature-major DRAM scratch (read back in fc2)
                x2fb = x2p.tile([128, T], F32, tag="x2fb", name="x2fb")
                nc.scalar.activation(out=x2fb, in_=x2T, func=AF.Identity,
                                     bias=fc2b_t[:, ft:ft + 1])
                nc.sync.dma_start(out=x2s_h[ft * 128:(ft + 1) * 128, :],
                                  in_=x2fb)

        ctxT_cm.__exit__(None, None, None)
        pw_cm.__exit__(None, None, None)

        if STOP >= 5:
            emit_ln_rows(rows2,
                         [r0[0:1, c0:c0 + cw] for (c0, cw) in CHUNKS],
                         [r1[0:1, c0:c0 + cw] for (c0, cw) in CHUNKS])

        with tc.tile_pool(name="bc2_ps", bufs=2, space="PSUM") as bc2ps, \
             tc.tile_pool(name="nrm2", bufs=3) as nrm2p:
            for ci, (c0, cw) in enumerate(CHUNKS):
                if STOP < 5:
                    break
                bc = bc2ps.tile([128, 2, 512], F32, tag="bc2", name="bc2")
                nc.tensor.matmul(bc[:, 0, :cw], lhsT=ones_row[0:1, :],
                                 rhs=abf[0:1, c0:c0 + cw], start=True, stop=True)
                nc.tensor.matmul(bc[:, 1, :cw], lhsT=ones_row[0:1, :],
                                 rhs=bbf[0:1, c0:c0 + cw], start=True, stop=True)
                bcs = nrm2p.tile([128, 2, 512], BF16, tag="bcs2", name="bcs2")
                nc.scalar.activation(out=bcs[:, :, :cw], in_=bc[:, :, :cw],
                                     func=AF.Copy)
                for c in range(IC):
                    tmp = nrm2p.tile([128, 512], BF16, tag="n2tmp", name="n2tmp")
                    nc.vector.tensor_mul(out=tmp[:, :cw],
                                         in0=xb2[c][:, c0:c0 + cw],
                                         in1=bcs[:, 0, :cw])
                    nc.vector.tensor_sub(out=h2T[:, c, c0:c0 + cw],
                                         in0=tmp[:, :cw], in1=bcs[:, 1, :cw])
        rows2_cm.__exit__(None, None, None)
        xb2_cm.__exit__(None, None, None)

        # ---------------- Phase E: MLP ----------------
        gT_cm = tc.tile_pool(name="gT_pool", bufs=1, side="right")
        gTp = gT_cm.__enter__()
        gT = gTp.tile([128, 32, T], BF16, tag="gT", name="gT")
        identf = gTp.tile([128, 128], F32, tag="identf", name="identf")
        masks.make_identity(nc, identf[:, :])
        with tc.tile_pool(name="fc1_ps", bufs=4, space="PSUM") as fc1ps:
            for Ht in range(32):
                if STOP < 6:
                    break
                wt = fc1wp.tile([128, D], BF16, tag="fc1w", name="fc1w")
                nc.sync.dma_start(out=wt, in_=fc1wT_h[Ht, :, :])
                for j in range(2):           # super-chunks of 788 = 2x394
                    ps = fc1ps.tile([128, 2, 512], F32, tag="fc1ps",
                                    name="fc1ps")
                    for k in range(2):
                        c0, cw = ECHUNKS[j * 2 + k]
                        for c in range(IC):
                            nc.tensor.matmul(
                                ps[:, k, :cw],
                                lhsT=wt[:, c * 128:(c + 1) * 128],
                                rhs=h2T[:, c, c0:c0 + cw],
                                start=(c == 0), stop=(c == IC - 1))
                    nc.scalar.activation(
                        out=gT[:, Ht, j * 788:(j + 1) * 788],
                        in_=ps[:, :, 0:394],
                        func=AF.Gelu, bias=fc1b_t[:, Ht:Ht + 1])
        h2T_cm.__exit__(None, None, None)
        fc1w_cm.__exit__(None, None, None)

        # ---- fc2 feature-major: out^T[ft, t] = sum_H fc2w^T . gT ----
        with tc.tile_pool(name="xf_sb", bufs=2) as xfp, \
             tc.tile_pool(name="ot_sb", bufs=2) as otp, \
             tc.tile_pool(name="stg_sb", bufs=2) as stgp, \
             tc.tile_pool(name="fc2_ps", bufs=2, space="PSUM") as fc2ps:
            for ft in range(IC):
                if STOP < 7:
                    break
                w2 = fc2wsb.tile([128, 32, 128], BF16, tag="fc2w", name="fc2w")
                nc.sync.dma_start(out=w2, in_=fc2wp_h[ft, :, :, :])
                xf = xfp.tile([128, T], F32, tag="xf", name="xf")
                nc.gpsimd.dma_start(out=xf,
                                    in_=x2s_h[ft * 128:(ft + 1) * 128, :])
                ps = fc2ps.tile([128, 4, 512], F32, tag="eps", name="eps_mm")
                ot = otp.tile([128, T], F32, tag="ot", name="ot")
                for ci, (c0, cw) in enumerate(ECHUNKS):
                    for Hkt in range(32):
                        nc.tensor.matmul(
                            ps[:, ci, :cw],
                            lhsT=w2[:, Hkt, :],
                            rhs=gT[:, Hkt, c0:c0 + cw],
                            start=(Hkt == 0), stop=(Hkt == 31))
                    nc.vector.tensor_add(out=ot[:, c0:c0 + cw],
                                         in0=ps[:, ci, :cw],
                                         in1=xf[:, c0:c0 + cw])
                # transpose to token-major + drain + store
                tps = fc2ps.tile([128, 16, 128], F32, tag="eps", name="eps_tp")
                for tt, (t0, p) in enumerate(tok_tiles):
                    nc.tensor.transpose(tps[:p, tt, :], ot[:, t0:t0 + p],
                                        identf[:, :])
                stg = stgp.tile([128, 16, 128], F32, tag="stg", name="stg")
                nc.vector.tensor_copy(out=stg[:, 0:8, :], in_=tps[:, 0:8, :])
                nc.scalar.activation(out=stg[:, 8:12, :], in_=tps[:, 8:12, :],
                                     func=AF.Identity, bias=0.0)
                nc.vector.tensor_copy(out=stg[0:LASTP, 12, :],
                                      in_=tps[0:LASTP, 12, :])
                for tt, (t0, p) in enumerate(tok_tiles):
                    nc.gpsimd.dma_start(
                        out=out_h[t0:t0 + p, ft * 128:(ft + 1) * 128],
                        in_=stg[:p, tt, :])
        fc2w_cm.__exit__(None, None, None)
        gT_cm.__exit__(None, None, None)
        consts_cm.__exit__(None, None, None)
    _split_sync_waits(nc)
    from concourse.library_overlay import lower_extended_insts
    lower_extended_insts(nc)
    return nc


_CACHED_NC = None


def _get_nc():
    global _CACHED_NC
    if _CACHED_NC is None:
        _CACHED_NC = build_program()
    return _CACHED_NC


def prepare_host_inputs(x, qkv_w, q_bias, v_bias, rel_bias_table, proj_w, proj_b,
                        ln1_g, ln1_b, ln2_g, ln2_b, fc1_w, fc1_b, fc2_w, fc2_b):
    bf = ml_dtypes.bfloat16
    f32 = np.float32
    x = np.asarray(x, f32)

    # fold LN1 gamma/beta into qkv weights, scale q by 1/8
    qkv_b = np.concatenate([q_bias, np.zeros_like(v_bias), v_bias]).astype(f32)
    W1 = qkv_w.astype(f32) * ln1_g[None, :].astype(f32)
    b1 = qkv_b + qkv_w.astype(f32) @ ln1_b.astype(f32)
    W1[:D] *= SCALE
    b1[:D] *= SCALE
    qkvwT = np.ascontiguousarray(W1.T).astype(bf)            # [1024, 3072]
    qkb = np.ascontiguousarray(b1[:2 * D].reshape(16, 128).T).astype(f32)
    vb_rep = np.broadcast_to(b1[2 * D:], (128, D)).copy().astype(bf)

    idx = _make_rel_pos_index()
    rel = rel_bias_table.astype(f32)[idx]                    # [q, k, h]
    # expbT[k, par, a, q] = exp(rel[q, k, 2a+par]): exp(S+B) = exp(S)*exp(B)
    ebk = np.exp(rel.transpose(1, 2, 0))                     # [k, h, q]
    expbT = np.ascontiguousarray(
        ebk.reshape(N_TOK, 8, 2, N_TOK).transpose(0, 2, 1, 3)).astype(bf)

    projwT = np.ascontiguousarray(proj_w.astype(f32).T).astype(bf)
    projb = np.ascontiguousarray(proj_b.astype(f32).reshape(8, 128).T)
    fc2b = np.ascontiguousarray(fc2_b.astype(f32).reshape(8, 128).T)

    W3 = fc1_w.astype(f32) * ln2_g[None, :].astype(f32)
    b3 = fc1_b.astype(f32) + fc1_w.astype(f32) @ ln2_b.astype(f32)
    W3T = np.ascontiguousarray(W3.T)                         # [1024, 4096]
    fc1wT = W3T.reshape(8, 128, 32, 128).transpose(2, 1, 0, 3)
    fc1wT = np.ascontiguousarray(fc1wT.reshape(32, 128, D)).astype(bf)
    fc1b = np.ascontiguousarray(b3.reshape(32, 128).T).astype(f32)

    # fc2 packed: fc2wp[ft, p, k, j] = fc2_w[ft*128+j, k*128+p]
    fc2wp = fc2_w.astype(f32).reshape(8, 128, 32, 128)       # [ft, j, k, p]
    fc2wp = np.ascontiguousarray(fc2wp.transpose(0, 3, 2, 1)).astype(bf)

    csel = np.zeros((128, NH, NH), np.float32)
    for h in range(NH):
        csel[:, h, h] = 1.0
    csel = csel.astype(bf)
    psel = np.zeros((16, IC, 128), f32)
    for c in range(IC):
        psel[2 * c, c, 0:64] = 1.0
        psel[2 * c + 1, c, 64:128] = 1.0
    psel = psel.astype(bf)

    shared = dict(qkvwT=qkvwT, qkb=qkb, vb_rep=vb_rep, expbT=expbT,
                  csel=csel, psel=psel,
                  projwT=projwT, projb=projb, fc1wT=fc1wT, fc1b=fc1b,
                  fc2wp=fc2wp, fc2b=fc2b)
    in_maps = []
    for cid in range(NCORES):
        sl = slice(cid * BL, (cid + 1) * BL)
        m = dict(shared)
        xTc = np.ascontiguousarray(x[sl].reshape(T, D).T)
        m["xT"] = xTc
        m["xbT"] = xTc.astype(bf)
        in_maps.append(m)
    return in_maps


def kernel(**inputs):
    nc = _get_nc()
    in_maps = prepare_host_inputs(**inputs)
    res = run_bass_kernel_spmd(nc, in_maps, list(range(NCORES)))
    outs = [res.results[c]["out"].reshape(BL, N_TOK, D) for c in range(NCORES)]
    return np.concatenate(outs, axis=0).astype(np.float32)



# revision 84
# speedup vs baseline: 1.0525x; 1.0000x over previous
"""Trainium2 Bass kernel v2 for the ViT transformer block — feature-major dataflow.

Everything on-chip flows feature-major ([feat, tok]); the host pre-transposes
x to xT and the kernel writes token-major output via cheap PE transposes.
LayerNorm statistics come from ones-vector matmuls (partition-dim reduction on
the PE); the per-token scale/shift rows are broadcast across partitions with
K=1 outer-product matmuls into PSUM and applied with two DVE passes.
Attention keeps the scores^T/exp/ones-column layout of v1, but context tiles
leave the attention phase through PE transposes (identity matmul) instead of
serialized DMA-transposes.  x2 returns to token-major through f32 PE
transposes so the fc2 drain and final residual run exactly like v1.

Sharding: data-parallel over batch, 8 samples per core on 8 cores.
"""

import sys
import os

sys.path.insert(0, "/opt/trn_rl_repo")

import numpy as np
import ml_dtypes

import concourse.bass as bass
import concourse.tile as tile
from concourse import mybir
from concourse import masks
from concourse.vector_clock import ScopedClock
from concourse.bass_utils import run_bass_kernel_spmd

F32 = mybir.dt.float32
BF16 = mybir.dt.bfloat16
AF = mybir.ActivationFunctionType
ALU = mybir.AluOpType

B, N_TOK, D = 64, 197, 1024
NCORES = 8
BL = B // NCORES            # samples per core = 8
T = BL * N_TOK              # tokens per core = 1576
NH, HD = 16, 64
HID = 4096
SCALE = HD ** -0.5
WH = WW = 14
NUM_REL = (2 * WH - 1) * (2 * WW - 1) + 3
LN_EPS = 1e-5

IC = 8                       # in-feature chunks of 128
CHUNKS = [(i * 512, min(512, T - i * 512)) for i in range((T + 511) // 512)]
ECHUNKS = [(i * 394, 394) for i in range(4)]   # uniform fc1/fc2 chunks
KTP = [128, N_TOK - 128]     # per-sample key tile sizes [128, 69]
NT = (T + 127) // 128        # 13 token tiles
LASTP = T - 128 * (NT - 1)   # 40


def _tok_tiles():
    return [(t * 128, 128 if t < NT - 1 else LASTP) for t in range(NT)]


def _sample_tiles():
    out = []
    for b in range(BL):
        for kt in range(2):
            out.append((b, kt, b * N_TOK + kt * 128, KTP[kt]))
    return out


def _make_rel_pos_index():
    coords = np.stack(np.meshgrid(np.arange(WH), np.arange(WW), indexing="ij"))
    flat = coords.reshape(2, -1)
    rel = flat[:, :, None] - flat[:, None, :]
    rel = rel.transpose(1, 2, 0).copy()
    rel[:, :, 0] += WH - 1
    rel[:, :, 1] += WW - 1
    rel[:, :, 0] *= 2 * WW - 1
    idx = np.zeros((N_TOK, N_TOK), dtype=np.int32)
    idx[1:, 1:] = rel.sum(-1)
    idx[0, 0:] = NUM_REL - 3
    idx[0:, 0] = NUM_REL - 2
    idx[0, 0] = NUM_REL - 1
    return idx


class SplitDrainTileContext(tile.TileContext):
    """Walrus in this toolchain rejects >1 sync-wait on the kernel-tail
    Drain; split the waits across a chain of drain instructions."""

    def _drain_and_barrier(self, tick_clock, wait_clock):
        drain_inst = self.nc.sync.drain()
        wait_clock.add_sem_waits(
            drain_inst.ins, ScopedClock({None: tick_clock.global_clock})
        )
        si = drain_inst.ins.sync_info
        waits = list(si.on_wait) if si and si.on_wait else []
        if len(waits) > 1:
            si.on_wait = waits[:1]
            for w in waits[1:]:
                d2 = self.nc.sync.drain()
                si2 = d2.ins.sync_info
                if si2 is None:
                    d2.ins.sync_info = mybir.SyncInfo(on_wait=[w], on_update=[])
                else:
                    si2.on_wait = [w]
        self.nc.all_engine_barrier()
        assert self.sems is not None
        popped = self.nc._tile_sem_poison_stack.pop()
        assert popped is self._sem_poison
        self.nc.clear_and_free_semaphores(list(self.sems.allocated().values()))
        self.nc.all_engine_barrier()


def _split_sync_waits(nc, cap=1):
    """Hoist excess sync-waits onto standalone event-semaphore instructions."""
    n = 0
    for fn in nc.m.functions:
        for bb in fn.blocks:
            insts = bb.instructions
            i = 0
            while i < len(insts):
                inst = insts[i]
                si = inst.sync_info
                waits = list(si.on_wait) if si and si.on_wait else []
                if len(waits) > cap and inst.engine != mybir.EngineType.Unassigned:
                    excess = waits[:len(waits) - cap]
                    si.on_wait = waits[len(waits) - cap:]
                    for w in excess:
                        ev = mybir.InstEventSemaphore(
                            name=f"waitsplit_{n}", ins=[], outs=[],
                            sync_info=mybir.SyncInfo(on_wait=[w], on_update=[]))
                        ev.engine = inst.engine
                        nc.register_instruction(ev)
                        insts.insert(i, ev)
                        n += 1
                        i += 1
                i += 1
    return n


def build_program():
    STOP = int(os.environ.get("K2_STOP", "7"))
    ASUB = os.environ.get("K2_ATTN_SUB", "full")
    NB = int(os.environ.get("K2_NB", str(BL)))
    NOEXP = os.environ.get("K2_NOEXP", "0") == "1"
    NOADD = os.environ.get("K2_NOADD", "0") == "1"
    NG = int(os.environ.get("K2_NG", "8"))
    NGI = int(os.environ.get("K2_NGI", "2"))
    nc = bass.Bass("TRN2", target_bir_lowering=False, debug=False,
                   num_devices=NCORES)

    # ---- DRAM I/O ----
    xT_h = nc.declare_dram_parameter("xT", [D, T], F32, isOutput=False)
    xbT_h = nc.declare_dram_parameter("xbT", [D, T], BF16, isOutput=False)
    qkvwT_h = nc.declare_dram_parameter("qkvwT", [D, 3 * D], BF16, isOutput=False)
    qkb_h = nc.declare_dram_parameter("qkb", [128, 16], F32, isOutput=False)
    vb_h = nc.declare_dram_parameter("vb_rep", [128, D], BF16, isOutput=False)
    expbT_h = nc.declare_dram_parameter("expbT", [N_TOK, 2, 8, N_TOK], BF16,
                                        isOutput=False)
    csel_h = nc.declare_dram_parameter("csel", [128, NH, NH], BF16,
                                       isOutput=False)
    psel_h = nc.declare_dram_parameter("psel", [16, IC, 128], BF16,
                                       isOutput=False)
    projwT_h = nc.declare_dram_parameter("projwT", [D, D], BF16, isOutput=False)
    projb_h = nc.declare_dram_parameter("projb", [128, 8], F32, isOutput=False)
    fc1wT_h = nc.declare_dram_parameter("fc1wT", [32, 128, D], BF16, isOutput=False)
    fc1b_h = nc.declare_dram_parameter("fc1b", [128, 32], F32, isOutput=False)
    fc2wp_h = nc.declare_dram_parameter("fc2wp", [IC, 128, 32, 128], BF16,
                                        isOutput=False)
    fc2b_h = nc.declare_dram_parameter("fc2b", [128, 8], F32, isOutput=False)
    out_h = nc.declare_dram_parameter("out", [T, D], F32, isOutput=True)
    x2s_h = nc.dram_tensor("x2s", [D, T], F32)   # x2 + fc2_b, feature-major

    tok_tiles = _tok_tiles()
    samp_tiles = _sample_tiles()

    with SplitDrainTileContext(nc) as tc:
        # ---------- right-side stack: consts > {ctxT | gT} ----------
        consts_cm = tc.tile_pool(name="consts", bufs=1, side="right")
        consts = consts_cm.__enter__()
        identb = consts.tile([128, 128], BF16, tag="identb", name="identb")
        masks.make_identity(nc, identb[:, :])
        ones_col = consts.tile([128, 1], BF16, tag="ones_col", name="ones_col")
        nc.vector.memset(ones_col, 1.0)
        ones_row = consts.tile([1, 128], BF16, tag="ones_row", name="ones_row")
        nc.vector.memset(ones_row, 1.0)
        qkb_t = consts.tile([128, 16], F32, tag="qkb", name="qkb")
        nc.sync.dma_start(out=qkb_t, in_=qkb_h[:, :])
        vb_t = consts.tile([128, D], BF16, tag="vb", name="vb")
        nc.sync.dma_start(out=vb_t, in_=vb_h[:, :])
        projb_t = consts.tile([128, 8], F32, tag="projb", name="projb")
        nc.sync.dma_start(out=projb_t, in_=projb_h[:, :])
        fc2b_t = consts.tile([128, 8], F32, tag="fc2b", name="fc2b")
        nc.sync.dma_start(out=fc2b_t, in_=fc2b_h[:, :])
        fc1b_t = consts.tile([128, 32], F32, tag="fc1b", name="fc1b")
        nc.sync.dma_start(out=fc1b_t, in_=fc1b_h[:, :])
        epsr = consts.tile([1, 1], F32, tag="epsr", name="epsr")
        nc.vector.memset(epsr, LN_EPS)

        def alloc_rows(pool, sfx):
            return dict(
                r0=pool.tile([1, T], F32, tag=f"row0{sfx}", name=f"row0{sfx}"),
                r1=pool.tile([1, T], F32, tag=f"row1{sfx}", name=f"row1{sfx}"),
                r2=pool.tile([1, T], F32, tag=f"row2{sfx}", name=f"row2{sfx}"),
                abf=pool.tile([1, T], BF16, tag=f"ra{sfx}", name=f"ra{sfx}"),
                bbf=pool.tile([1, T], BF16, tag=f"rb{sfx}", name=f"rb{sfx}"))

        def emit_ln_rows(rows, s1_ap_chunks, s2_ap_chunks):
            """a=rsqrt(var+eps) -> abf; b=mu*a -> bbf (normalize SUBTRACTS b).
            Chunk-pipelined so downstream bc/normalize of chunk 0 starts
            early.  r0..r2 scratch; sources may alias r0/r1."""
            r0, r1, r2 = rows['r0'], rows['r1'], rows['r2']
            abf, bbf = rows['abf'], rows['bbf']
            for ci, (c0, cw) in enumerate(CHUNKS):
                sl = slice(c0, c0 + cw)
                nc.scalar.activation(out=r2[0:1, sl], in_=s1_ap_chunks[ci],
                                     func=AF.Copy, scale=1.0 / D)   # mu
                nc.scalar.activation(out=r0[0:1, sl], in_=s2_ap_chunks[ci],
                                     func=AF.Copy, scale=1.0 / D,
                                     bias=float(LN_EPS))            # m2 + eps
                nc.vector.tensor_mul(out=r1[0:1, sl], in0=r2[0:1, sl],
                                     in1=r2[0:1, sl])
                nc.vector.tensor_sub(out=r0[0:1, sl], in0=r0[0:1, sl],
                                     in1=r1[0:1, sl])               # var + eps
                nc.vector.reciprocal_approx_fast(out=r1[0:1, sl],
                                                 in_=r0[0:1, sl])
                nc.scalar.activation(out=abf[0:1, sl], in_=r1[0:1, sl],
                                     func=AF.Sqrt)                  # rsqrt
                nc.vector.tensor_mul(out=bbf[0:1, sl], in0=r2[0:1, sl],
                                     in1=abf[0:1, sl])

        # ---------- left-side stack: attention superstructure ----------
        qk_cm = tc.tile_pool(name="qkp", bufs=1)
        qkp = qk_cm.__enter__()
        qkt = [qkp.tile([128, T], BF16, tag=f"qkt{ft}", name=f"qkt{ft}")
               for ft in range(16)]
        vt_cm = tc.tile_pool(name="vtp", bufs=1)
        vtp = vt_cm.__enter__()
        vt = {}
        for (b, kt, t0, kp) in samp_tiles:
            vt[(b, kt)] = vtp.tile([128, NH, 65], BF16, tag=f"v{b}_{kt}",
                                   name=f"v{b}_{kt}")
        h1T_cm = tc.tile_pool(name="h1Tp", bufs=1)
        h1Tp = h1T_cm.__enter__()
        h1T = h1Tp.tile([128, IC, T], BF16, tag="h1T", name="h1T")
        wv_cm = tc.tile_pool(name="wvp", bufs=1)
        wvp = wv_cm.__enter__()
        wv = [wvp.tile([128, D], BF16, tag=f"wv{c}", name=f"wv{c}")
              for c in range(IC)]
        wqk_cm = tc.tile_pool(name="wqk", bufs=1)
        wqkp = wqk_cm.__enter__()

        # ---------------- Phase A: load xT, LN1 stats ----------------
        ln1_xb_cm = tc.tile_pool(name="ln1_xb", bufs=1)
        xbp = ln1_xb_cm.__enter__()
        ln1_sq_cm = tc.tile_pool(name="ln1_sq", bufs=2)
        sqp = ln1_sq_cm.__enter__()
        ln1_nrm_cm = tc.tile_pool(name="nrm_tmp", bufs=2)
        nrmp = ln1_nrm_cm.__enter__()

        wqs = [wqkp.tile([128, D], BF16, tag=f"wq{c}", name=f"wq{c}")
               for c in range(IC)]

        rows1_cm = tc.tile_pool(name="rows1", bufs=1)
        rows1 = alloc_rows(rows1_cm.__enter__(), "1")
        abf, bbf = rows1['abf'], rows1['bbf']

        xb = []
        with tc.tile_pool(name="st1_ps", bufs=1, space="PSUM") as st1ps:
            # stat rows live at partition 32*ci so the four chunk-MMs hit
            # distinct PE column groups and run concurrently
            s1p = st1ps.tile([128, 4, 512], F32, tag="s1p", name="s1p")
            s2p = st1ps.tile([128, 4, 512], F32, tag="s2p", name="s2p")
            for c in range(IC):
                xb_cc = []
                for ci, (c0, cw) in enumerate(CHUNKS):
                    xt = xbp.tile([128, 512], BF16, tag=f"xb{c}_{ci}",
                                  name=f"xb{c}_{ci}")
                    nc.sync.dma_start(out=xt[:, :cw],
                                      in_=xbT_h[c * 128:(c + 1) * 128,
                                                c0:c0 + cw])
                    xb_cc.append(xt)
                xb.append(xb_cc)
                for ci, (c0, cw) in enumerate(CHUNKS):
                    rb = 32 * ci
                    xsq = sqp.tile([128, 512], BF16, tag="xsq", name="xsq")
                    nc.vector.tensor_mul(out=xsq[:, :cw],
                                         in0=xb_cc[ci][:, :cw],
                                         in1=xb_cc[ci][:, :cw])
                    nc.tensor.matmul(s1p[rb:rb + 1, ci, :cw],
                                     lhsT=ones_col[:, 0:1],
                                     rhs=xb_cc[ci][:, :cw],
                                     start=(c == 0), stop=(c == IC - 1),
                                     tile_position=(0, rb))
                    nc.tensor.matmul(s2p[rb:rb + 1, ci, :cw],
                                     lhsT=ones_col[:, 0:1],
                                     rhs=xsq[:, :cw],
                                     start=(c == 0), stop=(c == IC - 1),
                                     tile_position=(0, rb))
            for c in range(IC):          # Q weight slabs (after xT on the queue)
                nc.sync.dma_start(out=wqs[c],
                                  in_=qkvwT_h[c * 128:(c + 1) * 128, 0:D])
            for c in range(IC):          # V weight slabs (prefetch)
                nc.sync.dma_start(out=wv[c],
                                  in_=qkvwT_h[c * 128:(c + 1) * 128, 2 * D:3 * D])
            emit_ln_rows(rows1,
                         [s1p[32 * ci:32 * ci + 1, ci, :cw]
                          for ci, (c0, cw) in enumerate(CHUNKS)],
                         [s2p[32 * ci:32 * ci + 1, ci, :cw]
                          for ci, (c0, cw) in enumerate(CHUNKS)])

        # ---------- Phase B: normalize + Q (pipelined), K, V ----------
        with tc.tile_pool(name="bc_ps", bufs=2, space="PSUM") as bcps, \
             tc.tile_pool(name="qk_ps", bufs=4, space="PSUM") as qkps_pool:

            def emit_qk8(ci, base_ft):
                c0, cw = CHUNKS[ci]
                for fi in range(8):
                    ft = base_ft + fi
                    ps = qkps_pool.tile([128, 512], F32, tag="qkps", name="qkps")
                    for c in range(IC):
                        nc.tensor.matmul(
                            ps[:, :cw],
                            lhsT=wqs[c][:, fi * 128:(fi + 1) * 128],
                            rhs=h1T[:, c, c0:c0 + cw],
                            start=(c == 0), stop=(c == IC - 1))
                    if ft % 2 == 0:
                        nc.scalar.activation(out=qkt[ft][:, c0:c0 + cw],
                                             in_=ps[:, :cw], func=AF.Identity,
                                             bias=qkb_t[:, ft:ft + 1])
                    else:
                        nc.vector.tensor_add(
                            out=qkt[ft][:, c0:c0 + cw], in0=ps[:, :cw],
                            in1=qkb_t[:, ft:ft + 1].broadcast_to([128, cw]))

            for ci, (c0, cw) in enumerate(CHUNKS):
                if STOP < 2:
                    break
                bc = bcps.tile([128, 2, 512], F32, tag="bc", name="bc")
                nc.tensor.matmul(bc[:, 0, :cw], lhsT=ones_row[0:1, :],
                                 rhs=abf[0:1, c0:c0 + cw], start=True, stop=True)
                nc.tensor.matmul(bc[:, 1, :cw], lhsT=ones_row[0:1, :],
                                 rhs=bbf[0:1, c0:c0 + cw], start=True, stop=True)
                bcs = nrmp.tile([128, 2, 512], BF16, tag="bcs", name="bcs")
                nc.scalar.activation(out=bcs[:, :, :cw], in_=bc[:, :, :cw],
                                     func=AF.Copy)
                for c in range(IC):
                    tmp = nrmp.tile([128, 512], BF16, tag="ntmp", name="ntmp")
                    nc.vector.tensor_mul(out=tmp[:, :cw], in0=xb[c][ci][:, :cw],
                                         in1=bcs[:, 0, :cw])
                    nc.vector.tensor_sub(out=h1T[:, c, c0:c0 + cw],
                                         in0=tmp[:, :cw], in1=bcs[:, 1, :cw])
                if ci > 0:
                    emit_qk8(ci - 1, 0)
            if STOP >= 2:
                emit_qk8(len(CHUNKS) - 1, 0)

            # K: reload the slab slots (overwrite waits on Q readers)
            if STOP >= 2:
                wks = [wqkp.tile([128, D], BF16, tag=f"wq{c}", name=f"wqk{c}")
                       for c in range(IC)]
                for c in range(IC):
                    nc.sync.dma_start(out=wks[c],
                                      in_=qkvwT_h[c * 128:(c + 1) * 128, D:2 * D])
                wqs = wks

            if STOP >= 2:
                for ci in range(len(CHUNKS)):
                    emit_qk8(ci, 8)

            rows1_cm.__exit__(None, None, None)
            ln1_nrm_cm.__exit__(None, None, None)
            ln1_sq_cm.__exit__(None, None, None)
            ln1_xb_cm.__exit__(None, None, None)
            wqk_cm.__exit__(None, None, None)

        # ---------------- Phase C: attention ----------------
        ctxT_cm = tc.tile_pool(name="ctxTp", bufs=1, side="right")
        ctxTp = ctxT_cm.__enter__()
        ctxT = [ctxTp.tile([128, T], BF16, tag=f"ctxT{k}", name=f"ctxT{k}")
                for k in range(IC)]

        attc_cm = tc.tile_pool(name="attc", bufs=1)
        attc = attc_cm.__enter__()
        eb = []
        for kt in range(2):
            kp = KTP[kt]
            t_ = attc.tile([128, 2, 8, N_TOK], BF16, tag=f"expb{kt}",
                           name=f"expb{kt}")
            nc.sync.dma_start(out=t_[:kp, :, :, :],
                              in_=expbT_h[kt * 128: kt * 128 + kp, :, :, :])
            eb.append(t_)
        # csel[:, h, :]: ones in column h (head-select for denominator MMs)
        csel = attc.tile([128, NH, NH], BF16, tag="csel", name="csel")
        nc.sync.dma_start(out=csel, in_=csel_h[:, :, :])
        # psel[:, c, :]: rec-row pair -> partition halves (rc broadcast)
        psel = attc.tile([16, IC, 128], BF16, tag="psel", name="psel")
        nc.sync.dma_start(out=psel, in_=psel_h[:, :, :])

        with tc.tile_pool(name="p_pool", bufs=4) as ppool, \
             tc.tile_pool(name="praw_pool", bufs=3) as prawp, \
             tc.tile_pool(name="rcb_pool", bufs=2) as rcbp, \
             tc.tile_pool(name="rcs_pool", bufs=3) as rcsp, \
             tc.tile_pool(name="sc_ps", bufs=2, space="PSUM") as scps, \
             tc.tile_pool(name="ctx_ps", bufs=2, space="PSUM") as ctxps, \
             tc.tile_pool(name="s16_ps", bufs=1, space="PSUM") as s16ps, \
             tc.tile_pool(name="rc_ps", bufs=1, space="PSUM") as rcps, \
             tc.tile_pool(name="cproj_ps", bufs=2, space="PSUM") as projps:

            pt = {}

            def emit_ctx(b):
                q0 = b * N_TOK
                # softmax denominators, all 16 heads -> one [16, 197] bank
                s16 = s16ps.tile([16, 256], F32, tag="s16", name="s16")
                n = 0
                for h in range(NH):
                    par, a = h % 2, h // 2
                    for kt in range(2):
                        kp = KTP[kt]
                        nc.tensor.matmul(
                            s16[0:NH, 0:N_TOK],
                            lhsT=csel[:kp, h, :],
                            rhs=pt[(b, kt)][:kp, par, a, :],
                            start=(n == 0), stop=(n == 31))
                        n += 1
                rcb = rcbp.tile([16, 256], F32, tag="rcb", name="rcb")
                nc.vector.reciprocal_approx_fast(out=rcb[0:NH, 0:N_TOK],
                                                 in_=s16[0:NH, 0:N_TOK])
                rcbb = rcbp.tile([16, 256], BF16, tag="rcbb", name="rcbb")
                nc.vector.tensor_copy(out=rcbb[0:NH, 0:N_TOK],
                                      in_=rcb[0:NH, 0:N_TOK])
                for c in range(IC):
                    # heads (2c, 2c+1) column-stacked into one psum bank
                    psc = ctxps.tile([128, 256], F32, tag="ctxps", name="ctxps")
                    for par in range(2):
                        h = 2 * c + par
                        for kt in range(2):
                            kp = KTP[kt]
                            nc.tensor.matmul(
                                psc[par * 64:(par + 1) * 64, 0:N_TOK],
                                lhsT=vt[(b, kt)][:kp, h, 0:64],
                                rhs=pt[(b, kt)][:kp, par, c, :],
                                start=(kt == 0), stop=(kt == 1))
                    rc = rcps.tile([128, 256], F32, tag="rcps2", name="rc")
                    nc.tensor.matmul(rc[0:128, 0:N_TOK],
                                     lhsT=psel[0:NH, c, :],
                                     rhs=rcbb[0:NH, 0:N_TOK],
                                     start=True, stop=True)
                    rcs = rcsp.tile([128, 256], F32, tag="rcs", name="rcs")
                    nc.vector.tensor_copy(out=rcs[:, 0:N_TOK],
                                          in_=rc[:, 0:N_TOK])
                    nc.vector.tensor_mul(out=ctxT[c][:, q0:q0 + N_TOK],
                                         in0=psc[0:128, 0:N_TOK],
                                         in1=rcs[0:128, 0:N_TOK])

            def emit_v(b):
                # V matmuls for sample b: dense K=128 full-array MMs keep the
                # HAM clock gate open during the attention phase.
                for kt in range(2):
                    t0 = b * N_TOK + kt * 128
                    kp = KTP[kt]
                    vtile = vt[(b, kt)]
                    for vc in range(2):
                        ps = projps.tile([128, 512], F32, tag="vps",
                                         name="vps")
                        for c in range(IC):
                            nc.tensor.matmul(
                                ps[:kp, :],
                                lhsT=h1T[:, c, t0:t0 + kp],
                                rhs=wv[c][:, vc * 512:(vc + 1) * 512],
                                start=(c == 0), stop=(c == IC - 1))
                        nc.vector.tensor_add(
                            out=vtile[:kp, vc * 8:(vc + 1) * 8, 0:64],
                            in0=ps[:kp, :].rearrange("p (a d) -> p a d", a=8),
                            in1=vb_t[:kp, vc * 512:(vc + 1) * 512].rearrange(
                                "p (a d) -> p a d", a=8))

            for b in range(BL):
                if STOP < 4:
                    break
                q0 = b * N_TOK
                for kt in range(2):
                    kp = KTP[kt]
                    k0 = q0 + kt * 128
                    ptile = ppool.tile([128, 2, 8, N_TOK], BF16, tag="P",
                                       name="P")
                    pt[(b, kt)] = ptile
                    # pair-tile (par, j) holds heads a=j and a=j+4 (same rb:
                    # mixed lhsT base partitions within one PSUM bank fault).
                    # Emission alternates par so consecutive MMs use opposite
                    # rb and LDWEIGHTS overlaps the in-flight matmul.
                    for j in range(4):
                        pss = [scps.tile([128, 2, 256], F32, tag="scps",
                                         name="scps") for _ in range(2)]
                        for s in range(2):
                            a = j + 4 * s
                            for par in range(2):
                                rb = par * 64
                                nc.tensor.matmul(
                                    pss[par][:kp, s, 0:N_TOK],
                                    lhsT=qkt[8 + a][rb:rb + 64, k0:k0 + kp],
                                    rhs=qkt[a][rb:rb + 64, q0:q0 + N_TOK],
                                    start=True, stop=True)
                        for par in range(2):
                            praw = prawp.tile([128, 2, N_TOK], BF16,
                                              tag="praw", name="praw")
                            nc.scalar.activation(out=praw[:kp, :, :],
                                                 in_=pss[par][:kp, :, 0:N_TOK],
                                                 func=AF.Exp)
                            nc.vector.tensor_mul(
                                out=ptile[:kp, par, j::4, :],
                                in0=praw[:kp, :, :],
                                in1=eb[kt][:kp, par, j::4, :])
                emit_v(b)
                if b > 0:
                    emit_ctx(b - 1)
            if STOP >= 4:
                emit_ctx(BL - 1)

        attc_cm.__exit__(None, None, None)
        wv_cm.__exit__(None, None, None)
        h1T_cm.__exit__(None, None, None)
        vt_cm.__exit__(None, None, None)
        qk_cm.__exit__(None, None, None)

        # ------------- Phase D: proj + residual + LN2 + x2->token-major -------
        fc2w_cm = tc.tile_pool(name="fc2w", bufs=3)
        fc2wsb = fc2w_cm.__enter__()
        fc1w_cm = tc.tile_pool(name="fc1w", bufs=3)
        fc1wp = fc1w_cm.__enter__()
        h2T_cm = tc.tile_pool(name="h2Tp", bufs=1)
        h2Tp = h2T_cm.__enter__()
        h2T = h2Tp.tile([128, IC, T], BF16, tag="h2T", name="h2T")
        xb2_cm = tc.tile_pool(name="xb2p", bufs=1)
        xb2p = xb2_cm.__enter__()

        rows2_cm = tc.tile_pool(name="rows2", bufs=1)
        rows2 = alloc_rows(rows2_cm.__enter__(), "2")
        r0, r1 = rows2['r0'], rows2['r1']
        abf, bbf = rows2['abf'], rows2['bbf']
        nc.vector.memset(r0[0:1, :], 0.0)
        nc.vector.memset(r1[0:1, :], 0.0)

        xb2 = []

        pw_cm = tc.tile_pool(name="pwp", bufs=1)
        pwp = pw_cm.__enter__()
        pw = [pwp.tile([128, D], BF16, tag=f"pw{c}", name=f"pw{c}")
              for c in range(IC)]
        for c in range(IC):
            nc.sync.dma_start(out=pw[c], in_=projwT_h[c * 128:(c + 1) * 128, :])

        with tc.tile_pool(name="xt2", bufs=2) as xt2p, \
             tc.tile_pool(name="x2p", bufs=2) as x2p, \
             tc.tile_pool(name="sq2p", bufs=2) as sq2p, \
             tc.tile_pool(name="proj_ps", bufs=2, space="PSUM") as projps2, \
             tc.tile_pool(name="st2_ps", bufs=4, space="PSUM") as st2ps:
            for ft in range(IC):
                if STOP < 5:
                    break
                xt2 = xt2p.tile([128, T], F32, tag="xt2", name="xt2")
                nc.gpsimd.dma_start(out=xt2, in_=xT_h[ft * 128:(ft + 1) * 128, :])
                x2T = x2p.tile([128, T], F32, tag="x2T", name="x2T")
                for ci, (c0, cw) in enumerate(CHUNKS):
                    ps = projps2.tile([128, 512], F32, tag="projps",
                                      name="projps")
                    for c in range(IC):
                        nc.tensor.matmul(
                            ps[:, :cw],
                            lhsT=pw[c][:, ft * 128:(ft + 1) * 128],
                            rhs=ctxT[c][:, c0:c0 + cw],
                            start=(c == 0), stop=(c == IC - 1))
                    nc.scalar.activation(out=x2T[:, c0:c0 + cw], in_=ps[:, :cw],
                                         func=AF.Identity,
                                         bias=projb_t[:, ft:ft + 1])
                nc.vector.tensor_add(out=x2T, in0=x2T, in1=xt2)
                xb2_f = xb2p.tile([128, T], BF16, tag=f"xb2_{ft}",
                                  name=f"xb2_{ft}")
                nc.vector.tensor_copy(out=xb2_f, in_=x2T)
                xb2.append(xb2_f)
                xsq2 = sq2p.tile([128, T], BF16, tag="xsq2", name="xsq2")
                nc.vector.tensor_mul(out=xsq2, in0=xb2_f, in1=xb2_f)
                for (src_t, accr) in ((xb2_f, r0), (xsq2, r1)):
                    for ci, (c0, cw) in enumerate(CHUNKS):
                        rb = 32 * ci
                        p1 = st2ps.tile([128, 512], F32, tag="st2", name="st2")
                        nc.tensor.matmul(p1[rb:rb + 1, :cw],
                                         lhsT=ones_col[:, 0:1],
                                         rhs=src_t[:, c0:c0 + cw],
                                         start=True, stop=True,
                                         tile_position=(0, rb))
                        nc.vector.tensor_add(out=accr[0:1, c0:c0 + cw],
                                             in0=accr[0:1, c0:c0 + cw],
                                             in1=p1[rb:rb + 1, :cw])
                # x2 + fc2_b -> feature-major DRAM scratch (read back in fc2)
                x2fb = x2p.tile([128, T], F32, tag="x2fb", name="x2fb")
                nc.scalar.activation(out=x2fb, in_=x2T, func=AF.Identity,
                                     bias=fc2b_t[:, ft:ft + 1])
                nc.sync.dma_start(out=x2s_h[ft * 128:(ft + 1) * 128, :],
                                  in_=x2fb)

        ctxT_cm.__exit__(None, None, None)
        pw_cm.__exit__(None, None, None)

        if STOP >= 5:
            emit_ln_rows(rows2,
                         [r0[0:1, c0:c0 + cw] for (c0, cw) in CHUNKS],
                         [r1[0:1, c0:c0 + cw] for (c0, cw) in CHUNKS])

        with tc.tile_pool(name="bc2_ps", bufs=2, space="PSUM") as bc2ps, \
             tc.tile_pool(name="nrm2", bufs=3) as nrm2p:
            for ci, (c0, cw) in enumerate(CHUNKS):
                if STOP < 5:
                    break
                bc = bc2ps.tile([128, 2, 512], F32, tag="bc2", name="bc2")
                nc.tensor.matmul(bc[:, 0, :cw], lhsT=ones_row[0:1, :],
                                 rhs=abf[0:1, c0:c0 + cw], start=True, stop=True)
                nc.tensor.matmul(bc[:, 1, :cw], lhsT=ones_row[0:1, :],
                                 rhs=bbf[0:1, c0:c0 + cw], start=True, stop=True)
                bcs = nrm2p.tile([128, 2, 512], BF16, tag="bcs2", name="bcs2")
                nc.scalar.activation(out=bcs[:, :, :cw], in_=bc[:, :, :cw],
                                     func=AF.Copy)
                for c in range(IC):
                    tmp = nrm2p.tile([128, 512], BF16, tag="n2tmp", name="n2tmp")
                    nc.vector.tensor_mul(out=tmp[:, :cw],
                                         in0=xb2[c][:, c0:c0 + cw],
                                         in1=bcs[:, 0, :cw])
                    nc.vector.tensor_sub(out=h2T[:, c, c0:c0 + cw],
                                         in0=tmp[:, :cw], in1=bcs[:, 1, :cw])
        rows2_cm.__exit__(None, None, None)
        xb2_cm.__exit__(None, None, None)

        # ---------------- Phase E: MLP ----------------
        gT_cm = tc.tile_pool(name="gT_pool", bufs=1, side="right")
        gTp = gT_cm.__enter__()
        gT = gTp.tile([128, 32, T], BF16, tag="gT", name="gT")
        identf = gTp.tile([128, 128], F32, tag="identf", name="identf")
        masks.make_identity(nc, identf[:, :])
        with tc.tile_pool(name="fc1_ps", bufs=4, space="PSUM") as fc1ps:
            for Ht in range(32):
                if STOP < 6:
                    break
                wt = fc1wp.tile([128, D], BF16, tag="fc1w", name="fc1w")
                nc.sync.dma_start(out=wt, in_=fc1wT_h[Ht, :, :])
                for j in range(2):           # super-chunks of 788 = 2x394
                    ps = fc1ps.tile([128, 2, 512], F32, tag="fc1ps",
                                    name="fc1ps")
                    for k in range(2):
                        c0, cw = ECHUNKS[j * 2 + k]
                        for c in range(IC):
                            nc.tensor.matmul(
                                ps[:, k, :cw],
                                lhsT=wt[:, c * 128:(c + 1) * 128],
                                rhs=h2T[:, c, c0:c0 + cw],
                                start=(c == 0), stop=(c == IC - 1))
                    nc.scalar.activation(
                        out=gT[:, Ht, j * 788:(j + 1) * 788],
                        in_=ps[:, :, 0:394],
                        func=AF.Gelu, bias=fc1b_t[:, Ht:Ht + 1])
        h2T_cm.__exit__(None, None, None)
        fc1w_cm.__exit__(None, None, None)

        # ---- fc2 feature-major: out^T[ft, t] = sum_H fc2w^T . gT ----
        with tc.tile_pool(name="xf_sb", bufs=2) as xfp, \
             tc.tile_pool(name="ot_sb", bufs=2) as otp, \
             tc.tile_pool(name="stg_sb", bufs=2) as stgp, \
             tc.tile_pool(name="fc2_ps", bufs=2, space="PSUM") as fc2ps:
            for ft in range(IC):
                if STOP < 7:
                    break
                w2 = fc2wsb.tile([128, 32, 128], BF16, tag="fc2w", name="fc2w")
                nc.sync.dma_start(out=w2, in_=fc2wp_h[ft, :, :, :])
                xf = xfp.tile([128, T], F32, tag="xf", name="xf")
                nc.gpsimd.dma_start(out=xf,
                                    in_=x2s_h[ft * 128:(ft + 1) * 128, :])
                ps = fc2ps.tile([128, 4, 512], F32, tag="eps", name="eps_mm")
                ot = otp.tile([128, T], F32, tag="ot", name="ot")
                for ci, (c0, cw) in enumerate(ECHUNKS):
                    for Hkt in range(32):
                        nc.tensor.matmul(
                            ps[:, ci, :cw],
                            lhsT=w2[:, Hkt, :],
                            rhs=gT[:, Hkt, c0:c0 + cw],
                            start=(Hkt == 0), stop=(Hkt == 31))
                    nc.vector.tensor_add(out=ot[:, c0:c0 + cw],
                                         in0=ps[:, ci, :cw],
                                         in1=xf[:, c0:c0 + cw])
                # transpose to token-major + drain + store
                tps = fc2ps.tile([128, 16, 128], F32, tag="eps", name="eps_tp")
                for tt, (t0, p) in enumerate(tok_tiles):
                    nc.tensor.transpose(tps[:p, tt, :], ot[:, t0:t0 + p],
                                        identf[:, :])
                stg = stgp.tile([128, 16, 128], F32, tag="stg", name="stg")
                nc.vector.tensor_copy(out=stg[:, 0:8, :], in_=tps[:, 0:8, :])
                nc.scalar.activation(out=stg[:, 8:12, :], in_=tps[:, 8:12, :],
                                     func=AF.Identity, bias=0.0)
                nc.vector.tensor_copy(out=stg[0:LASTP, 12, :],
                                      in_=tps[0:LASTP, 12, :])
                for tt, (t0, p) in enumerate(tok_tiles):
                    nc.sync.dma_start(
                        out=out_h[t0:t0 + p, ft * 128:(ft + 1) * 128],
                        in_=stg[:p, tt, :])
        fc2w_cm.__exit__(None, None, None)
        gT_cm.__exit__(None, None, None)
        consts_cm.__exit__(None, None, None)
    _split_sync_waits(nc)
    from concourse.library_overlay import lower_extended_insts
    lower_extended_insts(nc)
    return nc


_CACHED_NC = None


def _get_nc():
    global _CACHED_NC
    if _CACHED_NC is None:
        _CACHED_NC = build_program()
    return _CACHED_NC


def prepare_host_inputs(x, qkv_w, q_bias, v_bias, rel_bias_table, proj_w, proj_b,
                        ln1_g, ln1_b, ln2_g, ln2_b, fc1_w, fc1_b, fc2_w, fc2_b):
    bf = ml_dtypes.bfloat16
    f32 = np.float32
    x = np.asarray(x, f32)

    # fold LN1 gamma/beta into qkv weights, scale q by 1/8
    qkv_b = np.concatenate([q_bias, np.zeros_like(v_bias), v_bias]).astype(f32)
    W1 = qkv_w.astype(f32) * ln1_g[None, :].astype(f32)
    b1 = qkv_b + qkv_w.astype(f32) @ ln1_b.astype(f32)
    W1[:D] *= SCALE
    b1[:D] *= SCALE
    qkvwT = np.ascontiguousarray(W1.T).astype(bf)            # [1024, 3072]
    qkb = np.ascontiguousarray(b1[:2 * D].reshape(16, 128).T).astype(f32)
    vb_rep = np.broadcast_to(b1[2 * D:], (128, D)).copy().astype(bf)

    idx = _make_rel_pos_index()
    rel = rel_bias_table.astype(f32)[idx]                    # [q, k, h]
    # expbT[k, par, a, q] = exp(rel[q, k, 2a+par]): exp(S+B) = exp(S)*exp(B)
    ebk = np.exp(rel.transpose(1, 2, 0))                     # [k, h, q]
    expbT = np.ascontiguousarray(
        ebk.reshape(N_TOK, 8, 2, N_TOK).transpose(0, 2, 1, 3)).astype(bf)

    projwT = np.ascontiguousarray(proj_w.astype(f32).T).astype(bf)
    projb = np.ascontiguousarray(proj_b.astype(f32).reshape(8, 128).T)
    fc2b = np.ascontiguousarray(fc2_b.astype(f32).reshape(8, 128).T)

    W3 = fc1_w.astype(f32) * ln2_g[None, :].astype(f32)
    b3 = fc1_b.astype(f32) + fc1_w.astype(f32) @ ln2_b.astype(f32)
    W3T = np.ascontiguousarray(W3.T)                         # [1024, 4096]
    fc1wT = W3T.reshape(8, 128, 32, 128).transpose(2, 1, 0, 3)
    fc1wT = np.ascontiguousarray(fc1wT.reshape(32, 128, D)).astype(bf)
    fc1b = np.ascontiguousarray(b3.reshape(32, 128).T).astype(f32)

    # fc2 packed: fc2wp[ft, p, k, j] = fc2_w[ft*128+j, k*128+p]
    fc2wp = fc2_w.astype(f32).reshape(8, 128, 32, 128)       # [ft, j, k, p]
    fc2wp = np.ascontiguousarray(fc2wp.transpose(0, 3, 2, 1)).astype(bf)

    csel = np.zeros((128, NH, NH), np.float32)
    for h in range(NH):
        csel[:, h, h] = 1.0
    csel = csel.astype(bf)
    psel = np.zeros((16, IC, 128), f32)
    for c in range(IC):
        psel[2 * c, c, 0:64] = 1.0
        psel[2 * c + 1, c, 64:128] = 1.0
    psel = psel.astype(bf)

    shared = dict(qkvwT=qkvwT, qkb=qkb, vb_rep=vb_rep, expbT=expbT,
                  csel=csel, psel=psel,
                  projwT=projwT, projb=projb, fc1wT=fc1wT, fc1b=fc1b,
                  fc2wp=fc2wp, fc2b=fc2b)
    in_maps = []
    for cid in range(NCORES):
        sl = slice(cid * BL, (cid + 1) * BL)
        m = dict(shared)
        xTc = np.ascontiguousarray(x[sl].reshape(T, D).T)
        m["xT"] = xTc
        m["xbT"] = xTc.astype(bf)
        in_maps.append(m)
    return in_maps


def kernel(**inputs):
    nc = _get_nc()
    in_maps = prepare_host_inputs(**inputs)
    res = run_bass_kernel_spmd(nc, in_maps, list(range(NCORES)))
    outs = [res.results[c]["out"].reshape(BL, N_TOK, D) for c in range(NCORES)]
    return np.concatenate(outs, axis=0).astype(np.float32)

